# revision 20
# baseline (speedup 1.0000x reference)
"""Trainium2 Bass kernel for DirectionAwareMessagePassing (B=8,N=128,R=4096,D=1024).

Sharding: data-parallel over images (1 image per NeuronCore, 8 cores). Weights
replicated. Per image, the math is restructured for the PE array:

  rep^T is kept feature-major on chip (contraction dims must live on SBUF
  partitions). The per-pair coefficient
      coeff_r = (S'[p0_r] * O[p1_r] * u_r) . w  (with u = union @ wu_w)
  is computed without ever transposing union_feats:
      fold w into S:  S' = rep @ (ws_w * w) + ws_b * w
      q_r  = S'[p0_r] * O[p1_r]                  (one-hot gather matmuls)
      g    = Q @ wu_w^T                          (the big [R,D]x[D,D] matmul)
      coeff= rowsum(union * g)                   (DVE fused mul+reduce)
  Gathers are one-hot matmuls (exact selection); the scatter-add into the
  dense [N,N] attention matrix is (P0*coeff)^T @ P1 accumulated in PSUM.

  The coeff/attention path tolerates bf16 (validated: <5e-6 final rel err,
  0 argmax flips), so union_feats / wu / ws / wo / gathers run bf16 at full
  PE rate and union DMA is halved. The rep -> ctx_rep -> scores path stays
  fp32.
"""

import os
import sys
import types
import contextlib

import numpy as np
import ml_dtypes

for _p in ("/opt/trn_rl_repo",):
    if _p not in sys.path:
        sys.path.insert(0, _p)

# ---------------------------------------------------------------------------
# Environment shims for the trimmed trn_rl_repo under axon.
# ---------------------------------------------------------------------------


def _install_shims():
    # 1) antenv.axon_hooks is missing in this image; provide it so
    #    run_bass_kernel_spmd(trace=True) can register the NTFF hook.
    if "antenv.axon_hooks" not in sys.modules:
        _hook = [None]
        mod = types.ModuleType("antenv.axon_hooks")
        mod.set_axon_ntff_profile_hook = lambda h: _hook.__setitem__(0, h)
        mod.get_axon_ntff_profile_hook = lambda: _hook[0]
        sys.modules["antenv.axon_hooks"] = mod
        try:
            from trn_agent_boot.trn_boot import _ntff_profile_via_ctypes

            h = _ntff_profile_via_ctypes("/opt/axon/libaxon_pjrt.so")
            if h is not None:
                mod.set_axon_ntff_profile_hook(h)
        except Exception:
            pass

    # 2) This walrus rejects >1 sync-wait on a CTRL(Drain) instruction, but
    #    Tile's tail drain carries one wait per live semaphore. Split them.
    import concourse.tile as tile
    import concourse.mybir as mybir
    from concourse.vector_clock import ScopedClock

    if getattr(tile.TileContext, "_damp_drain_patched", False):
        return

    def _drain_and_barrier(self, tick_clock, wait_clock):
        nc = self.nc
        drain_inst = nc.sync.drain()
        wait_clock.add_sem_waits(
            drain_inst.ins, ScopedClock({None: tick_clock.global_clock})
        )
        si = drain_inst.ins.sync_info
        waits = list(si.on_wait or []) if si is not None else []
        if len(waits) > 1:
            si.on_wait = waits[:1]
            for w in waits[1:]:
                extra = nc.sync.drain()
                esi = extra.ins.sync_info
                if esi is None:
                    extra.ins.sync_info = mybir.SyncInfo(on_wait=[w], on_update=[])
                else:
                    esi.on_wait = [w]
        nc.all_engine_barrier()
        assert self.sems is not None
        popped = nc._tile_sem_poison_stack.pop()
        assert popped is self._sem_poison
        # clear_and_free_semaphores with a wide range emits a sem_clear ISA
        # whose length this walrus rejects ("ISA wrong length") — clear in
        # chunks of <=4 sems instead.
        sems = list(self.sems.allocated().values())
        if sems:
            sem_nums = [s.num if hasattr(s, "num") else s for s in sems]
            for rg in bass.compact_to_ranges(sem_nums):
                vals = list(rg)
                for i in range(0, len(vals), 4):
                    sub = vals[i:i + 4]
                    subrange = range(sub[0], sub[-1] + 1)
                    nc.gpsimd.dma_reset(subrange)
                    nc.gpsimd.sem_clear(subrange)
            nc._state.prepend_free_semaphores(sem_nums)
            for poison_set in nc._tile_sem_poison_stack:
                poison_set.update(sem_nums)
        nc.all_engine_barrier()

    tile.TileContext._drain_and_barrier = _drain_and_barrier
    tile.TileContext._damp_drain_patched = True


_install_shims()


def _split_multi_waits(nc):
    """This walrus accepts at most one sync-wait per instruction. Move extra
    waits onto freshly inserted same-engine EventSemaphore instructions placed
    immediately before the original (per-engine program order is preserved, so
    semantics are identical)."""
    import concourse.mybir as mybir

    n = 0
    for f in nc.m.functions:
        for bb in f.blocks:
            new = []
            for inst in bb.instructions:
                si = inst.sync_info
                waits = list(si.on_wait) if (si is not None and si.on_wait) else []
                if len(waits) > 1:
                    for w in waits[:-1]:
                        n += 1
                        ev = mybir.InstEventSemaphore(
                            name=f"{inst.name}_sw{n}",
                            opcode="EventSemaphore",
                            engine=inst.engine,
                            sync_info=mybir.SyncInfo(on_wait=[w], on_update=[]),
                        )
                        new.append(ev)
                    si.on_wait = waits[-1:]
                new.append(inst)
            bb.instructions[:] = new
    return n


import concourse.bass as bass  # noqa: E402
import concourse.mybir as mybir  # noqa: E402
import concourse.tile as tile  # noqa: E402
from concourse.bass_utils import run_bass_kernel_spmd  # noqa: E402
from concourse.masks import make_identity  # noqa: E402
from concourse.tile import TileContext  # noqa: E402

F32 = mybir.dt.float32
BF16 = mybir.dt.bfloat16
I32 = mybir.dt.int32
U32 = mybir.dt.uint32
ALU = mybir.AluOpType
ACTF = mybir.ActivationFunctionType

B, N, R, D = 8, 128, 4096, 1024
E_EMB, C, PIN = 200, 151, 9
P = 128
RCHUNKS = R // P            # 32 pair chunks of 128
NGROUPS = 8                 # pair groups of 512 (4 chunks)
GC = RCHUNKS // NGROUPS     # 4 chunks per group
DC = D // P                 # 8 feature chunks
KPROJ = 11                  # padded concat input chunks: 8 obj + 2 emb + 1 pos

last_exec_time_ns = None
last_trace_path = None


# ---------------------------------------------------------------------------
# Program builder
# ---------------------------------------------------------------------------

def _build_program(with_bias_path: bool, with_ln_affine: bool, debug: bool):
    nc = bass.Bass("TRN2", target_bir_lowering=False, debug=False, num_devices=8)

    def inp(name, shape, dt):
        return nc.declare_dram_parameter(name, list(shape), dt, isOutput=False)

    def outp(name, shape, dt):
        return nc.declare_dram_parameter(name, list(shape), dt, isOutput=True)

    d_obj = inp("obj_feats", [P, D], F32)
    d_dists = inp("obj_dists", [P, C], F32)
    d_box = inp("box_info", [P, PIN], F32)
    d_idx = inp("rel_idx", [RCHUNKS, P, 2], I32)
    d_union = inp("union16", [RCHUNKS, P, D], BF16)

    d_pw = inp("pw", [KPROJ, P, D], F32)
    d_pbcol = inp("proj_bcol", [P, DC], F32)
    d_emb = inp("embed_w_p", [2, P, E_EMB], F32)
    d_pew1 = inp("pe_w1f", [PIN, 32], F32)
    d_peb1 = inp("pe_b1col", [32, 1], F32)
    d_pew2 = inp("pe_w2", [32, P], F32)
    d_peb2 = inp("pe_b2col", [P, 1], F32)
    d_ws = inp("ws16", [DC, P, D], BF16)
    d_wsb = inp("wsb16", [1, D], BF16)
    d_wo = inp("wo16", [DC, P, D], BF16)
    d_wob = inp("wob16", [1, D], BF16)
    d_wu = inp("wu16", [DC, P, D], BF16)
    d_wt3 = inp("wt3", [DC, P, D // 2], F32)
    d_wt3b = inp("wt3b", [1, D // 2], F32)
    d_tr1 = inp("tr1", [DC, P, D // 4], F32)
    d_tr1b = inp("tr1b", [1, D // 4], F32)
    d_trg = inp("trg", [1, D // 4], F32)
    d_trbe = inp("trbe", [1, D // 4], F32)
    d_tr2 = inp("tr2", [2, P, D], F32)
    d_tr2b = inp("tr2bcol", [P, DC], F32)
    d_outw = inp("outw", [DC, P, C], F32)
    d_outb = inp("outbcol", [P, 2], F32)
    d_wub = inp("wub16", [1, D], BF16) if with_bias_path else None
    d_wb = inp("w_b_scalar", [1, 1], F32) if with_bias_path else None

    o_scores = outp("scores", [P, C], F32)
    o_preds = outp("preds", [P, 1], I32)
    o_ctx = outp("ctx_rep", [P, D], F32)
    if debug:
        o_dbg_rep = outp("dbg_repT", [P, DC, P], F32)
        o_dbg_an = outp("dbg_an", [P, P], F32)
        o_dbg_coeff = outp("dbg_coeff", [P, RCHUNKS], F32)
        o_dbg_sp = outp("dbg_sp", [P, D], F32)
    coeff_dbg = None

    with TileContext(nc) as tc, contextlib.ExitStack() as ctx:
        wpool = ctx.enter_context(tc.tile_pool(name="wpool", bufs=1))
        respool = ctx.enter_context(tc.tile_pool(name="respool", bufs=1))

        # ---- static tiles -------------------------------------------------
        ident = wpool.tile([P, P], F32)
        make_identity(nc, ident[:])
        ident16 = wpool.tile([P, P], BF16)
        make_identity(nc, ident16[:])

        iota_i32 = wpool.tile([P, P], I32)
        nc.gpsimd.iota(iota_i32[:], pattern=[[1, P]], base=0, channel_multiplier=0)
        iota16 = wpool.tile([P, P], BF16)
        nc.vector.tensor_copy(iota16[:], iota_i32[:])
        if debug:
            coeff_dbg = wpool.tile([P, RCHUNKS], F32)

        # persistent weights
        wu_sb = wpool.tile([P, DC, D], BF16)
        nc.sync.dma_start(out=wu_sb[:], in_=d_wu.rearrange("c p e -> p c e"))
        tr1_sb = respool.tile([P, DC, D // 4], F32)
        nc.sync.dma_start(out=tr1_sb[:], in_=d_tr1.rearrange("c p e -> p c e"))
        tr2_sb = respool.tile([P, 2, D], F32)
        nc.sync.dma_start(out=tr2_sb[:], in_=d_tr2.rearrange("c p e -> p c e"))
        outw_sb = respool.tile([P, DC, C], F32)
        nc.sync.dma_start(out=outw_sb[:], in_=d_outw.rearrange("c p e -> p c e"))
        tr1b_sb = respool.tile([1, D // 4], F32)
        nc.sync.dma_start(out=tr1b_sb[:], in_=d_tr1b[:])
        tr2b_sb = respool.tile([P, DC], F32)
        nc.sync.dma_start(out=tr2b_sb[:], in_=d_tr2b[:])
        outb_sb = respool.tile([P, 2], F32)
        nc.sync.dma_start(out=outb_sb[:], in_=d_outb[:])
        pbcol_sb = wpool.tile([P, DC], F32)
        nc.sync.dma_start(out=pbcol_sb[:], in_=d_pbcol[:])

        ones_f = wpool.tile([1, P], F32)
        nc.gpsimd.memset(ones_f[:], 1.0)
        ones16 = wpool.tile([1, P], BF16)
        nc.gpsimd.memset(ones16[:], 1.0)

        # index prep: [P, RCHUNKS, 2] f32 + per-chunk rows
        idx_i = wpool.tile([P, RCHUNKS, 2], I32)
        nc.sync.dma_start(out=idx_i[:], in_=d_idx.rearrange("c p two -> p c two"))
        idx_f = wpool.tile([P, RCHUNKS, 2], F32)
        nc.vector.tensor_copy(idx_f[:], idx_i[:])

        # outputs of phase 1 kept on chip
        repT = wpool.tile([P, DC, P], F32)
        repT16 = wpool.tile([P, DC, P], BF16)
        S16 = wpool.tile([P, D], BF16)
        O16 = wpool.tile([P, D], BF16)
        W3 = wpool.tile([P, D // 2], F32)
        if with_ln_affine:
            trg_b = wpool.tile([P, D // 4], F32)
            trbe_b = wpool.tile([P, D // 4], F32)
        if with_bias_path:
            ST2 = wpool.tile([P, DC, P], BF16)   # (S' * wu_b)^T
            OT2 = wpool.tile([P, DC, P], BF16)   # O^T

        # =================================================================
        # PHASE 1: rep^T, S', O, W3
        # =================================================================
        with tc.tile_pool(name="ph1", bufs=1) as ph1, \
             tc.tile_pool(name="ph1ps", bufs=2, space="PSUM") as ph1ps, \
             tc.tile_pool(name="pwstream", bufs=3) as pwstream:

            # xT = [obj_feats^T ; obj_embed^T ; pos^T]  [P, KPROJ, P]
            xT = ph1.tile([P, KPROJ, P], F32)
            nc.vector.memset(xT[:], 0.0)

            objf = ph1.tile([P, D], F32)
            nc.sync.dma_start(out=objf[:], in_=d_obj[:])
            for c in range(DC):
                ps = ph1ps.tile([P, P], F32, space="PSUM", tag="tp")
                nc.tensor.transpose(ps[:], objf[:, c * P:(c + 1) * P], ident[:])
                nc.scalar.copy(xT[:, c, :], ps[:])

            # obj_dists^T (2 chunks, second is 23 rows zero-padded)
            dists = ph1.tile([P, C], F32)
            nc.sync.dma_start(out=dists[:], in_=d_dists[:])
            odT = ph1.tile([P, 2, P], F32)
            nc.vector.memset(odT[:], 0.0)
            ps = ph1ps.tile([P, P], F32, space="PSUM", tag="tp")
            nc.tensor.transpose(ps[:], dists[:, 0:P], ident[:])
            nc.scalar.copy(odT[:, 0, :], ps[:])
            ps = ph1ps.tile([P, P], F32, space="PSUM", tag="tp")
            nc.tensor.transpose(ps[:23, :], dists[:, P:C], ident[:])
            nc.scalar.copy(odT[:23, 1, :], ps[:23, :])

            # obj_embed^T via embed_w (c chunks padded to 128)
            emb_sb = ph1.tile([P, 2, E_EMB], F32)
            nc.sync.dma_start(out=emb_sb[:], in_=d_emb.rearrange("c p e -> p c e"))
            eps = ph1ps.tile([P, P], F32, space="PSUM", tag="mm")
            for kc in range(2):
                nc.tensor.matmul(eps[:, :], emb_sb[:, kc, 0:P], odT[:, kc, :],
                                 start=(kc == 0), stop=(kc == 1))
            nc.scalar.copy(xT[:, 8, :], eps[:])
            eps = ph1ps.tile([P, P], F32, space="PSUM", tag="mm")
            for kc in range(2):
                nc.tensor.matmul(eps[:72, :], emb_sb[:, kc, P:E_EMB], odT[:, kc, :],
                                 start=(kc == 0), stop=(kc == 1))
            nc.scalar.copy(xT[:72, 9, :], eps[:72, :])

            # pos^T: box^T -> h^T(32) -> pos^T(128) with BN folded on host
            box = ph1.tile([P, PIN], F32)
            nc.sync.dma_start(out=box[:], in_=d_box[:])
            bps = ph1ps.tile([P, P], F32, space="PSUM", tag="tp")
            nc.tensor.transpose(bps[:PIN, :], box[:], ident[:])
            boxT = ph1.tile([PIN, P], F32)
            nc.vector.tensor_copy(boxT[:], bps[:PIN, :])
            pew1 = ph1.tile([PIN, 32], F32)
            nc.sync.dma_start(out=pew1[:], in_=d_pew1[:])
            peb1 = ph1.tile([32, 1], F32)
            nc.sync.dma_start(out=peb1[:], in_=d_peb1[:])
            pew2 = ph1.tile([32, P], F32)
            nc.sync.dma_start(out=pew2[:], in_=d_pew2[:])
            peb2 = ph1.tile([P, 1], F32)
            nc.sync.dma_start(out=peb2[:], in_=d_peb2[:])
            hps = ph1ps.tile([P, P], F32, space="PSUM", tag="mm")
            nc.tensor.matmul(hps[:32, :], pew1[:], boxT[:], start=True, stop=True)
            hT = ph1.tile([32, P], F32)
            nc.scalar.activation(hT[:], hps[:32, :], ACTF.Identity, bias=peb1[:])
            pps = ph1ps.tile([P, P], F32, space="PSUM", tag="mm")
            nc.tensor.matmul(pps[:], pew2[:], hT[:], start=True, stop=True)
            nc.scalar.activation(xT[:, 10, :], pps[:], ACTF.Relu, bias=peb2[:])

            # rep^T: one PSUM bank per dout chunk (start=True clears the whole
            # bank, so regions must not share one), all proj chunks resident
            pwc = pwstream.tile([P, KPROJ, D], F32, tag="pw", bufs=1)
            nc.sync.dma_start(out=pwc[:], in_=d_pw.rearrange("c p o -> p c o"))
            for m in range(DC):
                rps = ph1ps.tile([P, P], F32, space="PSUM", tag="mm")
                for kc in range(KPROJ):
                    nc.tensor.matmul(rps[:], pwc[:, kc, m * P:(m + 1) * P],
                                     xT[:, kc, :],
                                     start=(kc == 0), stop=(kc == KPROJ - 1))
                nc.scalar.activation(repT[:, m, :], rps[:], ACTF.Relu,
                                     bias=pbcol_sb[:, m:m + 1])
            nc.vector.tensor_copy(repT16[:], repT[:])
            if debug:
                nc.sync.dma_start(out=o_dbg_rep[:], in_=repT[:])

            # S' = rep @ ws' + b  (bf16, w_w folded);  O likewise;  W3 fp32
            ws_sb = ph1.tile([P, DC, D], BF16)
            nc.sync.dma_start(out=ws_sb[:], in_=d_ws.rearrange("c p e -> p c e"))
            wo_sb = ph1.tile([P, DC, D], BF16)
            nc.sync.dma_start(out=wo_sb[:], in_=d_wo.rearrange("c p e -> p c e"))
            wsb = ph1.tile([1, D], BF16)
            nc.sync.dma_start(out=wsb[:], in_=d_wsb[:])
            wob = ph1.tile([1, D], BF16)
            nc.sync.dma_start(out=wob[:], in_=d_wob[:])

            for (wmat, wbias, dst) in ((ws_sb, wsb, S16), (wo_sb, wob, O16)):
                for h in range(2):
                    sps = ph1ps.tile([P, 512], F32, space="PSUM", tag="so")
                    for kc in range(DC):
                        nc.tensor.matmul(sps[:], repT16[:, kc, :],
                                         wmat[:, kc, h * 512:(h + 1) * 512],
                                         start=(kc == 0), stop=False)
                    nc.tensor.matmul(sps[:], ones16[:],
                                     wbias[:, h * 512:(h + 1) * 512],
                                     start=False, stop=True)
                    nc.vector.tensor_copy(dst[:, h * 512:(h + 1) * 512], sps[:])
            if debug:
                sdbg = ph1.tile([P, D], F32)
                nc.vector.tensor_copy(sdbg[:], S16[:])
                nc.sync.dma_start(out=o_dbg_sp[:], in_=sdbg[:])

            wt3_sb = ph1.tile([P, DC, D // 2], F32)
            nc.sync.dma_start(out=wt3_sb[:], in_=d_wt3.rearrange("c p e -> p c e"))
            wt3b_sb = ph1.tile([1, D // 2], F32)
            nc.sync.dma_start(out=wt3b_sb[:], in_=d_wt3b[:])
            wps = ph1ps.tile([P, 512], F32, space="PSUM", tag="so")
            for kc in range(DC):
                nc.tensor.matmul(wps[:], repT[:, kc, :], wt3_sb[:, kc, :],
                                 start=(kc == 0), stop=False)
            nc.tensor.matmul(wps[:], ones_f[:], wt3b_sb[:], start=False, stop=True)
            nc.scalar.activation(W3[:], wps[:], ACTF.Relu)

            if with_ln_affine:
                trg_row = ph1.tile([1, D // 4], F32)
                nc.sync.dma_start(out=trg_row[:], in_=d_trg[:])
                trbe_row = ph1.tile([1, D // 4], F32)
                nc.sync.dma_start(out=trbe_row[:], in_=d_trbe[:])
                gps = ph1ps.tile([P, 512], F32, space="PSUM", tag="so")
                nc.tensor.matmul(gps[:, :D // 4], ones_f[:], trg_row[:],
                                 start=True, stop=True)
                nc.vector.tensor_copy(trg_b[:], gps[:, :D // 4])
                gps = ph1ps.tile([P, 512], F32, space="PSUM", tag="so")
                nc.tensor.matmul(gps[:, :D // 4], ones_f[:], trbe_row[:],
                                 start=True, stop=True)
                nc.vector.tensor_copy(trbe_b[:], gps[:, :D // 4])

            if with_bias_path:
                # T[i,j] = sum_d (S'*wu_b)[i,d] * O[j,d] needs transposed copies
                wub = ph1.tile([1, D], BF16)
                nc.sync.dma_start(out=wub[:], in_=d_wub[:])
                wub_b = ph1.tile([P, D], BF16)
                bps2 = ph1ps.tile([P, 512], F32, space="PSUM", tag="so")
                for h in range(2):
                    nc.tensor.matmul(bps2[:], ones16[:], wub[:, h * 512:(h + 1) * 512],
                                     start=True, stop=True)
                    nc.vector.tensor_copy(wub_b[:, h * 512:(h + 1) * 512], bps2[:])
                s2 = ph1.tile([P, D], BF16)
                nc.vector.tensor_tensor(s2[:], S16[:], wub_b[:], op=ALU.mult)
                for c in range(DC):
                    tps = ph1ps.tile([P, P], BF16, space="PSUM", tag="tp")
                    nc.tensor.transpose(tps[:], s2[:, c * P:(c + 1) * P], ident16[:])
                    nc.vector.tensor_copy(ST2[:, c, :], tps[:])
                    tps = ph1ps.tile([P, P], BF16, space="PSUM", tag="tp")
                    nc.tensor.transpose(tps[:], O16[:, c * P:(c + 1) * P], ident16[:])
                    nc.vector.tensor_copy(OT2[:, c, :], tps[:])

        # =================================================================
        # PHASE 2: pair loop — gathers, g = Q @ wu^T, coeff, scatter into A
        # =================================================================
        # A and Count in separate PSUM banks (start=True clears a whole bank)
        apool = ctx.enter_context(tc.tile_pool(name="apool", bufs=1, space="PSUM"))
        a_acc = apool.tile([P, P], F32, space="PSUM")
        cnt_acc = (apool.tile([P, P], F32, space="PSUM", name="cnt_acc")
                   if with_bias_path else None)

        with tc.tile_pool(name="p2", bufs=2) as p2, \
             tc.tile_pool(name="p2ps", bufs=4, space="PSUM") as p2ps, \
             tc.tile_pool(name="p2tp", bufs=2, space="PSUM") as p2tp:

            for g in range(NGROUPS):
                union_sb = p2.tile([P, GC, D], BF16, tag="union")
                nc.sync.dma_start(
                    out=union_sb[:],
                    in_=d_union[g * GC:(g + 1) * GC].rearrange("c p e -> p c e"))

                # one-hot selectors for this group's 512 pairs
                p0t = p2.tile([P, GC * P], BF16, tag="p0t")
                p1t = p2.tile([P, GC * P], BF16, tag="p1t")
                pmats = []  # per chunk: (p0m, p1m) [r,i] one-hots
                for cc in range(GC):
                    ch = g * GC + cc
                    p0col = idx_f[:, ch, 0:1]
                    p1col = idx_f[:, ch, 1:2]
                    # one-hots for all GC chunks of a group stay live until the
                    # group's scatters: bufs must cover 2 groups' worth
                    p0m = p2.tile([P, P], BF16, tag="p0m", bufs=2 * GC + 2)
                    nc.vector.tensor_single_scalar(p0m[:], iota16[:], p0col,
                                                   op=ALU.is_equal)
                    p1m = p2.tile([P, P], BF16, tag="p1m", bufs=2 * GC + 2)
                    nc.vector.tensor_single_scalar(p1m[:], iota16[:], p1col,
                                                   op=ALU.is_equal)
                    tp = p2tp.tile([P, P], BF16, space="PSUM", tag="tp")
                    nc.tensor.transpose(tp[:], p0m[:], ident16[:])
                    nc.vector.tensor_copy(p0t[:, cc * P:(cc + 1) * P], tp[:])
                    tp = p2tp.tile([P, P], BF16, space="PSUM", tag="tp")
                    nc.tensor.transpose(tp[:], p1m[:], ident16[:])
                    nc.vector.tensor_copy(p1t[:, cc * P:(cc + 1) * P], tp[:])
                    pmats.append((p0m, p1m))

                # gathers: sT/oT [d-chunk, 512] then QT = sT*oT (bf16).
                # DVE can read only one PSUM operand; bounce sT via ACT copy.
                qt = p2.tile([P, DC, GC * P], BF16, tag="qt")
                for dcc in range(DC):
                    sps = p2ps.tile([P, 512], F32, space="PSUM", tag="big")
                    nc.tensor.matmul(sps[:], S16[:, dcc * P:(dcc + 1) * P], p0t[:],
                                     start=True, stop=True)
                    ops_ = p2ps.tile([P, 512], F32, space="PSUM", tag="big")
                    nc.tensor.matmul(ops_[:], O16[:, dcc * P:(dcc + 1) * P], p1t[:],
                                     start=True, stop=True)
                    s_sb = p2.tile([P, 512], BF16, tag="s_sb")
                    nc.scalar.copy(s_sb[:], sps[:])
                    nc.vector.tensor_tensor(qt[:, dcc, :], s_sb[:], ops_[:],
                                            op=ALU.mult)

                # g = Q @ wu^T per 128-pair chunk; coeff = rowsum(union * g)
                for cc in range(GC):
                    ch = g * GC + cc
                    gps0 = p2ps.tile([P, 512], F32, space="PSUM", tag="big")
                    gps1 = p2ps.tile([P, 512], F32, space="PSUM", tag="big")
                    for dcc in range(DC):
                        lhs = qt[:, dcc, cc * P:(cc + 1) * P]
                        nc.tensor.matmul(gps0[:], lhs, wu_sb[:, dcc, 0:512],
                                         start=(dcc == 0), stop=(dcc == DC - 1))
                        nc.tensor.matmul(gps1[:], lhs, wu_sb[:, dcc, 512:1024],
                                         start=(dcc == 0), stop=(dcc == DC - 1))
                    # fused mul + row-sum via TensorScalarPtr accum_out
                    # (tensor_tensor_reduce is an ISA op this walrus rejects)
                    c0 = p2.tile([P, 1], F32, tag="c0")
                    scr = p2.tile([P, 512], F32, tag="scr")
                    nc.vector.scalar_tensor_tensor(
                        out=scr[:], in0=union_sb[:, cc, 0:512], scalar=1.0,
                        in1=gps0[:], op0=ALU.bypass, op1=ALU.mult,
                        accum_out=c0[:])
                    c1 = p2.tile([P, 1], F32, tag="c1")
                    scr2 = p2.tile([P, 512], F32, tag="scr")
                    nc.vector.scalar_tensor_tensor(
                        out=scr2[:], in0=union_sb[:, cc, 512:1024], scalar=1.0,
                        in1=gps1[:], op0=ALU.bypass, op1=ALU.mult,
                        accum_out=c1[:])
                    coeff = p2.tile([P, 1], F32, tag="coeff")
                    nc.vector.tensor_tensor(coeff[:], c0[:], c1[:], op=ALU.add)
                    if debug:
                        nc.vector.tensor_copy(coeff_dbg[:, ch:ch + 1], coeff[:])
                        if ch == RCHUNKS - 1:
                            nc.sync.dma_start(out=o_dbg_coeff[:], in_=coeff_dbg[:])

                    # scatter: A += (P0*coeff)^T @ P1  (and Count += P0^T @ P1)
                    p0m, p1m = pmats[cc]
                    p0c = p2.tile([P, P], BF16, tag="p0c")
                    nc.vector.tensor_tensor(p0c[:], p0m[:],
                                            coeff[:].to_broadcast([P, P]),
                                            op=ALU.mult)
                    first = (ch == 0)
                    last = (ch == RCHUNKS - 1)
                    nc.tensor.matmul(a_acc[:], p0c[:], p1m[:],
                                     start=first, stop=last)
                    if with_bias_path:
                        nc.tensor.matmul(cnt_acc[:], p0m[:], p1m[:],
                                         start=first, stop=last)

        # =================================================================
        # PHASE 3: A -> ctx -> nb -> ctx_rep -> scores/preds
        # =================================================================
        with tc.tile_pool(name="p3", bufs=1) as p3, \
             tc.tile_pool(name="p3ps", bufs=2, space="PSUM") as p3ps:

            if with_bias_path:
                # T^T[j,i] = sum_d O^T[d,j] (S'wu_b)^T[d,i]; A += Count*(T + w_b)
                tt_ps = p3ps.tile([P, P], F32, space="PSUM", tag="mm")
                for dcc in range(DC):
                    nc.tensor.matmul(tt_ps[:], OT2[:, dcc, :], ST2[:, dcc, :],
                                     start=(dcc == 0), stop=(dcc == DC - 1))
                ttsb = p3.tile([P, P], F32)
                nc.vector.tensor_copy(ttsb[:], tt_ps[:])
                t_ps = p3ps.tile([P, P], F32, space="PSUM", tag="mm")
                nc.tensor.transpose(t_ps[:], ttsb[:], ident[:])
                wbsc = p3.tile([1, 1], F32)
                nc.sync.dma_start(out=wbsc[:], in_=d_wb[:])
                wbcol = p3.tile([P, 1], F32)
                bb = p3ps.tile([P, P], F32, space="PSUM", tag="mm2")
                nc.tensor.matmul(bb[:, 0:1], ones_f[:], wbsc[:], start=True, stop=True)
                nc.vector.tensor_copy(wbcol[:], bb[:, 0:1])
                tpw = p3.tile([P, P], F32)
                nc.vector.tensor_scalar_add(tpw[:], t_ps[:], wbcol[:])
                cnt_term = p3.tile([P, P], F32)
                nc.vector.tensor_tensor(cnt_term[:], cnt_acc[:], tpw[:],
                                        op=ALU.mult)
                apre = p3.tile([P, P], F32)
                nc.vector.tensor_tensor(apre[:], a_acc[:], cnt_term[:],
                                        op=ALU.add)
                asig = p3.tile([P, P], F32)
                nc.scalar.activation(asig[:], apre[:], ACTF.Sigmoid)
            else:
                asig = p3.tile([P, P], F32)
                nc.scalar.activation(asig[:], a_acc[:], ACTF.Sigmoid)

            # zero diagonal, row-normalize
            nc.gpsimd.affine_select(out=asig[:], in_=asig[:],
                                    compare_op=ALU.not_equal, fill=0.0,
                                    base=0, pattern=[[-1, P]], channel_multiplier=1)
            rs = p3.tile([P, 1], F32)
            nc.vector.reduce_sum(rs[:], asig[:], axis=mybir.AxisListType.X)
            rsr = p3.tile([P, 1], F32)
            nc.vector.reciprocal(rsr[:], rs[:])
            an = p3.tile([P, P], F32)
            nc.vector.tensor_scalar_mul(an[:], asig[:], rsr[:])
            if debug:
                nc.sync.dma_start(out=o_dbg_an[:], in_=an[:])
            antp = p3ps.tile([P, P], F32, space="PSUM", tag="mm")
            nc.tensor.transpose(antp[:], an[:], ident[:])
            anT = p3.tile([P, P], F32)
            nc.vector.tensor_copy(anT[:], antp[:])

            # ctx^T [d2, i]: first 512 rows from (An@W3)^T, last 512 from (An^T@W3)^T
            ctxT = p3.tile([P, DC, P], F32)
            for m4 in range(4):
                cps = p3ps.tile([P, P], F32, space="PSUM", tag="mm")
                nc.tensor.matmul(cps[:], W3[:, m4 * P:(m4 + 1) * P], anT[:],
                                 start=True, stop=True)
                nc.scalar.copy(ctxT[:, m4, :], cps[:])
            for m4 in range(4):
                cps = p3ps.tile([P, P], F32, space="PSUM", tag="mm")
                nc.tensor.matmul(cps[:], W3[:, m4 * P:(m4 + 1) * P], an[:],
                                 start=True, stop=True)
                nc.scalar.copy(ctxT[:, 4 + m4, :], cps[:])

            # t = ctx @ tr1 + b; LayerNorm; relu; transpose
            tps_ = p3ps.tile([P, 256], F32, space="PSUM", tag="mm")
            for dcc in range(DC):
                nc.tensor.matmul(tps_[:], ctxT[:, dcc, :], tr1_sb[:, dcc, :],
                                 start=(dcc == 0), stop=False)
            nc.tensor.matmul(tps_[:], ones_f[:], tr1b_sb[:], start=False, stop=True)
            stats = p3.tile([P, 6], F32)
            nc.vector.bn_stats(stats[:], tps_[:])
            aggr = p3.tile([P, 2], F32)
            nc.vector.bn_aggr(aggr[:], stats[:])
            veps = p3.tile([P, 1], F32)
            nc.vector.tensor_scalar_add(veps[:], aggr[:, 1:2], 1e-5)
            stdv = p3.tile([P, 1], F32)
            nc.scalar.sqrt(stdv[:], veps[:])
            rstd = p3.tile([P, 1], F32)
            nc.vector.reciprocal(rstd[:], stdv[:])
            tn = p3.tile([P, 256], F32)
            nc.vector.tensor_scalar(tn[:], tps_[:], aggr[:, 0:1], rstd[:],
                                    op0=ALU.subtract, op1=ALU.mult)
            if with_ln_affine:
                nc.vector.tensor_tensor(tn[:], tn[:], trg_b[:], op=ALU.mult)
                nc.vector.tensor_tensor(tn[:], tn[:], trbe_b[:], op=ALU.add)
            trl = p3.tile([P, 256], F32)
            nc.scalar.activation(trl[:], tn[:], ACTF.Relu)
            rT = p3.tile([P, 2, P], F32)
            for kc in range(2):
                tp2 = p3ps.tile([P, P], F32, space="PSUM", tag="mm")
                nc.tensor.transpose(tp2[:], trl[:, kc * P:(kc + 1) * P], ident[:])
                nc.vector.tensor_copy(rT[:, kc, :], tp2[:])

            # nb^T per d-chunk; ctx_rep^T = relu(rep^T + nb^T + b2)
            ctxrT = p3.tile([P, DC, P], F32)
            for dcc in range(DC):
                nps = p3ps.tile([P, P], F32, space="PSUM", tag="mm")
                for kc in range(2):
                    nc.tensor.matmul(nps[:], tr2_sb[:, kc, dcc * P:(dcc + 1) * P],
                                     rT[:, kc, :], start=(kc == 0), stop=(kc == 1))
                tmp = p3.tile([P, P], F32, tag="nbtmp")
                nc.vector.scalar_tensor_tensor(
                    out=tmp[:], in0=nps[:], scalar=tr2b_sb[:, dcc:dcc + 1],
                    in1=repT[:, dcc, :], op0=ALU.add, op1=ALU.add)
                nc.scalar.activation(ctxrT[:, dcc, :], tmp[:], ACTF.Relu)

            # ctx_rep output (transpose back)
            ctx_sb = p3.tile([P, D], F32)
            for dcc in range(DC):
                cps2 = p3ps.tile([P, P], F32, space="PSUM", tag="mm")
                nc.tensor.transpose(cps2[:], ctxrT[:, dcc, :], ident[:])
                nc.scalar.copy(ctx_sb[:, dcc * P:(dcc + 1) * P], cps2[:])
            nc.sync.dma_start(out=o_ctx[:], in_=ctx_sb[:])

            # scores^T then scores
            scT = p3.tile([P, 2, P], F32)
            for mc in range(2):
                mwid = P if mc == 0 else C - P
                sps2 = p3ps.tile([P, P], F32, space="PSUM", tag="mm")
                for dcc in range(DC):
                    nc.tensor.matmul(sps2[:mwid, :],
                                     outw_sb[:, dcc, mc * P:mc * P + mwid],
                                     ctxrT[:, dcc, :],
                                     start=(dcc == 0), stop=(dcc == DC - 1))
                nc.scalar.activation(scT[:mwid, mc, :], sps2[:mwid, :],
                                     ACTF.Identity, bias=outb_sb[:mwid, mc:mc + 1])
            scores_sb = p3.tile([P, C], F32)
            sps3 = p3ps.tile([P, P], F32, space="PSUM", tag="mm")
            nc.tensor.transpose(sps3[:], scT[:, 0, :], ident[:])
            nc.vector.tensor_copy(scores_sb[:, 0:P], sps3[:])
            sps3 = p3ps.tile([P, P], F32, space="PSUM", tag="mm")
            nc.tensor.transpose(sps3[:, 0:C - P], scT[:C - P, 1, :],
                                ident[:C - P, 0:C - P])
            nc.vector.tensor_copy(scores_sb[:, P:C], sps3[:, 0:C - P])
            nc.sync.dma_start(out=o_scores[:], in_=scores_sb[:])

            # preds = argmax(scores[:,1:]) + 1
            mx8 = p3.tile([P, 8], F32)
            nc.vector.max(mx8[:], scores_sb[:, 1:C])
            mi8 = p3.tile([P, 8], U32)
            nc.vector.max_index(mi8[:], mx8[:], scores_sb[:, 1:C])
            predf = p3.tile([P, 1], I32)
            nc.vector.tensor_single_scalar(predf[:], mi8[:, 0:1], 1, op=ALU.add)
            nc.sync.dma_start(out=o_preds[:], in_=predf[:])

    nsplit = _split_multi_waits(nc)
    if nsplit:
        print(f"[kernel] split {nsplit} extra sync-waits into EventSemaphores")
    return nc


# ---------------------------------------------------------------------------
# Host-side input prep + execution
# ---------------------------------------------------------------------------

_cache = {}


def _prep_weights(inputs):
    f32 = np.float32
    bf16 = ml_dtypes.bfloat16
    w = np.asarray(inputs["w_w"], f32)[:, 0]

    alpha = (np.asarray(inputs["pe_g"], f32) / np.sqrt(np.float32(1.0 + 1e-5)))
    pe_w1f = np.asarray(inputs["pe_w1"], f32) * alpha[None, :]
    pe_b1col = (np.asarray(inputs["pe_b1"], f32) * alpha
                + np.asarray(inputs["pe_be"], f32))[:, None].copy()

    proj_w = np.asarray(inputs["proj_w"], f32)
    pw = np.zeros((KPROJ, P, D), f32)
    pw.reshape(KPROJ * P, D)[0:D] = proj_w[0:D]
    pw.reshape(KPROJ * P, D)[D:D + P] = proj_w[D:D + P]
    pw.reshape(KPROJ * P, D)[9 * P + 0: 9 * P + (E_EMB - P)] = proj_w[D + P:D + E_EMB]
    pw.reshape(KPROJ * P, D)[10 * P:11 * P] = proj_w[D + E_EMB:]

    emb = np.asarray(inputs["embed_w"], f32)
    emb_p = np.zeros((2, P, E_EMB), f32)
    emb_p[0] = emb[0:P]
    emb_p[1, :C - P] = emb[P:C]

    ws16 = (np.asarray(inputs["ws_w"], f32) * w[None, :]).astype(bf16).reshape(DC, P, D)
    wsb16 = (np.asarray(inputs["ws_b"], f32) * w).astype(bf16)[None, :]
    wo16 = np.asarray(inputs["wo_w"], f32).astype(bf16).reshape(DC, P, D)
    wob16 = np.asarray(inputs["wo_b"], f32).astype(bf16)[None, :]
    wu16 = np.ascontiguousarray(np.asarray(inputs["wu_w"], f32).T).astype(bf16)
    wu16 = wu16.reshape(DC, P, D)

    out_b = np.asarray(inputs["out_b"], f32)
    outbcol = np.zeros((P, 2), f32)
    outbcol[:, 0] = out_b[0:P]
    outbcol[:C - P, 1] = out_b[P:C]

    return {
        "pw": pw,
        "proj_bcol": np.ascontiguousarray(
            np.asarray(inputs["proj_b"], f32).reshape(DC, P).T),
        "embed_w_p": emb_p,
        "pe_w1f": pe_w1f,
        "pe_b1col": pe_b1col,
        "pe_w2": np.asarray(inputs["pe_w2"], f32),
        "pe_b2col": np.asarray(inputs["pe_b2"], f32)[:, None].copy(),
        "ws16": ws16, "wsb16": wsb16,
        "wo16": wo16, "wob16": wob16,
        "wu16": wu16,
        "wt3": np.asarray(inputs["wt3_w"], f32).reshape(DC, P, D // 2),
        "wt3b": np.asarray(inputs["wt3_b"], f32)[None, :],
        "tr1": np.asarray(inputs["tr_w1"], f32).reshape(DC, P, D // 4),
        "tr1b": np.asarray(inputs["tr_b1"], f32)[None, :],
        "trg": np.asarray(inputs["tr_g"], f32)[None, :],
        "trbe": np.asarray(inputs["tr_be"], f32)[None, :],
        "tr2": np.asarray(inputs["tr_w2"], f32).reshape(2, P, D),
        "tr2bcol": np.ascontiguousarray(
            np.asarray(inputs["tr_b2"], f32).reshape(DC, P).T),
        "outw": np.asarray(inputs["out_w"], f32).reshape(DC, P, C),
        "outbcol": outbcol,
    }


def kernel(**inputs):
    global last_exec_time_ns, last_trace_path

    f32 = np.float32
    bf16 = ml_dtypes.bfloat16

    wu_b = np.asarray(inputs["wu_b"], f32)
    w_b = np.asarray(inputs["w_b"], f32)
    with_bias_path = bool(np.any(wu_b != 0) or np.any(w_b != 0))
    with_ln_affine = bool(
        np.any(np.asarray(inputs["tr_g"], f32) != 1.0)
        or np.any(np.asarray(inputs["tr_be"], f32) != 0.0))
    debug = bool(int(os.environ.get("DAMP_DEBUG", "0")))
    trace = bool(int(os.environ.get("DAMP_TRACE", "0")))

    key = (with_bias_path, with_ln_affine, debug)
    if key not in _cache:
        _cache[key] = _build_program(with_bias_path, with_ln_affine, debug)
    nc = _cache[key]

    weights = _prep_weights(inputs)
    if with_bias_path:
        weights["wub16"] = wu_b.astype(bf16)[None, :]
        weights["w_b_scalar"] = w_b.reshape(1, 1)

    obj_feats = np.asarray(inputs["obj_feats"], f32)
    obj_dists = np.asarray(inputs["obj_dists"], f32)
    box_info = np.asarray(inputs["box_info"], f32)
    rel_idx = np.asarray(inputs["rel_pair_idx"], np.int32)
    union = np.asarray(inputs["union_feats"], f32)

    in_maps = []
    for b in range(B):
        m = dict(weights)
        m["obj_feats"] = obj_feats[b]
        m["obj_dists"] = obj_dists[b]
        m["box_info"] = box_info[b]
        m["rel_idx"] = np.ascontiguousarray(rel_idx[b].reshape(RCHUNKS, P, 2))
        m["union16"] = np.ascontiguousarray(
            union[b].astype(bf16).reshape(RCHUNKS, P, D))
        in_maps.append(m)

    res = run_bass_kernel_spmd(nc, in_maps, core_ids=list(range(B)), trace=trace)
    global _last_res
    _last_res = res
    last_exec_time_ns = res.exec_time_ns
    if res.instructions_and_trace is not None:
        last_trace_path = res.instructions_and_trace[1]

    scores = np.concatenate([res.results[b]["scores"] for b in range(B)], 0)
    preds = np.concatenate(
        [res.results[b]["preds"][:, 0] for b in range(B)], 0).astype(np.int32)
    ctx_rep = np.concatenate([res.results[b]["ctx_rep"] for b in range(B)], 0)
    return scores, preds, ctx_rep


# revision 22
# speedup vs baseline: 1.0313x; 1.0313x over previous
"""Trainium2 Bass kernel for DirectionAwareMessagePassing (B=8,N=128,R=4096,D=1024).

Sharding: data-parallel over images (1 image per NeuronCore, 8 cores). Weights
replicated. Per image, the math is restructured for the PE array:

  rep^T is kept feature-major on chip (contraction dims must live on SBUF
  partitions). The per-pair coefficient
      coeff_r = (S'[p0_r] * O[p1_r] * u_r) . w  (with u = union @ wu_w)
  is computed without ever transposing union_feats:
      fold w into S:  S' = rep @ (ws_w * w) + ws_b * w
      q_r  = S'[p0_r] * O[p1_r]                  (one-hot gather matmuls)
      g    = Q @ wu_w^T                          (the big [R,D]x[D,D] matmul)
      coeff= rowsum(union * g)                   (DVE fused mul+reduce)
  Gathers are one-hot matmuls (exact selection); the scatter-add into the
  dense [N,N] attention matrix is (P0*coeff)^T @ P1 accumulated in PSUM.

  The coeff/attention path tolerates bf16 (validated: <5e-6 final rel err,
  0 argmax flips), so union_feats / wu / ws / wo / gathers run bf16 at full
  PE rate and union DMA is halved. The rep -> ctx_rep -> scores path stays
  fp32.
"""

import os
import sys
import types
import contextlib

import numpy as np
import ml_dtypes

for _p in ("/opt/trn_rl_repo",):
    if _p not in sys.path:
        sys.path.insert(0, _p)

# ---------------------------------------------------------------------------
# Environment shims for the trimmed trn_rl_repo under axon.
# ---------------------------------------------------------------------------


def _install_shims():
    # 1) antenv.axon_hooks is missing in this image; provide it so
    #    run_bass_kernel_spmd(trace=True) can register the NTFF hook.
    if "antenv.axon_hooks" not in sys.modules:
        _hook = [None]
        mod = types.ModuleType("antenv.axon_hooks")
        mod.set_axon_ntff_profile_hook = lambda h: _hook.__setitem__(0, h)
        mod.get_axon_ntff_profile_hook = lambda: _hook[0]
        sys.modules["antenv.axon_hooks"] = mod
        try:
            from trn_agent_boot.trn_boot import _ntff_profile_via_ctypes

            h = _ntff_profile_via_ctypes("/opt/axon/libaxon_pjrt.so")
            if h is not None:
                mod.set_axon_ntff_profile_hook(h)
        except Exception:
            pass

    # 2) This walrus rejects >1 sync-wait on a CTRL(Drain) instruction, but
    #    Tile's tail drain carries one wait per live semaphore. Split them.
    import concourse.tile as tile
    import concourse.mybir as mybir
    from concourse.vector_clock import ScopedClock

    if getattr(tile.TileContext, "_damp_drain_patched", False):
        return

    def _drain_and_barrier(self, tick_clock, wait_clock):
        nc = self.nc
        drain_inst = nc.sync.drain()
        wait_clock.add_sem_waits(
            drain_inst.ins, ScopedClock({None: tick_clock.global_clock})
        )
        si = drain_inst.ins.sync_info
        waits = list(si.on_wait or []) if si is not None else []
        if len(waits) > 1:
            si.on_wait = waits[:1]
            for w in waits[1:]:
                extra = nc.sync.drain()
                esi = extra.ins.sync_info
                if esi is None:
                    extra.ins.sync_info = mybir.SyncInfo(on_wait=[w], on_update=[])
                else:
                    esi.on_wait = [w]
        nc.all_engine_barrier()
        assert self.sems is not None
        popped = nc._tile_sem_poison_stack.pop()
        assert popped is self._sem_poison
        # clear_and_free_semaphores with a wide range emits a sem_clear ISA
        # whose length this walrus rejects ("ISA wrong length") — clear in
        # chunks of <=4 sems instead.
        sems = list(self.sems.allocated().values())
        if sems:
            sem_nums = [s.num if hasattr(s, "num") else s for s in sems]
            for rg in bass.compact_to_ranges(sem_nums):
                vals = list(rg)
                for i in range(0, len(vals), 4):
                    sub = vals[i:i + 4]
                    subrange = range(sub[0], sub[-1] + 1)
                    nc.gpsimd.dma_reset(subrange)
                    nc.gpsimd.sem_clear(subrange)
            nc._state.prepend_free_semaphores(sem_nums)
            for poison_set in nc._tile_sem_poison_stack:
                poison_set.update(sem_nums)
        nc.all_engine_barrier()

    tile.TileContext._drain_and_barrier = _drain_and_barrier
    tile.TileContext._damp_drain_patched = True


_install_shims()


def _split_multi_waits(nc):
    """This walrus accepts at most one sync-wait per instruction. Move extra
    waits onto freshly inserted same-engine EventSemaphore instructions placed
    immediately before the original (per-engine program order is preserved, so
    semantics are identical)."""
    import concourse.mybir as mybir

    n = 0
    for f in nc.m.functions:
        for bb in f.blocks:
            new = []
            for inst in bb.instructions:
                si = inst.sync_info
                waits = list(si.on_wait) if (si is not None and si.on_wait) else []
                if len(waits) > 1:
                    for w in waits[:-1]:
                        n += 1
                        ev = mybir.InstEventSemaphore(
                            name=f"{inst.name}_sw{n}",
                            opcode="EventSemaphore",
                            engine=inst.engine,
                            sync_info=mybir.SyncInfo(on_wait=[w], on_update=[]),
                        )
                        new.append(ev)
                    si.on_wait = waits[-1:]
                new.append(inst)
            bb.instructions[:] = new
    return n


import concourse.bass as bass  # noqa: E402
import concourse.mybir as mybir  # noqa: E402
import concourse.tile as tile  # noqa: E402
from concourse.bass_utils import run_bass_kernel_spmd  # noqa: E402
from concourse.masks import make_identity  # noqa: E402
from concourse.tile import TileContext  # noqa: E402

F32 = mybir.dt.float32
F32R = mybir.dt.float32r
BF16 = mybir.dt.bfloat16
I32 = mybir.dt.int32
U32 = mybir.dt.uint32
ALU = mybir.AluOpType
ACTF = mybir.ActivationFunctionType

B, N, R, D = 8, 128, 4096, 1024
E_EMB, C, PIN = 200, 151, 9
P = 128
RCHUNKS = R // P            # 32 pair chunks of 128
NGROUPS = 8                 # pair groups of 512 (4 chunks)
GC = RCHUNKS // NGROUPS     # 4 chunks per group
DC = D // P                 # 8 feature chunks
KPROJ = 11                  # padded concat input chunks: 8 obj + 2 emb + 1 pos

last_exec_time_ns = None
last_trace_path = None


# ---------------------------------------------------------------------------
# Program builder
# ---------------------------------------------------------------------------

def _build_program(with_bias_path: bool, with_ln_affine: bool, debug: bool):
    nc = bass.Bass("TRN2", target_bir_lowering=False, debug=False, num_devices=8)

    def inp(name, shape, dt):
        return nc.declare_dram_parameter(name, list(shape), dt, isOutput=False)

    def outp(name, shape, dt):
        return nc.declare_dram_parameter(name, list(shape), dt, isOutput=True)

    d_obj = inp("obj_feats", [P, D], F32)
    d_dists = inp("obj_dists", [P, C], F32)
    d_box = inp("box_info", [P, PIN], F32)
    d_idx = inp("rel_idx", [RCHUNKS, P, 2], I32)
    d_union = inp("union16", [RCHUNKS, P, D], BF16)

    d_pw = inp("pw", [KPROJ, P, D], F32R)
    d_pbcol = inp("proj_bcol", [P, DC], F32)
    d_pbrow = inp("proj_brow", [1, D], F32)
    d_emb = inp("embed_w_p", [2, P, E_EMB], F32)
    d_pew1 = inp("pe_w1f", [PIN, 32], F32)
    d_peb1 = inp("pe_b1col", [32, 1], F32)
    d_pew2 = inp("pe_w2", [32, P], F32)
    d_peb2 = inp("pe_b2col", [P, 1], F32)
    d_ws = inp("ws16", [DC, P, D], BF16)
    d_wsb = inp("wsb16", [1, D], BF16)
    d_wo = inp("wo16", [DC, P, D], BF16)
    d_wob = inp("wob16", [1, D], BF16)
    d_wu = inp("wu16", [DC, P, D], BF16)
    d_wt3 = inp("wt3", [DC, P, D // 2], F32R)
    d_wt3b = inp("wt3b", [1, D // 2], F32)
    d_tr1 = inp("tr1", [DC, P, D // 4], F32R)
    d_tr1b = inp("tr1b", [1, D // 4], F32)
    d_trg = inp("trg", [1, D // 4], F32)
    d_trbe = inp("trbe", [1, D // 4], F32)
    d_tr2 = inp("tr2", [2, P, D], F32)
    d_tr2b = inp("tr2bcol", [P, DC], F32)
    d_outw = inp("outw", [DC, P, C], F32)
    d_outb = inp("outbcol", [P, 2], F32)
    d_wub = inp("wub16", [1, D], BF16) if with_bias_path else None
    d_wb = inp("w_b_scalar", [1, 1], F32) if with_bias_path else None

    o_scores = outp("scores", [P, C], F32)
    o_preds = outp("preds", [P, 1], I32)
    o_ctx = outp("ctx_rep", [P, D], F32)
    if debug:
        o_dbg_rep = outp("dbg_repT", [P, DC, P], F32)
        o_dbg_an = outp("dbg_an", [P, P], F32)
        o_dbg_coeff = outp("dbg_coeff", [P, RCHUNKS], F32)
        o_dbg_sp = outp("dbg_sp", [P, D], F32)
    coeff_dbg = None

    with TileContext(nc) as tc, contextlib.ExitStack() as ctx:
        wpool = ctx.enter_context(tc.tile_pool(name="wpool", bufs=1))
        respool = ctx.enter_context(tc.tile_pool(name="respool", bufs=1))

        # ---- static tiles -------------------------------------------------
        ident = wpool.tile([P, P], F32)
        make_identity(nc, ident[:])
        ident16 = wpool.tile([P, P], BF16)
        make_identity(nc, ident16[:])

        iota_i32 = wpool.tile([P, P], I32)
        nc.gpsimd.iota(iota_i32[:], pattern=[[1, P]], base=0, channel_multiplier=0)
        iota16 = wpool.tile([P, P], BF16)
        nc.vector.tensor_copy(iota16[:], iota_i32[:])
        if debug:
            coeff_dbg = wpool.tile([P, RCHUNKS], F32)

        # persistent weights
        wu_sb = wpool.tile([P, DC, D], BF16)
        nc.sync.dma_start(out=wu_sb[:], in_=d_wu.rearrange("c p e -> p c e"))
        tr1_sb = respool.tile([P, DC, D // 4], F32R)
        nc.sync.dma_start(out=tr1_sb[:], in_=d_tr1.rearrange("c p e -> p c e"))
        tr2_sb = respool.tile([P, 2, D], F32)
        nc.sync.dma_start(out=tr2_sb[:], in_=d_tr2.rearrange("c p e -> p c e"))
        outw_sb = respool.tile([P, DC, C], F32)
        nc.sync.dma_start(out=outw_sb[:], in_=d_outw.rearrange("c p e -> p c e"))
        tr1b_sb = respool.tile([1, D // 4], F32)
        nc.sync.dma_start(out=tr1b_sb[:], in_=d_tr1b[:])
        tr2b_sb = respool.tile([P, DC], F32)
        nc.sync.dma_start(out=tr2b_sb[:], in_=d_tr2b[:])
        outb_sb = respool.tile([P, 2], F32)
        nc.sync.dma_start(out=outb_sb[:], in_=d_outb[:])
        pbcol_sb = wpool.tile([P, DC], F32)
        nc.sync.dma_start(out=pbcol_sb[:], in_=d_pbcol[:])

        ones_f = wpool.tile([1, P], F32)
        nc.gpsimd.memset(ones_f[:], 1.0)
        ones16 = wpool.tile([1, P], BF16)
        nc.gpsimd.memset(ones16[:], 1.0)

        # index prep: [P, RCHUNKS, 2] f32 + per-chunk rows
        idx_i = wpool.tile([P, RCHUNKS, 2], I32)
        nc.sync.dma_start(out=idx_i[:], in_=d_idx.rearrange("c p two -> p c two"))
        idx_f = wpool.tile([P, RCHUNKS, 2], F32)
        nc.vector.tensor_copy(idx_f[:], idx_i[:])

        # outputs of phase 1 kept on chip
        repT = wpool.tile([P, DC, P], F32)
        repT16 = wpool.tile([P, DC, P], BF16)
        repTr = wpool.tile([P, DC, P], F32R)
        S16 = wpool.tile([P, D], BF16)
        O16 = wpool.tile([P, D], BF16)
        W3 = wpool.tile([P, D // 2], F32)
        if with_ln_affine:
            trg_b = wpool.tile([P, D // 4], F32)
            trbe_b = wpool.tile([P, D // 4], F32)
        if with_bias_path:
            ST2 = wpool.tile([P, DC, P], BF16)   # (S' * wu_b)^T
            OT2 = wpool.tile([P, DC, P], BF16)   # O^T

        # =================================================================
        # PHASE 1: rep^T, S', O, W3
        # =================================================================
        with tc.tile_pool(name="ph1", bufs=1) as ph1, \
             tc.tile_pool(name="ph1ps", bufs=2, space="PSUM") as ph1ps, \
             tc.tile_pool(name="pwstream", bufs=3) as pwstream:

            # xT = [obj_feats^T ; obj_embed^T ; pos^T]  [P, KPROJ, P]
            xT = ph1.tile([P, KPROJ, P], F32R)
            nc.vector.memset(xT[:].bitcast(F32), 0.0)

            objf = ph1.tile([P, D], F32)
            nc.sync.dma_start(out=objf[:], in_=d_obj[:])
            for c in range(DC):
                ps = ph1ps.tile([P, P], F32, space="PSUM", tag="tp")
                nc.tensor.transpose(ps[:], objf[:, c * P:(c + 1) * P], ident[:])
                nc.scalar.copy(xT[:, c, :], ps[:])

            # obj_dists^T (2 chunks, second is 23 rows zero-padded)
            dists = ph1.tile([P, C], F32)
            nc.sync.dma_start(out=dists[:], in_=d_dists[:])
            odT = ph1.tile([P, 2, P], F32)
            nc.vector.memset(odT[:], 0.0)
            ps = ph1ps.tile([P, P], F32, space="PSUM", tag="tp")
            nc.tensor.transpose(ps[:], dists[:, 0:P], ident[:])
            nc.scalar.copy(odT[:, 0, :], ps[:])
            ps = ph1ps.tile([P, P], F32, space="PSUM", tag="tp")
            nc.tensor.transpose(ps[:23, :], dists[:, P:C], ident[:])
            nc.scalar.copy(odT[:23, 1, :], ps[:23, :])

            # obj_embed^T via embed_w (c chunks padded to 128)
            emb_sb = ph1.tile([P, 2, E_EMB], F32)
            nc.sync.dma_start(out=emb_sb[:], in_=d_emb.rearrange("c p e -> p c e"))
            eps = ph1ps.tile([P, P], F32, space="PSUM", tag="mm")
            for kc in range(2):
                nc.tensor.matmul(eps[:, :], emb_sb[:, kc, 0:P], odT[:, kc, :],
                                 start=(kc == 0), stop=(kc == 1))
            nc.scalar.copy(xT[:, 8, :], eps[:])
            eps = ph1ps.tile([P, P], F32, space="PSUM", tag="mm")
            for kc in range(2):
                nc.tensor.matmul(eps[:72, :], emb_sb[:, kc, P:E_EMB], odT[:, kc, :],
                                 start=(kc == 0), stop=(kc == 1))
            nc.scalar.copy(xT[:72, 9, :], eps[:72, :])

            # pos^T: box^T -> h^T(32) -> pos^T(128) with BN folded on host
            box = ph1.tile([P, PIN], F32)
            nc.sync.dma_start(out=box[:], in_=d_box[:])
            bps = ph1ps.tile([P, P], F32, space="PSUM", tag="tp")
            nc.tensor.transpose(bps[:PIN, :], box[:], ident[:])
            boxT = ph1.tile([PIN, P], F32)
            nc.vector.tensor_copy(boxT[:], bps[:PIN, :])
            pew1 = ph1.tile([PIN, 32], F32)
            nc.sync.dma_start(out=pew1[:], in_=d_pew1[:])
            peb1 = ph1.tile([32, 1], F32)
            nc.sync.dma_start(out=peb1[:], in_=d_peb1[:])
            pew2 = ph1.tile([32, P], F32)
            nc.sync.dma_start(out=pew2[:], in_=d_pew2[:])
            peb2 = ph1.tile([P, 1], F32)
            nc.sync.dma_start(out=peb2[:], in_=d_peb2[:])
            hps = ph1ps.tile([P, P], F32, space="PSUM", tag="mm")
            nc.tensor.matmul(hps[:32, :], pew1[:], boxT[:], start=True, stop=True)
            hT = ph1.tile([32, P], F32)
            nc.scalar.activation(hT[:], hps[:32, :], ACTF.Identity, bias=peb1[:])
            pps = ph1ps.tile([P, P], F32, space="PSUM", tag="mm")
            nc.tensor.matmul(pps[:], pew2[:], hT[:], start=True, stop=True)
            nc.scalar.activation(xT[:, 10, :], pps[:], ACTF.Relu, bias=peb2[:])

            # rep = x @ proj_w: form (i), fp32r full-rate, streamed pw chunks
            # (lhsT = xT chunk stationary, rhs = proj chunk moving N=512)
            rep_ps = [ph1ps.tile([P, 512], F32, space="PSUM", tag=f"rep{h}",
                                 bufs=1, name=f"rep_ps{h}") for h in range(2)]
            for kc in range(KPROJ):
                pwc = pwstream.tile([P, D], F32R, tag="pw")
                nc.sync.dma_start(out=pwc[:], in_=d_pw[kc])
                for h in range(2):
                    nc.tensor.matmul(rep_ps[h][:], xT[:, kc, :],
                                     pwc[:, h * 512:(h + 1) * 512],
                                     start=(kc == 0), stop=False)
            pbrow = ph1.tile([1, D], F32)
            nc.sync.dma_start(out=pbrow[:], in_=d_pbrow[:])
            rep_sb = ph1.tile([P, D], F32)
            for h in range(2):
                nc.tensor.matmul(rep_ps[h][:], ones_f[:],
                                 pbrow[:, h * 512:(h + 1) * 512],
                                 start=False, stop=True)
                nc.scalar.activation(rep_sb[:, h * 512:(h + 1) * 512],
                                     rep_ps[h][:], ACTF.Relu)
            # transpose rep -> repT (+ bf16 / f32r copies for downstream lhsT)
            for m in range(DC):
                tps2 = ph1ps.tile([P, P], F32, space="PSUM", tag="tp")
                nc.tensor.transpose(tps2[:], rep_sb[:, m * P:(m + 1) * P], ident[:])
                nc.scalar.copy(repT[:, m, :], tps2[:])
                nc.vector.tensor_copy(repTr[:, m, :], tps2[:])
            nc.vector.tensor_copy(repT16[:], repT[:])
            if debug:
                nc.sync.dma_start(out=o_dbg_rep[:], in_=repT[:])

            # S' = rep @ ws' + b  (bf16, w_w folded);  O likewise;  W3 fp32
            ws_sb = ph1.tile([P, DC, D], BF16)
            nc.sync.dma_start(out=ws_sb[:], in_=d_ws.rearrange("c p e -> p c e"))
            wo_sb = ph1.tile([P, DC, D], BF16)
            nc.sync.dma_start(out=wo_sb[:], in_=d_wo.rearrange("c p e -> p c e"))
            wsb = ph1.tile([1, D], BF16)
            nc.sync.dma_start(out=wsb[:], in_=d_wsb[:])
            wob = ph1.tile([1, D], BF16)
            nc.sync.dma_start(out=wob[:], in_=d_wob[:])

            for (wmat, wbias, dst) in ((ws_sb, wsb, S16), (wo_sb, wob, O16)):
                for h in range(2):
                    sps = ph1ps.tile([P, 512], F32, space="PSUM", tag="so")
                    for kc in range(DC):
                        nc.tensor.matmul(sps[:], repT16[:, kc, :],
                                         wmat[:, kc, h * 512:(h + 1) * 512],
                                         start=(kc == 0), stop=False)
                    nc.tensor.matmul(sps[:], ones16[:],
                                     wbias[:, h * 512:(h + 1) * 512],
                                     start=False, stop=True)
                    nc.vector.tensor_copy(dst[:, h * 512:(h + 1) * 512], sps[:])
            if debug:
                sdbg = ph1.tile([P, D], F32)
                nc.vector.tensor_copy(sdbg[:], S16[:])
                nc.sync.dma_start(out=o_dbg_sp[:], in_=sdbg[:])

            wt3_sb = ph1.tile([P, DC, D // 2], F32R)
            nc.sync.dma_start(out=wt3_sb[:], in_=d_wt3.rearrange("c p e -> p c e"))
            wt3b_sb = ph1.tile([1, D // 2], F32)
            nc.sync.dma_start(out=wt3b_sb[:], in_=d_wt3b[:])
            wps = ph1ps.tile([P, 512], F32, space="PSUM", tag="so")
            for kc in range(DC):
                nc.tensor.matmul(wps[:], repTr[:, kc, :], wt3_sb[:, kc, :],
                                 start=(kc == 0), stop=False)
            nc.tensor.matmul(wps[:], ones_f[:], wt3b_sb[:], start=False, stop=True)
            nc.scalar.activation(W3[:], wps[:], ACTF.Relu)

            if with_ln_affine:
                trg_row = ph1.tile([1, D // 4], F32)
                nc.sync.dma_start(out=trg_row[:], in_=d_trg[:])
                trbe_row = ph1.tile([1, D // 4], F32)
                nc.sync.dma_start(out=trbe_row[:], in_=d_trbe[:])
                gps = ph1ps.tile([P, 512], F32, space="PSUM", tag="so")
                nc.tensor.matmul(gps[:, :D // 4], ones_f[:], trg_row[:],
                                 start=True, stop=True)
                nc.vector.tensor_copy(trg_b[:], gps[:, :D // 4])
                gps = ph1ps.tile([P, 512], F32, space="PSUM", tag="so")
                nc.tensor.matmul(gps[:, :D // 4], ones_f[:], trbe_row[:],
                                 start=True, stop=True)
                nc.vector.tensor_copy(trbe_b[:], gps[:, :D // 4])

            if with_bias_path:
                # T[i,j] = sum_d (S'*wu_b)[i,d] * O[j,d] needs transposed copies
                wub = ph1.tile([1, D], BF16)
                nc.sync.dma_start(out=wub[:], in_=d_wub[:])
                wub_b = ph1.tile([P, D], BF16)
                bps2 = ph1ps.tile([P, 512], F32, space="PSUM", tag="so")
                for h in range(2):
                    nc.tensor.matmul(bps2[:], ones16[:], wub[:, h * 512:(h + 1) * 512],
                                     start=True, stop=True)
                    nc.vector.tensor_copy(wub_b[:, h * 512:(h + 1) * 512], bps2[:])
                s2 = ph1.tile([P, D], BF16)
                nc.vector.tensor_tensor(s2[:], S16[:], wub_b[:], op=ALU.mult)
                for c in range(DC):
                    tps = ph1ps.tile([P, P], BF16, space="PSUM", tag="tp")
                    nc.tensor.transpose(tps[:], s2[:, c * P:(c + 1) * P], ident16[:])
                    nc.vector.tensor_copy(ST2[:, c, :], tps[:])
                    tps = ph1ps.tile([P, P], BF16, space="PSUM", tag="tp")
                    nc.tensor.transpose(tps[:], O16[:, c * P:(c + 1) * P], ident16[:])
                    nc.vector.tensor_copy(OT2[:, c, :], tps[:])

        # =================================================================
        # PHASE 2: pair loop — gathers, g = Q @ wu^T, coeff, scatter into A
        # =================================================================
        # A and Count in separate PSUM banks (start=True clears a whole bank)
        apool = ctx.enter_context(tc.tile_pool(name="apool", bufs=1, space="PSUM"))
        a_acc = apool.tile([P, P], F32, space="PSUM")
        cnt_acc = (apool.tile([P, P], F32, space="PSUM", name="cnt_acc")
                   if with_bias_path else None)

        with tc.tile_pool(name="p2", bufs=2) as p2, \
             tc.tile_pool(name="p2ps", bufs=4, space="PSUM") as p2ps, \
             tc.tile_pool(name="p2tp", bufs=2, space="PSUM") as p2tp:

            for g in range(NGROUPS):
                union_sb = p2.tile([P, GC, D], BF16, tag="union")
                nc.sync.dma_start(
                    out=union_sb[:],
                    in_=d_union[g * GC:(g + 1) * GC].rearrange("c p e -> p c e"))

                # one-hot selectors for this group's 512 pairs
                p0t = p2.tile([P, GC * P], BF16, tag="p0t")
                p1t = p2.tile([P, GC * P], BF16, tag="p1t")
                pmats = []  # per chunk: (p0m, p1m) [r,i] one-hots
                for cc in range(GC):
                    ch = g * GC + cc
                    p0col = idx_f[:, ch, 0:1]
                    p1col = idx_f[:, ch, 1:2]
                    # one-hots for all GC chunks of a group stay live until the
                    # group's scatters: bufs must cover 2 groups' worth
                    p0m = p2.tile([P, P], BF16, tag="p0m", bufs=2 * GC + 2)
                    nc.vector.tensor_single_scalar(p0m[:], iota16[:], p0col,
                                                   op=ALU.is_equal)
                    p1m = p2.tile([P, P], BF16, tag="p1m", bufs=2 * GC + 2)
                    nc.vector.tensor_single_scalar(p1m[:], iota16[:], p1col,
                                                   op=ALU.is_equal)
                    tp = p2tp.tile([P, P], BF16, space="PSUM", tag="tp")
                    nc.tensor.transpose(tp[:], p0m[:], ident16[:])
                    nc.vector.tensor_copy(p0t[:, cc * P:(cc + 1) * P], tp[:])
                    tp = p2tp.tile([P, P], BF16, space="PSUM", tag="tp")
                    nc.tensor.transpose(tp[:], p1m[:], ident16[:])
                    nc.vector.tensor_copy(p1t[:, cc * P:(cc + 1) * P], tp[:])
                    pmats.append((p0m, p1m))

                # gathers: sT/oT [d-chunk, 512] then QT = sT*oT (bf16).
                # DVE can read only one PSUM operand; bounce sT via ACT copy.
                qt = p2.tile([P, DC, GC * P], BF16, tag="qt")
                for dcc in range(DC):
                    sps = p2ps.tile([P, 512], F32, space="PSUM", tag="big")
                    nc.tensor.matmul(sps[:], S16[:, dcc * P:(dcc + 1) * P], p0t[:],
                                     start=True, stop=True)
                    ops_ = p2ps.tile([P, 512], F32, space="PSUM", tag="big")
                    nc.tensor.matmul(ops_[:], O16[:, dcc * P:(dcc + 1) * P], p1t[:],
                                     start=True, stop=True)
                    s_sb = p2.tile([P, 512], BF16, tag="s_sb")
                    nc.scalar.copy(s_sb[:], sps[:])
                    nc.vector.tensor_tensor(qt[:, dcc, :], s_sb[:], ops_[:],
                                            op=ALU.mult)

                # g = Q @ wu^T per 128-pair chunk; coeff = rowsum(union * g)
                for cc in range(GC):
                    ch = g * GC + cc
                    gps0 = p2ps.tile([P, 512], F32, space="PSUM", tag="big")
                    gps1 = p2ps.tile([P, 512], F32, space="PSUM", tag="big")
                    for dcc in range(DC):
                        lhs = qt[:, dcc, cc * P:(cc + 1) * P]
                        nc.tensor.matmul(gps0[:], lhs, wu_sb[:, dcc, 0:512],
                                         start=(dcc == 0), stop=(dcc == DC - 1))
                        nc.tensor.matmul(gps1[:], lhs, wu_sb[:, dcc, 512:1024],
                                         start=(dcc == 0), stop=(dcc == DC - 1))
                    # fused mul + row-sum via TensorScalarPtr accum_out
                    # (tensor_tensor_reduce is an ISA op this walrus rejects)
                    c0 = p2.tile([P, 1], F32, tag="c0")
                    scr = p2.tile([P, 512], F32, tag="scr")
                    nc.vector.scalar_tensor_tensor(
                        out=scr[:], in0=union_sb[:, cc, 0:512], scalar=1.0,
                        in1=gps0[:], op0=ALU.bypass, op1=ALU.mult,
                        accum_out=c0[:])
                    c1 = p2.tile([P, 1], F32, tag="c1")
                    scr2 = p2.tile([P, 512], F32, tag="scr")
                    nc.vector.scalar_tensor_tensor(
                        out=scr2[:], in0=union_sb[:, cc, 512:1024], scalar=1.0,
                        in1=gps1[:], op0=ALU.bypass, op1=ALU.mult,
                        accum_out=c1[:])
                    coeff = p2.tile([P, 1], F32, tag="coeff")
                    nc.vector.tensor_tensor(coeff[:], c0[:], c1[:], op=ALU.add)
                    if debug:
                        nc.vector.tensor_copy(coeff_dbg[:, ch:ch + 1], coeff[:])
                        if ch == RCHUNKS - 1:
                            nc.sync.dma_start(out=o_dbg_coeff[:], in_=coeff_dbg[:])

                    # scatter: A += (P0*coeff)^T @ P1  (and Count += P0^T @ P1)
                    p0m, p1m = pmats[cc]
                    p0c = p2.tile([P, P], BF16, tag="p0c")
                    nc.vector.tensor_tensor(p0c[:], p0m[:],
                                            coeff[:].to_broadcast([P, P]),
                                            op=ALU.mult)
                    first = (ch == 0)
                    last = (ch == RCHUNKS - 1)
                    nc.tensor.matmul(a_acc[:], p0c[:], p1m[:],
                                     start=first, stop=last)
                    if with_bias_path:
                        nc.tensor.matmul(cnt_acc[:], p0m[:], p1m[:],
                                         start=first, stop=last)

        # =================================================================
        # PHASE 3: A -> ctx -> nb -> ctx_rep -> scores/preds
        # =================================================================
        with tc.tile_pool(name="p3", bufs=1) as p3, \
             tc.tile_pool(name="p3ps", bufs=2, space="PSUM") as p3ps:

            if with_bias_path:
                # T^T[j,i] = sum_d O^T[d,j] (S'wu_b)^T[d,i]; A += Count*(T + w_b)
                tt_ps = p3ps.tile([P, P], F32, space="PSUM", tag="mm")
                for dcc in range(DC):
                    nc.tensor.matmul(tt_ps[:], OT2[:, dcc, :], ST2[:, dcc, :],
                                     start=(dcc == 0), stop=(dcc == DC - 1))
                ttsb = p3.tile([P, P], F32)
                nc.vector.tensor_copy(ttsb[:], tt_ps[:])
                t_ps = p3ps.tile([P, P], F32, space="PSUM", tag="mm")
                nc.tensor.transpose(t_ps[:], ttsb[:], ident[:])
                wbsc = p3.tile([1, 1], F32)
                nc.sync.dma_start(out=wbsc[:], in_=d_wb[:])
                wbcol = p3.tile([P, 1], F32)
                bb = p3ps.tile([P, P], F32, space="PSUM", tag="mm2")
                nc.tensor.matmul(bb[:, 0:1], ones_f[:], wbsc[:], start=True, stop=True)
                nc.vector.tensor_copy(wbcol[:], bb[:, 0:1])
                tpw = p3.tile([P, P], F32)
                nc.vector.tensor_scalar_add(tpw[:], t_ps[:], wbcol[:])
                cnt_term = p3.tile([P, P], F32)
                nc.vector.tensor_tensor(cnt_term[:], cnt_acc[:], tpw[:],
                                        op=ALU.mult)
                apre = p3.tile([P, P], F32)
                nc.vector.tensor_tensor(apre[:], a_acc[:], cnt_term[:],
                                        op=ALU.add)
                asig = p3.tile([P, P], F32)
                nc.scalar.activation(asig[:], apre[:], ACTF.Sigmoid)
            else:
                asig = p3.tile([P, P], F32)
                nc.scalar.activation(asig[:], a_acc[:], ACTF.Sigmoid)

            # zero diagonal, row-normalize
            nc.gpsimd.affine_select(out=asig[:], in_=asig[:],
                                    compare_op=ALU.not_equal, fill=0.0,
                                    base=0, pattern=[[-1, P]], channel_multiplier=1)
            rs = p3.tile([P, 1], F32)
            nc.vector.reduce_sum(rs[:], asig[:], axis=mybir.AxisListType.X)
            rsr = p3.tile([P, 1], F32)
            nc.vector.reciprocal(rsr[:], rs[:])
            an = p3.tile([P, P], F32)
            nc.vector.tensor_scalar_mul(an[:], asig[:], rsr[:])
            if debug:
                nc.sync.dma_start(out=o_dbg_an[:], in_=an[:])
            antp = p3ps.tile([P, P], F32, space="PSUM", tag="mm")
            nc.tensor.transpose(antp[:], an[:], ident[:])
            anT = p3.tile([P, P], F32)
            nc.vector.tensor_copy(anT[:], antp[:])

            # ctx^T [d2, i]: first 512 rows from (An@W3)^T, last 512 from (An^T@W3)^T
            ctxT = p3.tile([P, DC, P], F32R)
            for m4 in range(4):
                cps = p3ps.tile([P, P], F32, space="PSUM", tag="mm")
                nc.tensor.matmul(cps[:], W3[:, m4 * P:(m4 + 1) * P], anT[:],
                                 start=True, stop=True)
                nc.scalar.copy(ctxT[:, m4, :], cps[:])
            for m4 in range(4):
                cps = p3ps.tile([P, P], F32, space="PSUM", tag="mm")
                nc.tensor.matmul(cps[:], W3[:, m4 * P:(m4 + 1) * P], an[:],
                                 start=True, stop=True)
                nc.scalar.copy(ctxT[:, 4 + m4, :], cps[:])

            # t = ctx @ tr1 + b; LayerNorm; relu; transpose
            tps_ = p3ps.tile([P, 256], F32, space="PSUM", tag="mm")
            for dcc in range(DC):
                nc.tensor.matmul(tps_[:], ctxT[:, dcc, :], tr1_sb[:, dcc, :],
                                 start=(dcc == 0), stop=False)
            nc.tensor.matmul(tps_[:], ones_f[:], tr1b_sb[:], start=False, stop=True)
            stats = p3.tile([P, 6], F32)
            nc.vector.bn_stats(stats[:], tps_[:])
            aggr = p3.tile([P, 2], F32)
            nc.vector.bn_aggr(aggr[:], stats[:])
            veps = p3.tile([P, 1], F32)
            nc.vector.tensor_scalar_add(veps[:], aggr[:, 1:2], 1e-5)
            stdv = p3.tile([P, 1], F32)
            nc.scalar.sqrt(stdv[:], veps[:])
            rstd = p3.tile([P, 1], F32)
            nc.vector.reciprocal(rstd[:], stdv[:])
            tn = p3.tile([P, 256], F32)
            nc.vector.tensor_scalar(tn[:], tps_[:], aggr[:, 0:1], rstd[:],
                                    op0=ALU.subtract, op1=ALU.mult)
            if with_ln_affine:
                nc.vector.tensor_tensor(tn[:], tn[:], trg_b[:], op=ALU.mult)
                nc.vector.tensor_tensor(tn[:], tn[:], trbe_b[:], op=ALU.add)
            trl = p3.tile([P, 256], F32)
            nc.scalar.activation(trl[:], tn[:], ACTF.Relu)
            rT = p3.tile([P, 2, P], F32)
            for kc in range(2):
                tp2 = p3ps.tile([P, P], F32, space="PSUM", tag="mm")
                nc.tensor.transpose(tp2[:], trl[:, kc * P:(kc + 1) * P], ident[:])
                nc.vector.tensor_copy(rT[:, kc, :], tp2[:])

            # nb^T per d-chunk; ctx_rep^T = relu(rep^T + nb^T + b2)
            ctxrT = p3.tile([P, DC, P], F32)
            for dcc in range(DC):
                nps = p3ps.tile([P, P], F32, space="PSUM", tag="mm")
                for kc in range(2):
                    nc.tensor.matmul(nps[:], tr2_sb[:, kc, dcc * P:(dcc + 1) * P],
                                     rT[:, kc, :], start=(kc == 0), stop=(kc == 1))
                tmp = p3.tile([P, P], F32, tag="nbtmp")
                nc.vector.scalar_tensor_tensor(
                    out=tmp[:], in0=nps[:], scalar=tr2b_sb[:, dcc:dcc + 1],
                    in1=repT[:, dcc, :], op0=ALU.add, op1=ALU.add)
                nc.scalar.activation(ctxrT[:, dcc, :], tmp[:], ACTF.Relu)

            # ctx_rep output (transpose back)
            ctx_sb = p3.tile([P, D], F32)
            for dcc in range(DC):
                cps2 = p3ps.tile([P, P], F32, space="PSUM", tag="mm")
                nc.tensor.transpose(cps2[:], ctxrT[:, dcc, :], ident[:])
                nc.scalar.copy(ctx_sb[:, dcc * P:(dcc + 1) * P], cps2[:])
            nc.sync.dma_start(out=o_ctx[:], in_=ctx_sb[:])

            # scores^T then scores
            scT = p3.tile([P, 2, P], F32)
            for mc in range(2):
                mwid = P if mc == 0 else C - P
                sps2 = p3ps.tile([P, P], F32, space="PSUM", tag="mm")
                for dcc in range(DC):
                    nc.tensor.matmul(sps2[:mwid, :],
                                     outw_sb[:, dcc, mc * P:mc * P + mwid],
                                     ctxrT[:, dcc, :],
                                     start=(dcc == 0), stop=(dcc == DC - 1))
                nc.scalar.activation(scT[:mwid, mc, :], sps2[:mwid, :],
                                     ACTF.Identity, bias=outb_sb[:mwid, mc:mc + 1])
            scores_sb = p3.tile([P, C], F32)
            sps3 = p3ps.tile([P, P], F32, space="PSUM", tag="mm")
            nc.tensor.transpose(sps3[:], scT[:, 0, :], ident[:])
            nc.vector.tensor_copy(scores_sb[:, 0:P], sps3[:])
            sps3 = p3ps.tile([P, P], F32, space="PSUM", tag="mm")
            nc.tensor.transpose(sps3[:, 0:C - P], scT[:C - P, 1, :],
                                ident[:C - P, 0:C - P])
            nc.vector.tensor_copy(scores_sb[:, P:C], sps3[:, 0:C - P])
            nc.sync.dma_start(out=o_scores[:], in_=scores_sb[:])

            # preds = argmax(scores[:,1:]) + 1
            mx8 = p3.tile([P, 8], F32)
            nc.vector.max(mx8[:], scores_sb[:, 1:C])
            mi8 = p3.tile([P, 8], U32)
            nc.vector.max_index(mi8[:], mx8[:], scores_sb[:, 1:C])
            predf = p3.tile([P, 1], I32)
            nc.vector.tensor_single_scalar(predf[:], mi8[:, 0:1], 1, op=ALU.add)
            nc.sync.dma_start(out=o_preds[:], in_=predf[:])

    nsplit = _split_multi_waits(nc)
    if nsplit:
        print(f"[kernel] split {nsplit} extra sync-waits into EventSemaphores")
    return nc


# ---------------------------------------------------------------------------
# Host-side input prep + execution
# ---------------------------------------------------------------------------

_cache = {}


def _prep_weights(inputs):
    f32 = np.float32
    bf16 = ml_dtypes.bfloat16
    w = np.asarray(inputs["w_w"], f32)[:, 0]

    alpha = (np.asarray(inputs["pe_g"], f32) / np.sqrt(np.float32(1.0 + 1e-5)))
    pe_w1f = np.asarray(inputs["pe_w1"], f32) * alpha[None, :]
    pe_b1col = (np.asarray(inputs["pe_b1"], f32) * alpha
                + np.asarray(inputs["pe_be"], f32))[:, None].copy()

    proj_w = np.asarray(inputs["proj_w"], f32)
    pw = np.zeros((KPROJ, P, D), f32)
    pw.reshape(KPROJ * P, D)[0:D] = proj_w[0:D]
    pw.reshape(KPROJ * P, D)[D:D + P] = proj_w[D:D + P]
    pw.reshape(KPROJ * P, D)[9 * P + 0: 9 * P + (E_EMB - P)] = proj_w[D + P:D + E_EMB]
    pw.reshape(KPROJ * P, D)[10 * P:11 * P] = proj_w[D + E_EMB:]

    emb = np.asarray(inputs["embed_w"], f32)
    emb_p = np.zeros((2, P, E_EMB), f32)
    emb_p[0] = emb[0:P]
    emb_p[1, :C - P] = emb[P:C]

    ws16 = (np.asarray(inputs["ws_w"], f32) * w[None, :]).astype(bf16).reshape(DC, P, D)
    wsb16 = (np.asarray(inputs["ws_b"], f32) * w).astype(bf16)[None, :]
    wo16 = np.asarray(inputs["wo_w"], f32).astype(bf16).reshape(DC, P, D)
    wob16 = np.asarray(inputs["wo_b"], f32).astype(bf16)[None, :]
    wu16 = np.ascontiguousarray(np.asarray(inputs["wu_w"], f32).T).astype(bf16)
    wu16 = wu16.reshape(DC, P, D)

    out_b = np.asarray(inputs["out_b"], f32)
    outbcol = np.zeros((P, 2), f32)
    outbcol[:, 0] = out_b[0:P]
    outbcol[:C - P, 1] = out_b[P:C]

    return {
        "pw": pw,
        "proj_bcol": np.ascontiguousarray(
            np.asarray(inputs["proj_b"], f32).reshape(DC, P).T),
        "proj_brow": np.asarray(inputs["proj_b"], f32)[None, :],
        "embed_w_p": emb_p,
        "pe_w1f": pe_w1f,
        "pe_b1col": pe_b1col,
        "pe_w2": np.asarray(inputs["pe_w2"], f32),
        "pe_b2col": np.asarray(inputs["pe_b2"], f32)[:, None].copy(),
        "ws16": ws16, "wsb16": wsb16,
        "wo16": wo16, "wob16": wob16,
        "wu16": wu16,
        "wt3": np.asarray(inputs["wt3_w"], f32).reshape(DC, P, D // 2),
        "wt3b": np.asarray(inputs["wt3_b"], f32)[None, :],
        "tr1": np.asarray(inputs["tr_w1"], f32).reshape(DC, P, D // 4),
        "tr1b": np.asarray(inputs["tr_b1"], f32)[None, :],
        "trg": np.asarray(inputs["tr_g"], f32)[None, :],
        "trbe": np.asarray(inputs["tr_be"], f32)[None, :],
        "tr2": np.asarray(inputs["tr_w2"], f32).reshape(2, P, D),
        "tr2bcol": np.ascontiguousarray(
            np.asarray(inputs["tr_b2"], f32).reshape(DC, P).T),
        "outw": np.asarray(inputs["out_w"], f32).reshape(DC, P, C),
        "outbcol": outbcol,
    }


def kernel(**inputs):
    global last_exec_time_ns, last_trace_path

    f32 = np.float32
    bf16 = ml_dtypes.bfloat16

    wu_b = np.asarray(inputs["wu_b"], f32)
    w_b = np.asarray(inputs["w_b"], f32)
    with_bias_path = bool(np.any(wu_b != 0) or np.any(w_b != 0))
    with_ln_affine = bool(
        np.any(np.asarray(inputs["tr_g"], f32) != 1.0)
        or np.any(np.asarray(inputs["tr_be"], f32) != 0.0))
    debug = bool(int(os.environ.get("DAMP_DEBUG", "0")))
    trace = bool(int(os.environ.get("DAMP_TRACE", "0")))

    key = (with_bias_path, with_ln_affine, debug)
    if key not in _cache:
        _cache[key] = _build_program(with_bias_path, with_ln_affine, debug)
    nc = _cache[key]

    weights = _prep_weights(inputs)
    if with_bias_path:
        weights["wub16"] = wu_b.astype(bf16)[None, :]
        weights["w_b_scalar"] = w_b.reshape(1, 1)

    obj_feats = np.asarray(inputs["obj_feats"], f32)
    obj_dists = np.asarray(inputs["obj_dists"], f32)
    box_info = np.asarray(inputs["box_info"], f32)
    rel_idx = np.asarray(inputs["rel_pair_idx"], np.int32)
    union = np.asarray(inputs["union_feats"], f32)

    in_maps = []
    for b in range(B):
        m = dict(weights)
        m["obj_feats"] = obj_feats[b]
        m["obj_dists"] = obj_dists[b]
        m["box_info"] = box_info[b]
        m["rel_idx"] = np.ascontiguousarray(rel_idx[b].reshape(RCHUNKS, P, 2))
        m["union16"] = np.ascontiguousarray(
            union[b].astype(bf16).reshape(RCHUNKS, P, D))
        in_maps.append(m)

    res = run_bass_kernel_spmd(nc, in_maps, core_ids=list(range(B)), trace=trace)
    global _last_res
    _last_res = res
    last_exec_time_ns = res.exec_time_ns
    if res.instructions_and_trace is not None:
        last_trace_path = res.instructions_and_trace[1]

    scores = np.concatenate([res.results[b]["scores"] for b in range(B)], 0)
    preds = np.concatenate(
        [res.results[b]["preds"][:, 0] for b in range(B)], 0).astype(np.int32)
    ctx_rep = np.concatenate([res.results[b]["ctx_rep"] for b in range(B)], 0)
    return scores, preds, ctx_rep


# revision 23
# speedup vs baseline: 1.0401x; 1.0086x over previous
"""Trainium2 Bass kernel for DirectionAwareMessagePassing (B=8,N=128,R=4096,D=1024).

Sharding: data-parallel over images (1 image per NeuronCore, 8 cores). Weights
replicated. Per image, the math is restructured for the PE array:

  rep^T is kept feature-major on chip (contraction dims must live on SBUF
  partitions). The per-pair coefficient
      coeff_r = (S'[p0_r] * O[p1_r] * u_r) . w  (with u = union @ wu_w)
  is computed without ever transposing union_feats:
      fold w into S:  S' = rep @ (ws_w * w) + ws_b * w
      q_r  = S'[p0_r] * O[p1_r]                  (one-hot gather matmuls)
      g    = Q @ wu_w^T                          (the big [R,D]x[D,D] matmul)
      coeff= rowsum(union * g)                   (DVE fused mul+reduce)
  Gathers are one-hot matmuls (exact selection); the scatter-add into the
  dense [N,N] attention matrix is (P0*coeff)^T @ P1 accumulated in PSUM.

  The coeff/attention path tolerates bf16 (validated: <5e-6 final rel err,
  0 argmax flips), so union_feats / wu / ws / wo / gathers run bf16 at full
  PE rate and union DMA is halved. The rep -> ctx_rep -> scores path stays
  fp32.
"""

import os
import sys
import types
import contextlib

import numpy as np
import ml_dtypes

for _p in ("/opt/trn_rl_repo",):
    if _p not in sys.path:
        sys.path.insert(0, _p)

# ---------------------------------------------------------------------------
# Environment shims for the trimmed trn_rl_repo under axon.
# ---------------------------------------------------------------------------


def _install_shims():
    # 1) antenv.axon_hooks is missing in this image; provide it so
    #    run_bass_kernel_spmd(trace=True) can register the NTFF hook.
    if "antenv.axon_hooks" not in sys.modules:
        _hook = [None]
        mod = types.ModuleType("antenv.axon_hooks")
        mod.set_axon_ntff_profile_hook = lambda h: _hook.__setitem__(0, h)
        mod.get_axon_ntff_profile_hook = lambda: _hook[0]
        sys.modules["antenv.axon_hooks"] = mod
        try:
            from trn_agent_boot.trn_boot import _ntff_profile_via_ctypes

            h = _ntff_profile_via_ctypes("/opt/axon/libaxon_pjrt.so")
            if h is not None:
                mod.set_axon_ntff_profile_hook(h)
        except Exception:
            pass

    # 2) This walrus rejects >1 sync-wait on a CTRL(Drain) instruction, but
    #    Tile's tail drain carries one wait per live semaphore. Split them.
    import concourse.tile as tile
    import concourse.mybir as mybir
    from concourse.vector_clock import ScopedClock

    if getattr(tile.TileContext, "_damp_drain_patched", False):
        return

    def _drain_and_barrier(self, tick_clock, wait_clock):
        nc = self.nc
        drain_inst = nc.sync.drain()
        wait_clock.add_sem_waits(
            drain_inst.ins, ScopedClock({None: tick_clock.global_clock})
        )
        si = drain_inst.ins.sync_info
        waits = list(si.on_wait or []) if si is not None else []
        if len(waits) > 1:
            si.on_wait = waits[:1]
            for w in waits[1:]:
                extra = nc.sync.drain()
                esi = extra.ins.sync_info
                if esi is None:
                    extra.ins.sync_info = mybir.SyncInfo(on_wait=[w], on_update=[])
                else:
                    esi.on_wait = [w]
        nc.all_engine_barrier()
        assert self.sems is not None
        popped = nc._tile_sem_poison_stack.pop()
        assert popped is self._sem_poison
        # clear_and_free_semaphores with a wide range emits a sem_clear ISA
        # whose length this walrus rejects ("ISA wrong length") — clear in
        # chunks of <=4 sems instead.
        sems = list(self.sems.allocated().values())
        if sems:
            sem_nums = [s.num if hasattr(s, "num") else s for s in sems]
            for rg in bass.compact_to_ranges(sem_nums):
                vals = list(rg)
                for i in range(0, len(vals), 4):
                    sub = vals[i:i + 4]
                    subrange = range(sub[0], sub[-1] + 1)
                    nc.gpsimd.dma_reset(subrange)
                    nc.gpsimd.sem_clear(subrange)
            nc._state.prepend_free_semaphores(sem_nums)
            for poison_set in nc._tile_sem_poison_stack:
                poison_set.update(sem_nums)
        nc.all_engine_barrier()

    tile.TileContext._drain_and_barrier = _drain_and_barrier
    tile.TileContext._damp_drain_patched = True


_install_shims()


def _split_multi_waits(nc):
    """This walrus accepts at most one sync-wait per instruction. Move extra
    waits onto freshly inserted same-engine EventSemaphore instructions placed
    immediately before the original (per-engine program order is preserved, so
    semantics are identical)."""
    import concourse.mybir as mybir

    n = 0
    for f in nc.m.functions:
        for bb in f.blocks:
            new = []
            for inst in bb.instructions:
                si = inst.sync_info
                waits = list(si.on_wait) if (si is not None and si.on_wait) else []
                if len(waits) > 1:
                    for w in waits[:-1]:
                        n += 1
                        ev = mybir.InstEventSemaphore(
                            name=f"{inst.name}_sw{n}",
                            opcode="EventSemaphore",
                            engine=inst.engine,
                            sync_info=mybir.SyncInfo(on_wait=[w], on_update=[]),
                        )
                        new.append(ev)
                    si.on_wait = waits[-1:]
                new.append(inst)
            bb.instructions[:] = new
    return n


import concourse.bass as bass  # noqa: E402
import concourse.mybir as mybir  # noqa: E402
import concourse.tile as tile  # noqa: E402
from concourse.bass_utils import run_bass_kernel_spmd  # noqa: E402
from concourse.masks import make_identity  # noqa: E402
from concourse.tile import TileContext  # noqa: E402

F32 = mybir.dt.float32
F32R = mybir.dt.float32r
BF16 = mybir.dt.bfloat16
I32 = mybir.dt.int32
U32 = mybir.dt.uint32
ALU = mybir.AluOpType
ACTF = mybir.ActivationFunctionType

B, N, R, D = 8, 128, 4096, 1024
E_EMB, C, PIN = 200, 151, 9
P = 128
RCHUNKS = R // P            # 32 pair chunks of 128
NGROUPS = 8                 # pair groups of 512 (4 chunks)
GC = RCHUNKS // NGROUPS     # 4 chunks per group
DC = D // P                 # 8 feature chunks
KPROJ = 11                  # padded concat input chunks: 8 obj + 2 emb + 1 pos

last_exec_time_ns = None
last_trace_path = None


# ---------------------------------------------------------------------------
# Program builder
# ---------------------------------------------------------------------------

def _build_program(with_bias_path: bool, with_ln_affine: bool, debug: bool):
    nc = bass.Bass("TRN2", target_bir_lowering=False, debug=False, num_devices=8)

    def inp(name, shape, dt):
        return nc.declare_dram_parameter(name, list(shape), dt, isOutput=False)

    def outp(name, shape, dt):
        return nc.declare_dram_parameter(name, list(shape), dt, isOutput=True)

    d_obj = inp("obj_feats", [P, D], F32)
    d_dists = inp("obj_dists", [P, C], F32)
    d_box = inp("box_info", [P, PIN], F32)
    d_idx = inp("rel_idx", [RCHUNKS, P, 2], I32)
    d_union = inp("union16", [RCHUNKS, P, D], BF16)

    d_pw = inp("pw", [KPROJ, P, D], F32R)
    d_pbcol = inp("proj_bcol", [P, DC], F32)
    d_pbrow = inp("proj_brow", [1, D], F32)
    d_emb = inp("embed_w_p", [2, P, E_EMB], F32)
    d_pew1 = inp("pe_w1f", [PIN, 32], F32)
    d_peb1 = inp("pe_b1col", [32, 1], F32)
    d_pew2 = inp("pe_w2", [32, P], F32)
    d_peb2 = inp("pe_b2col", [P, 1], F32)
    d_ws = inp("ws16", [DC, P, D], BF16)
    d_wsb = inp("wsb16", [1, D], BF16)
    d_wo = inp("wo16", [DC, P, D], BF16)
    d_wob = inp("wob16", [1, D], BF16)
    d_wu = inp("wu16", [DC, P, D], BF16)
    d_wt3 = inp("wt3", [DC, P, D // 2], F32R)
    d_wt3b = inp("wt3b", [1, D // 2], F32)
    d_tr1 = inp("tr1", [DC, P, D // 4], F32R)
    d_tr1b = inp("tr1b", [1, D // 4], F32)
    d_trg = inp("trg", [1, D // 4], F32)
    d_trbe = inp("trbe", [1, D // 4], F32)
    d_tr2 = inp("tr2", [2, P, D], F32)
    d_tr2b = inp("tr2bcol", [P, DC], F32)
    d_outw = inp("outw", [DC, P, C], F32)
    d_outb = inp("outbcol", [P, 2], F32)
    d_wub = inp("wub16", [1, D], BF16) if with_bias_path else None
    d_wb = inp("w_b_scalar", [1, 1], F32) if with_bias_path else None

    o_scores = outp("scores", [P, C], F32)
    o_preds = outp("preds", [P, 1], I32)
    o_ctx = outp("ctx_rep", [P, D], F32)
    if debug:
        o_dbg_rep = outp("dbg_repT", [P, DC, P], F32)
        o_dbg_an = outp("dbg_an", [P, P], F32)
        o_dbg_coeff = outp("dbg_coeff", [P, RCHUNKS], F32)
        o_dbg_sp = outp("dbg_sp", [P, D], F32)
    coeff_dbg = None

    with TileContext(nc) as tc, contextlib.ExitStack() as ctx:
        wpool = ctx.enter_context(tc.tile_pool(name="wpool", bufs=1))
        respool = ctx.enter_context(tc.tile_pool(name="respool", bufs=1))

        # ---- static tiles -------------------------------------------------
        ident = wpool.tile([P, P], F32)
        make_identity(nc, ident[:])
        ident16 = wpool.tile([P, P], BF16)
        make_identity(nc, ident16[:])

        iota_i32 = wpool.tile([P, P], I32)
        nc.gpsimd.iota(iota_i32[:], pattern=[[1, P]], base=0, channel_multiplier=0)
        iota16 = wpool.tile([P, P], BF16)
        nc.vector.tensor_copy(iota16[:], iota_i32[:])
        if debug:
            coeff_dbg = wpool.tile([P, RCHUNKS], F32)

        # persistent weights (tiles allocated here; DMAs issued later, on the
        # scalar HWDGE queue, so the SP queue serves phase-1-critical data)
        wu_sb = wpool.tile([P, DC, D], BF16)
        tr1_sb = respool.tile([P, DC, D // 4], F32R)
        tr2_sb = respool.tile([P, 2, D], F32)
        outw_sb = respool.tile([P, DC, C], F32)
        tr1b_sb = respool.tile([1, D // 4], F32)
        tr2b_sb = respool.tile([P, DC], F32)
        outb_sb = respool.tile([P, 2], F32)

        ones_f = wpool.tile([1, P], F32)
        nc.gpsimd.memset(ones_f[:], 1.0)
        ones16 = wpool.tile([1, P], BF16)
        nc.gpsimd.memset(ones16[:], 1.0)

        # index prep: [P, RCHUNKS, 2] f32 + per-chunk rows
        idx_i = wpool.tile([P, RCHUNKS, 2], I32)
        nc.sync.dma_start(out=idx_i[:], in_=d_idx.rearrange("c p two -> p c two"))
        idx_f = wpool.tile([P, RCHUNKS, 2], F32)
        nc.vector.tensor_copy(idx_f[:], idx_i[:])

        # outputs of phase 1 kept on chip
        repT = wpool.tile([P, DC, P], F32)
        repT16 = wpool.tile([P, DC, P], BF16)
        repTr = wpool.tile([P, DC, P], F32R)
        S16 = wpool.tile([P, D], BF16)
        O16 = wpool.tile([P, D], BF16)
        W3 = wpool.tile([P, D // 2], F32)
        if with_ln_affine:
            trg_b = wpool.tile([P, D // 4], F32)
            trbe_b = wpool.tile([P, D // 4], F32)
        if with_bias_path:
            ST2 = wpool.tile([P, DC, P], BF16)   # (S' * wu_b)^T
            OT2 = wpool.tile([P, DC, P], BF16)   # O^T

        # =================================================================
        # PHASE 1: rep^T, S', O, W3
        # =================================================================
        with tc.tile_pool(name="ph1", bufs=1) as ph1, \
             tc.tile_pool(name="ph1ps", bufs=2, space="PSUM") as ph1ps, \
             tc.tile_pool(name="pwstream", bufs=3) as pwstream:

            # xT = [obj_feats^T ; obj_embed^T ; pos^T]  [P, KPROJ, P]
            xT = ph1.tile([P, KPROJ, P], F32R)
            nc.vector.memset(xT[:].bitcast(F32), 0.0)

            objf = ph1.tile([P, D], F32)
            nc.sync.dma_start(out=objf[:], in_=d_obj[:])
            for c in range(DC):
                ps = ph1ps.tile([P, P], F32, space="PSUM", tag="tp")
                nc.tensor.transpose(ps[:], objf[:, c * P:(c + 1) * P], ident[:])
                nc.scalar.copy(xT[:, c, :], ps[:])

            # obj_dists^T (2 chunks, second is 23 rows zero-padded)
            dists = ph1.tile([P, C], F32)
            nc.sync.dma_start(out=dists[:], in_=d_dists[:])
            odT = ph1.tile([P, 2, P], F32)
            nc.vector.memset(odT[:], 0.0)
            ps = ph1ps.tile([P, P], F32, space="PSUM", tag="tp")
            nc.tensor.transpose(ps[:], dists[:, 0:P], ident[:])
            nc.scalar.copy(odT[:, 0, :], ps[:])
            ps = ph1ps.tile([P, P], F32, space="PSUM", tag="tp")
            nc.tensor.transpose(ps[:23, :], dists[:, P:C], ident[:])
            nc.scalar.copy(odT[:23, 1, :], ps[:23, :])

            # obj_embed^T via embed_w (c chunks padded to 128)
            emb_sb = ph1.tile([P, 2, E_EMB], F32)
            nc.sync.dma_start(out=emb_sb[:], in_=d_emb.rearrange("c p e -> p c e"))
            eps = ph1ps.tile([P, P], F32, space="PSUM", tag="mm")
            for kc in range(2):
                nc.tensor.matmul(eps[:, :], emb_sb[:, kc, 0:P], odT[:, kc, :],
                                 start=(kc == 0), stop=(kc == 1))
            nc.scalar.copy(xT[:, 8, :], eps[:])
            eps = ph1ps.tile([P, P], F32, space="PSUM", tag="mm")
            for kc in range(2):
                nc.tensor.matmul(eps[:72, :], emb_sb[:, kc, P:E_EMB], odT[:, kc, :],
                                 start=(kc == 0), stop=(kc == 1))
            nc.scalar.copy(xT[:72, 9, :], eps[:72, :])

            # pos^T: box^T -> h^T(32) -> pos^T(128) with BN folded on host
            box = ph1.tile([P, PIN], F32)
            nc.sync.dma_start(out=box[:], in_=d_box[:])
            bps = ph1ps.tile([P, P], F32, space="PSUM", tag="tp")
            nc.tensor.transpose(bps[:PIN, :], box[:], ident[:])
            boxT = ph1.tile([PIN, P], F32)
            nc.vector.tensor_copy(boxT[:], bps[:PIN, :])
            pew1 = ph1.tile([PIN, 32], F32)
            nc.sync.dma_start(out=pew1[:], in_=d_pew1[:])
            peb1 = ph1.tile([32, 1], F32)
            nc.sync.dma_start(out=peb1[:], in_=d_peb1[:])
            pew2 = ph1.tile([32, P], F32)
            nc.sync.dma_start(out=pew2[:], in_=d_pew2[:])
            peb2 = ph1.tile([P, 1], F32)
            nc.sync.dma_start(out=peb2[:], in_=d_peb2[:])
            hps = ph1ps.tile([P, P], F32, space="PSUM", tag="mm")
            nc.tensor.matmul(hps[:32, :], pew1[:], boxT[:], start=True, stop=True)
            hT = ph1.tile([32, P], F32)
            nc.scalar.activation(hT[:], hps[:32, :], ACTF.Identity, bias=peb1[:])
            pps = ph1ps.tile([P, P], F32, space="PSUM", tag="mm")
            nc.tensor.matmul(pps[:], pew2[:], hT[:], start=True, stop=True)
            nc.scalar.activation(xT[:, 10, :], pps[:], ACTF.Relu, bias=peb2[:])

            # rep = x @ proj_w: form (i), fp32r full-rate, streamed pw chunks
            # (lhsT = xT chunk stationary, rhs = proj chunk moving N=512)
            rep_ps = [ph1ps.tile([P, 512], F32, space="PSUM", tag=f"rep{h}",
                                 bufs=1, name=f"rep_ps{h}") for h in range(2)]
            for kc in range(KPROJ):
                pwc = pwstream.tile([P, D], F32R, tag="pw")
                nc.sync.dma_start(out=pwc[:], in_=d_pw[kc])
                for h in range(2):
                    nc.tensor.matmul(rep_ps[h][:], xT[:, kc, :],
                                     pwc[:, h * 512:(h + 1) * 512],
                                     start=(kc == 0), stop=False)
            pbrow = ph1.tile([1, D], F32)
            nc.sync.dma_start(out=pbrow[:], in_=d_pbrow[:])
            rep_sb = ph1.tile([P, D], F32)
            for h in range(2):
                nc.tensor.matmul(rep_ps[h][:], ones_f[:],
                                 pbrow[:, h * 512:(h + 1) * 512],
                                 start=False, stop=True)
                nc.scalar.activation(rep_sb[:, h * 512:(h + 1) * 512],
                                     rep_ps[h][:], ACTF.Relu)
            # transpose rep -> repT (+ bf16 / f32r copies for downstream lhsT)
            for m in range(DC):
                tps2 = ph1ps.tile([P, P], F32, space="PSUM", tag="tp")
                nc.tensor.transpose(tps2[:], rep_sb[:, m * P:(m + 1) * P], ident[:])
                nc.scalar.copy(repT[:, m, :], tps2[:])
                nc.vector.tensor_copy(repTr[:, m, :], tps2[:])
            nc.vector.tensor_copy(repT16[:], repT[:])
            if debug:
                nc.sync.dma_start(out=o_dbg_rep[:], in_=repT[:])

            # deferred persistent-weight DMAs (scalar HWDGE queue)
            nc.scalar.dma_start(out=wu_sb[:], in_=d_wu.rearrange("c p e -> p c e"))
            nc.scalar.dma_start(out=tr1_sb[:], in_=d_tr1.rearrange("c p e -> p c e"))
            nc.scalar.dma_start(out=tr2_sb[:], in_=d_tr2.rearrange("c p e -> p c e"))
            nc.scalar.dma_start(out=outw_sb[:], in_=d_outw.rearrange("c p e -> p c e"))
            nc.scalar.dma_start(out=tr1b_sb[:], in_=d_tr1b[:])
            nc.scalar.dma_start(out=tr2b_sb[:], in_=d_tr2b[:])
            nc.scalar.dma_start(out=outb_sb[:], in_=d_outb[:])

            # S' = rep @ ws' + b  (bf16, w_w folded);  O likewise;  W3 fp32
            ws_sb = ph1.tile([P, DC, D], BF16)
            nc.scalar.dma_start(out=ws_sb[:], in_=d_ws.rearrange("c p e -> p c e"))
            wo_sb = ph1.tile([P, DC, D], BF16)
            nc.scalar.dma_start(out=wo_sb[:], in_=d_wo.rearrange("c p e -> p c e"))
            wsb = ph1.tile([1, D], BF16)
            nc.sync.dma_start(out=wsb[:], in_=d_wsb[:])
            wob = ph1.tile([1, D], BF16)
            nc.sync.dma_start(out=wob[:], in_=d_wob[:])

            for (wmat, wbias, dst) in ((ws_sb, wsb, S16), (wo_sb, wob, O16)):
                for h in range(2):
                    sps = ph1ps.tile([P, 512], F32, space="PSUM", tag="so")
                    for kc in range(DC):
                        nc.tensor.matmul(sps[:], repT16[:, kc, :],
                                         wmat[:, kc, h * 512:(h + 1) * 512],
                                         start=(kc == 0), stop=False)
                    nc.tensor.matmul(sps[:], ones16[:],
                                     wbias[:, h * 512:(h + 1) * 512],
                                     start=False, stop=True)
                    nc.vector.tensor_copy(dst[:, h * 512:(h + 1) * 512], sps[:])
            if debug:
                sdbg = ph1.tile([P, D], F32)
                nc.vector.tensor_copy(sdbg[:], S16[:])
                nc.sync.dma_start(out=o_dbg_sp[:], in_=sdbg[:])

            wt3_sb = ph1.tile([P, DC, D // 2], F32R)
            nc.scalar.dma_start(out=wt3_sb[:], in_=d_wt3.rearrange("c p e -> p c e"))
            wt3b_sb = ph1.tile([1, D // 2], F32)
            nc.sync.dma_start(out=wt3b_sb[:], in_=d_wt3b[:])
            wps = ph1ps.tile([P, 512], F32, space="PSUM", tag="so")
            for kc in range(DC):
                nc.tensor.matmul(wps[:], repTr[:, kc, :], wt3_sb[:, kc, :],
                                 start=(kc == 0), stop=False)
            nc.tensor.matmul(wps[:], ones_f[:], wt3b_sb[:], start=False, stop=True)
            nc.scalar.activation(W3[:], wps[:], ACTF.Relu)

            if with_ln_affine:
                trg_row = ph1.tile([1, D // 4], F32)
                nc.sync.dma_start(out=trg_row[:], in_=d_trg[:])
                trbe_row = ph1.tile([1, D // 4], F32)
                nc.sync.dma_start(out=trbe_row[:], in_=d_trbe[:])
                gps = ph1ps.tile([P, 512], F32, space="PSUM", tag="so")
                nc.tensor.matmul(gps[:, :D // 4], ones_f[:], trg_row[:],
                                 start=True, stop=True)
                nc.vector.tensor_copy(trg_b[:], gps[:, :D // 4])
                gps = ph1ps.tile([P, 512], F32, space="PSUM", tag="so")
                nc.tensor.matmul(gps[:, :D // 4], ones_f[:], trbe_row[:],
                                 start=True, stop=True)
                nc.vector.tensor_copy(trbe_b[:], gps[:, :D // 4])

            if with_bias_path:
                # T[i,j] = sum_d (S'*wu_b)[i,d] * O[j,d] needs transposed copies
                wub = ph1.tile([1, D], BF16)
                nc.sync.dma_start(out=wub[:], in_=d_wub[:])
                wub_b = ph1.tile([P, D], BF16)
                bps2 = ph1ps.tile([P, 512], F32, space="PSUM", tag="so")
                for h in range(2):
                    nc.tensor.matmul(bps2[:], ones16[:], wub[:, h * 512:(h + 1) * 512],
                                     start=True, stop=True)
                    nc.vector.tensor_copy(wub_b[:, h * 512:(h + 1) * 512], bps2[:])
                s2 = ph1.tile([P, D], BF16)
                nc.vector.tensor_tensor(s2[:], S16[:], wub_b[:], op=ALU.mult)
                for c in range(DC):
                    tps = ph1ps.tile([P, P], BF16, space="PSUM", tag="tp")
                    nc.tensor.transpose(tps[:], s2[:, c * P:(c + 1) * P], ident16[:])
                    nc.vector.tensor_copy(ST2[:, c, :], tps[:])
                    tps = ph1ps.tile([P, P], BF16, space="PSUM", tag="tp")
                    nc.tensor.transpose(tps[:], O16[:, c * P:(c + 1) * P], ident16[:])
                    nc.vector.tensor_copy(OT2[:, c, :], tps[:])

        # =================================================================
        # PHASE 2: pair loop — gathers, g = Q @ wu^T, coeff, scatter into A
        # =================================================================
        # A and Count in separate PSUM banks (start=True clears a whole bank)
        apool = ctx.enter_context(tc.tile_pool(name="apool", bufs=1, space="PSUM"))
        a_acc = apool.tile([P, P], F32, space="PSUM")
        cnt_acc = (apool.tile([P, P], F32, space="PSUM", name="cnt_acc")
                   if with_bias_path else None)

        with tc.tile_pool(name="p2", bufs=2) as p2, \
             tc.tile_pool(name="p2ps", bufs=3, space="PSUM") as p2ps:

            for g in range(NGROUPS):
                union_sb = p2.tile([P, GC, D], BF16, tag="union")
                nc.sync.dma_start(
                    out=union_sb[:],
                    in_=d_union[g * GC:(g + 1) * GC].rearrange("c p e -> p c e"))

                # one-hot selectors for this group's 512 pairs
                p0t = p2.tile([P, GC * P], BF16, tag="p0t")
                p1t = p2.tile([P, GC * P], BF16, tag="p1t")
                pmats = []  # per chunk: (p0m, p1m) [r,i] one-hots
                for cc in range(GC):
                    ch = g * GC + cc
                    p0col = idx_f[:, ch, 0:1]
                    p1col = idx_f[:, ch, 1:2]
                    # one-hots for all GC chunks of a group stay live until the
                    # group's scatters: bufs must cover 2 groups' worth
                    p0m = p2.tile([P, P], BF16, tag="p0m", bufs=2 * GC + 2)
                    nc.vector.tensor_single_scalar(p0m[:], iota16[:], p0col,
                                                   op=ALU.is_equal)
                    p1m = p2.tile([P, P], BF16, tag="p1m", bufs=2 * GC + 2)
                    nc.vector.tensor_single_scalar(p1m[:], iota16[:], p1col,
                                                   op=ALU.is_equal)
                    tp = p2ps.tile([P, P], BF16, space="PSUM", tag="big")
                    nc.tensor.transpose(tp[:], p0m[:], ident16[:])
                    nc.vector.tensor_copy(p0t[:, cc * P:(cc + 1) * P], tp[:])
                    tp = p2ps.tile([P, P], BF16, space="PSUM", tag="big")
                    nc.tensor.transpose(tp[:], p1m[:], ident16[:])
                    nc.vector.tensor_copy(p1t[:, cc * P:(cc + 1) * P], tp[:])
                    pmats.append((p0m, p1m))

                # gathers: sT/oT [d-chunk, 512] then QT = sT*oT (bf16).
                # DVE can read only one PSUM operand; bounce sT via ACT copy.
                qt = p2.tile([P, DC, GC * P], BF16, tag="qt")
                for dcc in range(DC):
                    sps = p2ps.tile([P, 512], F32, space="PSUM", tag="big")
                    nc.tensor.matmul(sps[:], S16[:, dcc * P:(dcc + 1) * P], p0t[:],
                                     start=True, stop=True)
                    ops_ = p2ps.tile([P, 512], F32, space="PSUM", tag="big")
                    nc.tensor.matmul(ops_[:], O16[:, dcc * P:(dcc + 1) * P], p1t[:],
                                     start=True, stop=True)
                    s_sb = p2.tile([P, 512], BF16, tag="s_sb")
                    nc.scalar.copy(s_sb[:], sps[:])
                    nc.vector.tensor_tensor(qt[:, dcc, :], s_sb[:], ops_[:],
                                            op=ALU.mult)

                # g = Q @ wu^T per 128-pair chunk; coeff = rowsum(union * g)
                for cc in range(GC):
                    ch = g * GC + cc
                    gps = p2ps.tile([P, D], F32, space="PSUM", tag="g", bufs=2)
                    for dcc in range(DC):
                        lhs = qt[:, dcc, cc * P:(cc + 1) * P]
                        nc.tensor.matmul(gps[:, 0:512], lhs, wu_sb[:, dcc, 0:512],
                                         start=(dcc == 0), stop=(dcc == DC - 1))
                        nc.tensor.matmul(gps[:, 512:1024], lhs,
                                         wu_sb[:, dcc, 512:1024],
                                         start=(dcc == 0), stop=(dcc == DC - 1))
                    # fused mul + row-sum via TensorScalarPtr accum_out
                    # (tensor_tensor_reduce is an ISA op this walrus rejects)
                    coeff = p2.tile([P, 1], F32, tag="coeff")
                    scr = p2.tile([P, D], F32, tag="scr")
                    nc.vector.scalar_tensor_tensor(
                        out=scr[:], in0=union_sb[:, cc, :], scalar=1.0,
                        in1=gps[:], op0=ALU.bypass, op1=ALU.mult,
                        accum_out=coeff[:])
                    if debug:
                        nc.vector.tensor_copy(coeff_dbg[:, ch:ch + 1], coeff[:])
                        if ch == RCHUNKS - 1:
                            nc.sync.dma_start(out=o_dbg_coeff[:], in_=coeff_dbg[:])

                    # scatter: A += (P0*coeff)^T @ P1  (and Count += P0^T @ P1)
                    p0m, p1m = pmats[cc]
                    p0c = p2.tile([P, P], BF16, tag="p0c")
                    nc.vector.tensor_tensor(p0c[:], p0m[:],
                                            coeff[:].to_broadcast([P, P]),
                                            op=ALU.mult)
                    first = (ch == 0)
                    last = (ch == RCHUNKS - 1)
                    nc.tensor.matmul(a_acc[:], p0c[:], p1m[:],
                                     start=first, stop=last)
                    if with_bias_path:
                        nc.tensor.matmul(cnt_acc[:], p0m[:], p1m[:],
                                         start=first, stop=last)

        # =================================================================
        # PHASE 3: A -> ctx -> nb -> ctx_rep -> scores/preds
        # =================================================================
        with tc.tile_pool(name="p3", bufs=1) as p3, \
             tc.tile_pool(name="p3ps", bufs=2, space="PSUM") as p3ps:

            if with_bias_path:
                # T^T[j,i] = sum_d O^T[d,j] (S'wu_b)^T[d,i]; A += Count*(T + w_b)
                tt_ps = p3ps.tile([P, P], F32, space="PSUM", tag="mm")
                for dcc in range(DC):
                    nc.tensor.matmul(tt_ps[:], OT2[:, dcc, :], ST2[:, dcc, :],
                                     start=(dcc == 0), stop=(dcc == DC - 1))
                ttsb = p3.tile([P, P], F32)
                nc.vector.tensor_copy(ttsb[:], tt_ps[:])
                t_ps = p3ps.tile([P, P], F32, space="PSUM", tag="mm")
                nc.tensor.transpose(t_ps[:], ttsb[:], ident[:])
                wbsc = p3.tile([1, 1], F32)
                nc.sync.dma_start(out=wbsc[:], in_=d_wb[:])
                wbcol = p3.tile([P, 1], F32)
                bb = p3ps.tile([P, P], F32, space="PSUM", tag="mm2")
                nc.tensor.matmul(bb[:, 0:1], ones_f[:], wbsc[:], start=True, stop=True)
                nc.vector.tensor_copy(wbcol[:], bb[:, 0:1])
                tpw = p3.tile([P, P], F32)
                nc.vector.tensor_scalar_add(tpw[:], t_ps[:], wbcol[:])
                cnt_term = p3.tile([P, P], F32)
                nc.vector.tensor_tensor(cnt_term[:], cnt_acc[:], tpw[:],
                                        op=ALU.mult)
                apre = p3.tile([P, P], F32)
                nc.vector.tensor_tensor(apre[:], a_acc[:], cnt_term[:],
                                        op=ALU.add)
                asig = p3.tile([P, P], F32)
                nc.scalar.activation(asig[:], apre[:], ACTF.Sigmoid)
            else:
                asig = p3.tile([P, P], F32)
                nc.scalar.activation(asig[:], a_acc[:], ACTF.Sigmoid)

            # zero diagonal, row-normalize
            nc.gpsimd.affine_select(out=asig[:], in_=asig[:],
                                    compare_op=ALU.not_equal, fill=0.0,
                                    base=0, pattern=[[-1, P]], channel_multiplier=1)
            rs = p3.tile([P, 1], F32)
            nc.vector.reduce_sum(rs[:], asig[:], axis=mybir.AxisListType.X)
            rsr = p3.tile([P, 1], F32)
            nc.vector.reciprocal(rsr[:], rs[:])
            an = p3.tile([P, P], F32)
            nc.vector.tensor_scalar_mul(an[:], asig[:], rsr[:])
            if debug:
                nc.sync.dma_start(out=o_dbg_an[:], in_=an[:])
            antp = p3ps.tile([P, P], F32, space="PSUM", tag="mm")
            nc.tensor.transpose(antp[:], an[:], ident[:])
            anT = p3.tile([P, P], F32)
            nc.vector.tensor_copy(anT[:], antp[:])

            # ctx^T [d2, i]: first 512 rows from (An@W3)^T, last 512 from (An^T@W3)^T
            ctxT = p3.tile([P, DC, P], F32R)
            for m4 in range(4):
                cps = p3ps.tile([P, P], F32, space="PSUM", tag="mm")
                nc.tensor.matmul(cps[:], W3[:, m4 * P:(m4 + 1) * P], anT[:],
                                 start=True, stop=True)
                nc.vector.tensor_copy(ctxT[:, m4, :], cps[:])
            for m4 in range(4):
                cps = p3ps.tile([P, P], F32, space="PSUM", tag="mm")
                nc.tensor.matmul(cps[:], W3[:, m4 * P:(m4 + 1) * P], an[:],
                                 start=True, stop=True)
                nc.vector.tensor_copy(ctxT[:, 4 + m4, :], cps[:])

            # t = ctx @ tr1 + b; LayerNorm; relu; transpose
            tps_ = p3ps.tile([P, 256], F32, space="PSUM", tag="mm")
            for dcc in range(DC):
                nc.tensor.matmul(tps_[:], ctxT[:, dcc, :], tr1_sb[:, dcc, :],
                                 start=(dcc == 0), stop=False)
            nc.tensor.matmul(tps_[:], ones_f[:], tr1b_sb[:], start=False, stop=True)
            stats = p3.tile([P, 6], F32)
            nc.vector.bn_stats(stats[:], tps_[:])
            aggr = p3.tile([P, 2], F32)
            nc.vector.bn_aggr(aggr[:], stats[:])
            veps = p3.tile([P, 1], F32)
            nc.vector.tensor_scalar_add(veps[:], aggr[:, 1:2], 1e-5)
            stdv = p3.tile([P, 1], F32)
            nc.scalar.sqrt(stdv[:], veps[:])
            rstd = p3.tile([P, 1], F32)
            nc.vector.reciprocal(rstd[:], stdv[:])
            tn = p3.tile([P, 256], F32)
            nc.vector.tensor_scalar(tn[:], tps_[:], aggr[:, 0:1], rstd[:],
                                    op0=ALU.subtract, op1=ALU.mult)
            if with_ln_affine:
                nc.vector.tensor_tensor(tn[:], tn[:], trg_b[:], op=ALU.mult)
                nc.vector.tensor_tensor(tn[:], tn[:], trbe_b[:], op=ALU.add)
            trl = p3.tile([P, 256], F32)
            nc.scalar.activation(trl[:], tn[:], ACTF.Relu)
            rT = p3.tile([P, 2, P], F32)
            for kc in range(2):
                tp2 = p3ps.tile([P, P], F32, space="PSUM", tag="mm")
                nc.tensor.transpose(tp2[:], trl[:, kc * P:(kc + 1) * P], ident[:])
                nc.vector.tensor_copy(rT[:, kc, :], tp2[:])

            # nb^T per d-chunk; ctx_rep^T = relu(rep^T + nb^T + b2)
            ctxrT = p3.tile([P, DC, P], F32)
            for dcc in range(DC):
                nps = p3ps.tile([P, P], F32, space="PSUM", tag="mm")
                for kc in range(2):
                    nc.tensor.matmul(nps[:], tr2_sb[:, kc, dcc * P:(dcc + 1) * P],
                                     rT[:, kc, :], start=(kc == 0), stop=(kc == 1))
                tmp = p3.tile([P, P], F32, tag="nbtmp")
                nc.vector.scalar_tensor_tensor(
                    out=tmp[:], in0=nps[:], scalar=tr2b_sb[:, dcc:dcc + 1],
                    in1=repT[:, dcc, :], op0=ALU.add, op1=ALU.add)
                nc.scalar.activation(ctxrT[:, dcc, :], tmp[:], ACTF.Relu)

            # ctx_rep output (transpose back)
            ctx_sb = p3.tile([P, D], F32)
            for dcc in range(DC):
                cps2 = p3ps.tile([P, P], F32, space="PSUM", tag="mm")
                nc.tensor.transpose(cps2[:], ctxrT[:, dcc, :], ident[:])
                nc.vector.tensor_copy(ctx_sb[:, dcc * P:(dcc + 1) * P], cps2[:])
            nc.sync.dma_start(out=o_ctx[:], in_=ctx_sb[:])

            # scores^T then scores
            scT = p3.tile([P, 2, P], F32)
            for mc in range(2):
                mwid = P if mc == 0 else C - P
                sps2 = p3ps.tile([P, P], F32, space="PSUM", tag="mm")
                for dcc in range(DC):
                    nc.tensor.matmul(sps2[:mwid, :],
                                     outw_sb[:, dcc, mc * P:mc * P + mwid],
                                     ctxrT[:, dcc, :],
                                     start=(dcc == 0), stop=(dcc == DC - 1))
                nc.scalar.activation(scT[:mwid, mc, :], sps2[:mwid, :],
                                     ACTF.Identity, bias=outb_sb[:mwid, mc:mc + 1])
            scores_sb = p3.tile([P, C], F32)
            sps3 = p3ps.tile([P, P], F32, space="PSUM", tag="mm")
            nc.tensor.transpose(sps3[:], scT[:, 0, :], ident[:])
            nc.vector.tensor_copy(scores_sb[:, 0:P], sps3[:])
            sps3 = p3ps.tile([P, P], F32, space="PSUM", tag="mm")
            nc.tensor.transpose(sps3[:, 0:C - P], scT[:C - P, 1, :],
                                ident[:C - P, 0:C - P])
            nc.vector.tensor_copy(scores_sb[:, P:C], sps3[:, 0:C - P])
            nc.sync.dma_start(out=o_scores[:], in_=scores_sb[:])

            # preds = argmax(scores[:,1:]) + 1
            mx8 = p3.tile([P, 8], F32)
            nc.vector.max(mx8[:], scores_sb[:, 1:C])
            mi8 = p3.tile([P, 8], U32)
            nc.vector.max_index(mi8[:], mx8[:], scores_sb[:, 1:C])
            predf = p3.tile([P, 1], I32)
            nc.vector.tensor_single_scalar(predf[:], mi8[:, 0:1], 1, op=ALU.add)
            nc.sync.dma_start(out=o_preds[:], in_=predf[:])

    nsplit = _split_multi_waits(nc)
    if nsplit:
        print(f"[kernel] split {nsplit} extra sync-waits into EventSemaphores")
    return nc


# ---------------------------------------------------------------------------
# Host-side input prep + execution
# ---------------------------------------------------------------------------

_cache = {}


def _prep_weights(inputs):
    f32 = np.float32
    bf16 = ml_dtypes.bfloat16
    w = np.asarray(inputs["w_w"], f32)[:, 0]

    alpha = (np.asarray(inputs["pe_g"], f32) / np.sqrt(np.float32(1.0 + 1e-5)))
    pe_w1f = np.asarray(inputs["pe_w1"], f32) * alpha[None, :]
    pe_b1col = (np.asarray(inputs["pe_b1"], f32) * alpha
                + np.asarray(inputs["pe_be"], f32))[:, None].copy()

    proj_w = np.asarray(inputs["proj_w"], f32)
    pw = np.zeros((KPROJ, P, D), f32)
    pw.reshape(KPROJ * P, D)[0:D] = proj_w[0:D]
    pw.reshape(KPROJ * P, D)[D:D + P] = proj_w[D:D + P]
    pw.reshape(KPROJ * P, D)[9 * P + 0: 9 * P + (E_EMB - P)] = proj_w[D + P:D + E_EMB]
    pw.reshape(KPROJ * P, D)[10 * P:11 * P] = proj_w[D + E_EMB:]

    emb = np.asarray(inputs["embed_w"], f32)
    emb_p = np.zeros((2, P, E_EMB), f32)
    emb_p[0] = emb[0:P]
    emb_p[1, :C - P] = emb[P:C]

    ws16 = (np.asarray(inputs["ws_w"], f32) * w[None, :]).astype(bf16).reshape(DC, P, D)
    wsb16 = (np.asarray(inputs["ws_b"], f32) * w).astype(bf16)[None, :]
    wo16 = np.asarray(inputs["wo_w"], f32).astype(bf16).reshape(DC, P, D)
    wob16 = np.asarray(inputs["wo_b"], f32).astype(bf16)[None, :]
    wu16 = np.ascontiguousarray(np.asarray(inputs["wu_w"], f32).T).astype(bf16)
    wu16 = wu16.reshape(DC, P, D)

    out_b = np.asarray(inputs["out_b"], f32)
    outbcol = np.zeros((P, 2), f32)
    outbcol[:, 0] = out_b[0:P]
    outbcol[:C - P, 1] = out_b[P:C]

    return {
        "pw": pw,
        "proj_bcol": np.ascontiguousarray(
            np.asarray(inputs["proj_b"], f32).reshape(DC, P).T),
        "proj_brow": np.asarray(inputs["proj_b"], f32)[None, :],
        "embed_w_p": emb_p,
        "pe_w1f": pe_w1f,
        "pe_b1col": pe_b1col,
        "pe_w2": np.asarray(inputs["pe_w2"], f32),
        "pe_b2col": np.asarray(inputs["pe_b2"], f32)[:, None].copy(),
        "ws16": ws16, "wsb16": wsb16,
        "wo16": wo16, "wob16": wob16,
        "wu16": wu16,
        "wt3": np.asarray(inputs["wt3_w"], f32).reshape(DC, P, D // 2),
        "wt3b": np.asarray(inputs["wt3_b"], f32)[None, :],
        "tr1": np.asarray(inputs["tr_w1"], f32).reshape(DC, P, D // 4),
        "tr1b": np.asarray(inputs["tr_b1"], f32)[None, :],
        "trg": np.asarray(inputs["tr_g"], f32)[None, :],
        "trbe": np.asarray(inputs["tr_be"], f32)[None, :],
        "tr2": np.asarray(inputs["tr_w2"], f32).reshape(2, P, D),
        "tr2bcol": np.ascontiguousarray(
            np.asarray(inputs["tr_b2"], f32).reshape(DC, P).T),
        "outw": np.asarray(inputs["out_w"], f32).reshape(DC, P, C),
        "outbcol": outbcol,
    }


def kernel(**inputs):
    global last_exec_time_ns, last_trace_path

    f32 = np.float32
    bf16 = ml_dtypes.bfloat16

    wu_b = np.asarray(inputs["wu_b"], f32)
    w_b = np.asarray(inputs["w_b"], f32)
    with_bias_path = bool(np.any(wu_b != 0) or np.any(w_b != 0))
    with_ln_affine = bool(
        np.any(np.asarray(inputs["tr_g"], f32) != 1.0)
        or np.any(np.asarray(inputs["tr_be"], f32) != 0.0))
    debug = bool(int(os.environ.get("DAMP_DEBUG", "0")))
    trace = bool(int(os.environ.get("DAMP_TRACE", "0")))

    key = (with_bias_path, with_ln_affine, debug)
    if key not in _cache:
        _cache[key] = _build_program(with_bias_path, with_ln_affine, debug)
    nc = _cache[key]

    weights = _prep_weights(inputs)
    if with_bias_path:
        weights["wub16"] = wu_b.astype(bf16)[None, :]
        weights["w_b_scalar"] = w_b.reshape(1, 1)

    obj_feats = np.asarray(inputs["obj_feats"], f32)
    obj_dists = np.asarray(inputs["obj_dists"], f32)
    box_info = np.asarray(inputs["box_info"], f32)
    rel_idx = np.asarray(inputs["rel_pair_idx"], np.int32)
    union = np.asarray(inputs["union_feats"], f32)

    in_maps = []
    for b in range(B):
        m = dict(weights)
        m["obj_feats"] = obj_feats[b]
        m["obj_dists"] = obj_dists[b]
        m["box_info"] = box_info[b]
        m["rel_idx"] = np.ascontiguousarray(rel_idx[b].reshape(RCHUNKS, P, 2))
        m["union16"] = np.ascontiguousarray(
            union[b].astype(bf16).reshape(RCHUNKS, P, D))
        in_maps.append(m)

    res = run_bass_kernel_spmd(nc, in_maps, core_ids=list(range(B)), trace=trace)
    global _last_res
    _last_res = res
    last_exec_time_ns = res.exec_time_ns
    if res.instructions_and_trace is not None:
        last_trace_path = res.instructions_and_trace[1]

    scores = np.concatenate([res.results[b]["scores"] for b in range(B)], 0)
    preds = np.concatenate(
        [res.results[b]["preds"][:, 0] for b in range(B)], 0).astype(np.int32)
    ctx_rep = np.concatenate([res.results[b]["ctx_rep"] for b in range(B)], 0)
    return scores, preds, ctx_rep


# revision 28
# speedup vs baseline: 1.0623x; 1.0213x over previous
"""Trainium2 Bass kernel for DirectionAwareMessagePassing (B=8,N=128,R=4096,D=1024).

Sharding: data-parallel over images (1 image per NeuronCore, 8 cores). Weights
replicated. Per image, the math is restructured for the PE array:

  rep^T is kept feature-major on chip (contraction dims must live on SBUF
  partitions). The per-pair coefficient
      coeff_r = (S'[p0_r] * O[p1_r] * u_r) . w  (with u = union @ wu_w)
  is computed without ever transposing union_feats:
      fold w into S:  S' = rep @ (ws_w * w) + ws_b * w
      q_r  = S'[p0_r] * O[p1_r]                  (one-hot gather matmuls)
      g    = Q @ wu_w^T                          (the big [R,D]x[D,D] matmul)
      coeff= rowsum(union * g)                   (DVE fused mul+reduce)
  Gathers are one-hot matmuls (exact selection); the scatter-add into the
  dense [N,N] attention matrix is (P0*coeff)^T @ P1 accumulated in PSUM.

  The coeff/attention path tolerates bf16 (validated: <5e-6 final rel err,
  0 argmax flips), so union_feats / wu / ws / wo / gathers run bf16 at full
  PE rate and union DMA is halved. The rep -> ctx_rep -> scores path stays
  fp32.
"""

import os
import sys
import types
import contextlib

import numpy as np
import ml_dtypes

for _p in ("/opt/trn_rl_repo",):
    if _p not in sys.path:
        sys.path.insert(0, _p)

# ---------------------------------------------------------------------------
# Environment shims for the trimmed trn_rl_repo under axon.
# ---------------------------------------------------------------------------


def _install_shims():
    # 1) antenv.axon_hooks is missing in this image; provide it so
    #    run_bass_kernel_spmd(trace=True) can register the NTFF hook.
    if "antenv.axon_hooks" not in sys.modules:
        _hook = [None]
        mod = types.ModuleType("antenv.axon_hooks")
        mod.set_axon_ntff_profile_hook = lambda h: _hook.__setitem__(0, h)
        mod.get_axon_ntff_profile_hook = lambda: _hook[0]
        sys.modules["antenv.axon_hooks"] = mod
        try:
            from trn_agent_boot.trn_boot import _ntff_profile_via_ctypes

            h = _ntff_profile_via_ctypes("/opt/axon/libaxon_pjrt.so")
            if h is not None:
                mod.set_axon_ntff_profile_hook(h)
        except Exception:
            pass

    # 2) This walrus rejects >1 sync-wait on a CTRL(Drain) instruction, but
    #    Tile's tail drain carries one wait per live semaphore. Split them.
    import concourse.tile as tile
    import concourse.mybir as mybir
    from concourse.vector_clock import ScopedClock

    if getattr(tile.TileContext, "_damp_drain_patched", False):
        return

    def _drain_and_barrier(self, tick_clock, wait_clock):
        nc = self.nc
        drain_inst = nc.sync.drain()
        wait_clock.add_sem_waits(
            drain_inst.ins, ScopedClock({None: tick_clock.global_clock})
        )
        si = drain_inst.ins.sync_info
        waits = list(si.on_wait or []) if si is not None else []
        if len(waits) > 1:
            si.on_wait = waits[:1]
            for w in waits[1:]:
                extra = nc.sync.drain()
                esi = extra.ins.sync_info
                if esi is None:
                    extra.ins.sync_info = mybir.SyncInfo(on_wait=[w], on_update=[])
                else:
                    esi.on_wait = [w]
        nc.all_engine_barrier()
        assert self.sems is not None
        popped = nc._tile_sem_poison_stack.pop()
        assert popped is self._sem_poison
        # clear_and_free_semaphores with a wide range emits a sem_clear ISA
        # whose length this walrus rejects ("ISA wrong length") — clear in
        # chunks of <=4 sems instead.
        sems = list(self.sems.allocated().values())
        if sems:
            sem_nums = [s.num if hasattr(s, "num") else s for s in sems]
            for rg in bass.compact_to_ranges(sem_nums):
                vals = list(rg)
                for i in range(0, len(vals), 4):
                    sub = vals[i:i + 4]
                    subrange = range(sub[0], sub[-1] + 1)
                    nc.gpsimd.dma_reset(subrange)
                    nc.gpsimd.sem_clear(subrange)
            nc._state.prepend_free_semaphores(sem_nums)
            for poison_set in nc._tile_sem_poison_stack:
                poison_set.update(sem_nums)
        nc.all_engine_barrier()

    tile.TileContext._drain_and_barrier = _drain_and_barrier
    tile.TileContext._damp_drain_patched = True


_install_shims()


def _split_multi_waits(nc):
    """This walrus accepts at most one sync-wait per instruction. Move extra
    waits onto freshly inserted same-engine EventSemaphore instructions placed
    immediately before the original (per-engine program order is preserved, so
    semantics are identical)."""
    import concourse.mybir as mybir

    n = 0
    for f in nc.m.functions:
        for bb in f.blocks:
            new = []
            for inst in bb.instructions:
                si = inst.sync_info
                waits = list(si.on_wait) if (si is not None and si.on_wait) else []
                if len(waits) > 1:
                    for w in waits[:-1]:
                        n += 1
                        ev = mybir.InstEventSemaphore(
                            name=f"{inst.name}_sw{n}",
                            opcode="EventSemaphore",
                            engine=inst.engine,
                            sync_info=mybir.SyncInfo(on_wait=[w], on_update=[]),
                        )
                        new.append(ev)
                    si.on_wait = waits[-1:]
                new.append(inst)
            bb.instructions[:] = new
    return n


import concourse.bass as bass  # noqa: E402
import concourse.mybir as mybir  # noqa: E402
import concourse.tile as tile  # noqa: E402
from concourse.bass_utils import run_bass_kernel_spmd  # noqa: E402
from concourse.masks import make_identity  # noqa: E402
from concourse.tile import TileContext  # noqa: E402

F32 = mybir.dt.float32
F32R = mybir.dt.float32r
BF16 = mybir.dt.bfloat16
I32 = mybir.dt.int32
U32 = mybir.dt.uint32
ALU = mybir.AluOpType
ACTF = mybir.ActivationFunctionType

B, N, R, D = 8, 128, 4096, 1024
E_EMB, C, PIN = 200, 151, 9
P = 128
RCHUNKS = R // P            # 32 pair chunks of 128
NGROUPS = 8                 # pair groups of 512 (4 chunks)
GC = RCHUNKS // NGROUPS     # 4 chunks per group
DC = D // P                 # 8 feature chunks
KPROJ = 11                  # padded concat input chunks: 8 obj + 2 emb + 1 pos

last_exec_time_ns = None
last_trace_path = None


# ---------------------------------------------------------------------------
# Program builder
# ---------------------------------------------------------------------------

def _build_program(with_bias_path: bool, with_ln_affine: bool, debug: bool):
    nc = bass.Bass("TRN2", target_bir_lowering=False, debug=False, num_devices=8)

    def inp(name, shape, dt):
        return nc.declare_dram_parameter(name, list(shape), dt, isOutput=False)

    def outp(name, shape, dt):
        return nc.declare_dram_parameter(name, list(shape), dt, isOutput=True)

    d_obj = inp("obj_feats", [P, D], F32)
    d_dists = inp("obj_dists", [P, C], F32)
    d_box = inp("box_info", [P, PIN], F32)
    d_idx = inp("rel_idx", [RCHUNKS, P, 2], I32)
    d_union = inp("union16", [RCHUNKS, P, D], BF16)

    d_pw = inp("pw", [KPROJ, P, D], F32R)
    d_pbcol = inp("proj_bcol", [P, DC], F32)
    d_pbrow = inp("proj_brow", [1, D], F32)
    d_emb = inp("embed_w_p", [2, P, E_EMB], F32)
    d_pew1 = inp("pe_w1f", [PIN, 32], F32)
    d_peb1 = inp("pe_b1col", [32, 1], F32)
    d_pew2 = inp("pe_w2", [32, P], F32)
    d_peb2 = inp("pe_b2col", [P, 1], F32)
    d_ws = inp("ws16", [DC, P, D], BF16)
    d_wsb = inp("wsb16", [1, D], BF16)
    d_wo = inp("wo16", [DC, P, D], BF16)
    d_wob = inp("wob16", [1, D], BF16)
    d_wu = inp("wu16", [DC, P, D], BF16)
    d_wt3 = inp("wt3", [DC, P, D // 2], F32R)
    d_wt3b = inp("wt3b", [1, D // 2], F32)
    d_tr1 = inp("tr1", [DC, P, D // 4], F32R)
    d_tr1b = inp("tr1b", [1, D // 4], F32)
    d_trg = inp("trg", [1, D // 4], F32)
    d_trbe = inp("trbe", [1, D // 4], F32)
    d_tr2 = inp("tr2", [2, P, D], F32)
    d_tr2b = inp("tr2bcol", [P, DC], F32)
    d_outw = inp("outw", [DC, P, C], F32)
    d_outb = inp("outbcol", [P, 2], F32)
    d_wub = inp("wub16", [1, D], BF16) if with_bias_path else None
    d_wb = inp("w_b_scalar", [1, 1], F32) if with_bias_path else None

    o_scores = outp("scores", [P, C], F32)
    o_preds = outp("preds", [P, 1], I32)
    o_ctx = outp("ctx_rep", [P, D], F32)
    if debug:
        o_dbg_rep = outp("dbg_repT", [P, DC, P], F32)
        o_dbg_an = outp("dbg_an", [P, P], F32)
        o_dbg_coeff = outp("dbg_coeff", [P, RCHUNKS], F32)
        o_dbg_sp = outp("dbg_sp", [P, D], F32)
    coeff_dbg = None

    with TileContext(nc) as tc, contextlib.ExitStack() as ctx:
        wpool = ctx.enter_context(tc.tile_pool(name="wpool", bufs=1))
        respool = ctx.enter_context(tc.tile_pool(name="respool", bufs=1))

        # ---- static tiles -------------------------------------------------
        ident = wpool.tile([P, P], F32)
        make_identity(nc, ident[:])
        ident16 = wpool.tile([P, P], BF16)
        make_identity(nc, ident16[:])

        iota_i32 = wpool.tile([P, P], I32)
        nc.gpsimd.iota(iota_i32[:], pattern=[[1, P]], base=0, channel_multiplier=0)
        iota16 = wpool.tile([P, P], BF16)
        nc.vector.tensor_copy(iota16[:], iota_i32[:])
        iotap_i = wpool.tile([P, 1], I32)
        nc.gpsimd.iota(iotap_i[:], pattern=[[0, 1]], base=0, channel_multiplier=1)
        iotap_f = wpool.tile([P, 1], F32)
        nc.vector.tensor_copy(iotap_f[:], iotap_i[:])
        if debug:
            coeff_dbg = wpool.tile([P, RCHUNKS], F32)

        # persistent weights (tiles allocated here; DMAs issued later, on the
        # scalar HWDGE queue, so the SP queue serves phase-1-critical data)
        wu_sb = wpool.tile([P, DC, D], BF16)
        tr1_sb = respool.tile([P, DC, D // 4], F32R)
        tr2_sb = respool.tile([P, 2, D], F32)
        outw_sb = respool.tile([P, DC, C], F32)
        tr1b_sb = respool.tile([1, D // 4], F32)
        tr2b_sb = respool.tile([P, DC], F32)
        outb_sb = respool.tile([P, 2], F32)

        ones_f = wpool.tile([1, P], F32)
        nc.gpsimd.memset(ones_f[:], 1.0)
        ones16 = wpool.tile([1, P], BF16)
        nc.gpsimd.memset(ones16[:], 1.0)

        # index prep: [P, RCHUNKS, 2] f32 + per-chunk rows
        idx_i = wpool.tile([P, RCHUNKS, 2], I32)
        nc.sync.dma_start(out=idx_i[:], in_=d_idx.rearrange("c p two -> p c two"))
        idx_f = wpool.tile([P, RCHUNKS, 2], F32)
        nc.vector.tensor_copy(idx_f[:], idx_i[:])
        idxrows = wpool.tile([1, 2 * RCHUNKS, P], F32)

        # outputs of phase 1 kept on chip
        repT = wpool.tile([P, DC, P], F32)
        repT16 = wpool.tile([P, DC, P], BF16)
        repTr = wpool.tile([P, DC, P], F32R)
        S16 = wpool.tile([P, D], BF16)
        O16 = wpool.tile([P, D], BF16)
        W3 = wpool.tile([P, D // 2], F32)
        if with_ln_affine:
            trg_b = wpool.tile([P, D // 4], F32)
            trbe_b = wpool.tile([P, D // 4], F32)
        if with_bias_path:
            ST2 = wpool.tile([P, DC, P], BF16)   # (S' * wu_b)^T
            OT2 = wpool.tile([P, DC, P], BF16)   # O^T

        # =================================================================
        # PHASE 1: rep^T, S', O, W3
        # =================================================================
        with tc.tile_pool(name="ph1", bufs=1) as ph1, \
             tc.tile_pool(name="ph1ps", bufs=2, space="PSUM") as ph1ps, \
             tc.tile_pool(name="pwstream", bufs=3) as pwstream:

            # xT = [obj_feats^T ; obj_embed^T ; pos^T]  [P, KPROJ, P]
            xT = ph1.tile([P, KPROJ, P], F32R)
            nc.vector.memset(xT[:].bitcast(F32), 0.0)

            # transpose rel indices to rows, bounce to partition 0 so
            # gpsimd.partition_broadcast can read them (needs base partition 0)
            idx_j = ph1.tile([P, 2, RCHUNKS], F32)
            nc.vector.tensor_copy(idx_j[:], idx_f[:].rearrange("p c two -> p two c"))
            itp = ph1ps.tile([P, P], F32, space="PSUM", tag="tp")
            nc.tensor.transpose(itp[:2 * RCHUNKS, :],
                                idx_j[:].rearrange("p two c -> p (two c)"), ident[:])
            idxT_sb = ph1.tile([2 * RCHUNKS, P], F32)
            nc.vector.tensor_copy(idxT_sb[:], itp[:2 * RCHUNKS, :])
            idx_dram = nc.dram_tensor("idx_rows_dram", [2 * RCHUNKS, P], F32)
            nc.sync.dma_start(out=idx_dram[:], in_=idxT_sb[:])
            nc.sync.dma_start(out=idxrows[:],
                              in_=idx_dram[:].rearrange("r p -> (r p)")[None])

            objf = ph1.tile([P, D], F32)
            nc.sync.dma_start(out=objf[:], in_=d_obj[:])
            for c in range(DC):
                ps = ph1ps.tile([P, P], F32, space="PSUM", tag="tp")
                nc.tensor.transpose(ps[:], objf[:, c * P:(c + 1) * P], ident[:])
                nc.scalar.copy(xT[:, c, :], ps[:])

            # obj_dists^T (2 chunks, second is 23 rows zero-padded)
            dists = ph1.tile([P, C], F32)
            nc.sync.dma_start(out=dists[:], in_=d_dists[:])
            odT = ph1.tile([P, 2, P], F32)
            nc.vector.memset(odT[:], 0.0)
            ps = ph1ps.tile([P, P], F32, space="PSUM", tag="tp")
            nc.tensor.transpose(ps[:], dists[:, 0:P], ident[:])
            nc.scalar.copy(odT[:, 0, :], ps[:])
            ps = ph1ps.tile([P, P], F32, space="PSUM", tag="tp")
            nc.tensor.transpose(ps[:23, :], dists[:, P:C], ident[:])
            nc.scalar.copy(odT[:23, 1, :], ps[:23, :])

            # obj_embed^T via embed_w (c chunks padded to 128)
            emb_sb = ph1.tile([P, 2, E_EMB], F32)
            nc.sync.dma_start(out=emb_sb[:], in_=d_emb.rearrange("c p e -> p c e"))
            eps = ph1ps.tile([P, P], F32, space="PSUM", tag="mm")
            for kc in range(2):
                nc.tensor.matmul(eps[:, :], emb_sb[:, kc, 0:P], odT[:, kc, :],
                                 start=(kc == 0), stop=(kc == 1))
            nc.scalar.copy(xT[:, 8, :], eps[:])
            eps = ph1ps.tile([P, P], F32, space="PSUM", tag="mm")
            for kc in range(2):
                nc.tensor.matmul(eps[:72, :], emb_sb[:, kc, P:E_EMB], odT[:, kc, :],
                                 start=(kc == 0), stop=(kc == 1))
            nc.scalar.copy(xT[:72, 9, :], eps[:72, :])

            # pos^T: box^T -> h^T(32) -> pos^T(128) with BN folded on host
            box = ph1.tile([P, PIN], F32)
            nc.sync.dma_start(out=box[:], in_=d_box[:])
            bps = ph1ps.tile([P, P], F32, space="PSUM", tag="tp")
            nc.tensor.transpose(bps[:PIN, :], box[:], ident[:])
            boxT = ph1.tile([PIN, P], F32)
            nc.vector.tensor_copy(boxT[:], bps[:PIN, :])
            pew1 = ph1.tile([PIN, 32], F32)
            nc.sync.dma_start(out=pew1[:], in_=d_pew1[:])
            peb1 = ph1.tile([32, 1], F32)
            nc.sync.dma_start(out=peb1[:], in_=d_peb1[:])
            pew2 = ph1.tile([32, P], F32)
            nc.sync.dma_start(out=pew2[:], in_=d_pew2[:])
            peb2 = ph1.tile([P, 1], F32)
            nc.sync.dma_start(out=peb2[:], in_=d_peb2[:])
            hps = ph1ps.tile([P, P], F32, space="PSUM", tag="mm")
            nc.tensor.matmul(hps[:32, :], pew1[:], boxT[:], start=True, stop=True)
            hT = ph1.tile([32, P], F32)
            nc.scalar.activation(hT[:], hps[:32, :], ACTF.Identity, bias=peb1[:])
            pps = ph1ps.tile([P, P], F32, space="PSUM", tag="mm")
            nc.tensor.matmul(pps[:], pew2[:], hT[:], start=True, stop=True)
            nc.scalar.activation(xT[:, 10, :], pps[:], ACTF.Relu, bias=peb2[:])

            # rep = x @ proj_w: form (i), fp32r full-rate, streamed pw chunks
            # (lhsT = xT chunk stationary, rhs = proj chunk moving N=512)
            rep_ps = [ph1ps.tile([P, 512], F32, space="PSUM", tag=f"rep{h}",
                                 bufs=1, name=f"rep_ps{h}") for h in range(2)]
            for kc in range(KPROJ):
                pwc = pwstream.tile([P, D], F32R, tag="pw")
                nc.sync.dma_start(out=pwc[:], in_=d_pw[kc])
                for h in range(2):
                    nc.tensor.matmul(rep_ps[h][:], xT[:, kc, :],
                                     pwc[:, h * 512:(h + 1) * 512],
                                     start=(kc == 0), stop=False)
            pbrow = ph1.tile([1, D], F32)
            nc.sync.dma_start(out=pbrow[:], in_=d_pbrow[:])
            rep_sb = ph1.tile([P, D], F32)
            for h in range(2):
                nc.tensor.matmul(rep_ps[h][:], ones_f[:],
                                 pbrow[:, h * 512:(h + 1) * 512],
                                 start=False, stop=True)
                nc.scalar.activation(rep_sb[:, h * 512:(h + 1) * 512],
                                     rep_ps[h][:], ACTF.Relu)
            # transpose rep -> repT (+ bf16 / f32r copies for downstream lhsT)
            for m in range(DC):
                tps2 = ph1ps.tile([P, P], F32, space="PSUM", tag="tp")
                nc.tensor.transpose(tps2[:], rep_sb[:, m * P:(m + 1) * P], ident[:])
                nc.scalar.copy(repT[:, m, :], tps2[:])
                nc.vector.tensor_copy(repTr[:, m, :], tps2[:])
            nc.vector.tensor_copy(repT16[:], repT[:])
            if debug:
                nc.sync.dma_start(out=o_dbg_rep[:], in_=repT[:])

            # deferred persistent-weight DMAs (scalar HWDGE queue)
            nc.scalar.dma_start(out=wu_sb[:], in_=d_wu.rearrange("c p e -> p c e"))
            nc.scalar.dma_start(out=tr1_sb[:], in_=d_tr1.rearrange("c p e -> p c e"))
            nc.scalar.dma_start(out=tr2_sb[:], in_=d_tr2.rearrange("c p e -> p c e"))
            nc.scalar.dma_start(out=outw_sb[:], in_=d_outw.rearrange("c p e -> p c e"))
            nc.scalar.dma_start(out=tr1b_sb[:], in_=d_tr1b[:])
            nc.scalar.dma_start(out=tr2b_sb[:], in_=d_tr2b[:])
            nc.scalar.dma_start(out=outb_sb[:], in_=d_outb[:])

            # S' = rep @ ws' + b  (bf16, w_w folded);  O likewise;  W3 fp32
            ws_sb = ph1.tile([P, DC, D], BF16)
            nc.scalar.dma_start(out=ws_sb[:], in_=d_ws.rearrange("c p e -> p c e"))
            wo_sb = ph1.tile([P, DC, D], BF16)
            nc.scalar.dma_start(out=wo_sb[:], in_=d_wo.rearrange("c p e -> p c e"))
            wsb = ph1.tile([1, D], BF16)
            nc.sync.dma_start(out=wsb[:], in_=d_wsb[:])
            wob = ph1.tile([1, D], BF16)
            nc.sync.dma_start(out=wob[:], in_=d_wob[:])

            for (wmat, wbias, dst) in ((ws_sb, wsb, S16), (wo_sb, wob, O16)):
                for h in range(2):
                    sps = ph1ps.tile([P, 512], F32, space="PSUM", tag="so")
                    for kc in range(DC):
                        nc.tensor.matmul(sps[:], repT16[:, kc, :],
                                         wmat[:, kc, h * 512:(h + 1) * 512],
                                         start=(kc == 0), stop=False)
                    nc.tensor.matmul(sps[:], ones16[:],
                                     wbias[:, h * 512:(h + 1) * 512],
                                     start=False, stop=True)
                    nc.vector.tensor_copy(dst[:, h * 512:(h + 1) * 512], sps[:])
            if debug:
                sdbg = ph1.tile([P, D], F32)
                nc.vector.tensor_copy(sdbg[:], S16[:])
                nc.sync.dma_start(out=o_dbg_sp[:], in_=sdbg[:])

            wt3_sb = ph1.tile([P, DC, D // 2], F32R)
            nc.scalar.dma_start(out=wt3_sb[:], in_=d_wt3.rearrange("c p e -> p c e"))
            wt3b_sb = ph1.tile([1, D // 2], F32)
            nc.sync.dma_start(out=wt3b_sb[:], in_=d_wt3b[:])
            wps = ph1ps.tile([P, 512], F32, space="PSUM", tag="so")
            for kc in range(DC):
                nc.tensor.matmul(wps[:], repTr[:, kc, :], wt3_sb[:, kc, :],
                                 start=(kc == 0), stop=False)
            nc.tensor.matmul(wps[:], ones_f[:], wt3b_sb[:], start=False, stop=True)
            nc.scalar.activation(W3[:], wps[:], ACTF.Relu)

            if with_ln_affine:
                trg_row = ph1.tile([1, D // 4], F32)
                nc.sync.dma_start(out=trg_row[:], in_=d_trg[:])
                trbe_row = ph1.tile([1, D // 4], F32)
                nc.sync.dma_start(out=trbe_row[:], in_=d_trbe[:])
                gps = ph1ps.tile([P, 512], F32, space="PSUM", tag="so")
                nc.tensor.matmul(gps[:, :D // 4], ones_f[:], trg_row[:],
                                 start=True, stop=True)
                nc.vector.tensor_copy(trg_b[:], gps[:, :D // 4])
                gps = ph1ps.tile([P, 512], F32, space="PSUM", tag="so")
                nc.tensor.matmul(gps[:, :D // 4], ones_f[:], trbe_row[:],
                                 start=True, stop=True)
                nc.vector.tensor_copy(trbe_b[:], gps[:, :D // 4])

            if with_bias_path:
                # T[i,j] = sum_d (S'*wu_b)[i,d] * O[j,d] needs transposed copies
                wub = ph1.tile([1, D], BF16)
                nc.sync.dma_start(out=wub[:], in_=d_wub[:])
                wub_b = ph1.tile([P, D], BF16)
                bps2 = ph1ps.tile([P, 512], F32, space="PSUM", tag="so")
                for h in range(2):
                    nc.tensor.matmul(bps2[:], ones16[:], wub[:, h * 512:(h + 1) * 512],
                                     start=True, stop=True)
                    nc.vector.tensor_copy(wub_b[:, h * 512:(h + 1) * 512], bps2[:])
                s2 = ph1.tile([P, D], BF16)
                nc.vector.tensor_tensor(s2[:], S16[:], wub_b[:], op=ALU.mult)
                for c in range(DC):
                    tps = ph1ps.tile([P, P], BF16, space="PSUM", tag="tp")
                    nc.tensor.transpose(tps[:], s2[:, c * P:(c + 1) * P], ident16[:])
                    nc.vector.tensor_copy(ST2[:, c, :], tps[:])
                    tps = ph1ps.tile([P, P], BF16, space="PSUM", tag="tp")
                    nc.tensor.transpose(tps[:], O16[:, c * P:(c + 1) * P], ident16[:])
                    nc.vector.tensor_copy(OT2[:, c, :], tps[:])

        # =================================================================
        # PHASE 2: pair loop — gathers, g = Q @ wu^T, coeff, scatter into A
        # =================================================================
        # A and Count in separate PSUM banks (start=True clears a whole bank)
        apool = ctx.enter_context(tc.tile_pool(name="apool", bufs=1, space="PSUM"))
        a_acc = apool.tile([P, P], F32, space="PSUM")
        cnt_acc = (apool.tile([P, P], F32, space="PSUM", name="cnt_acc")
                   if with_bias_path else None)

        with tc.tile_pool(name="p2", bufs=2) as p2, \
             tc.tile_pool(name="p2ps", bufs=2, space="PSUM") as p2ps:

            for g in range(NGROUPS):
                union_sb = p2.tile([P, GC, D], BF16, tag="union")
                nc.sync.dma_start(
                    out=union_sb[:],
                    in_=d_union[g * GC:(g + 1) * GC].rearrange("c p e -> p c e"))

                # one-hot selectors for this group's 512 pairs.
                # P0T[i,r] = (p0[r] == i): gpsimd broadcasts the index row
                # across partitions, DVE compares vs the partition-iota.
                p0t = p2.tile([P, GC * P], BF16, tag="p0t")
                p1t = p2.tile([P, GC * P], BF16, tag="p1t")
                pmats = []  # per chunk: (p0m, p1m) [r,i] one-hots
                for j, pt in ((0, p0t), (1, p1t)):
                    bc = p2ps.tile([P, GC * P], F32, space="PSUM", tag="bc",
                                   bufs=1, name="bc")
                    nc.tensor.matmul(
                        bc[:], ones_f[:],
                        idxrows[0:1, j * RCHUNKS + g * GC:
                                j * RCHUNKS + (g + 1) * GC, :],
                        start=True, stop=True)
                    nc.vector.tensor_tensor(
                        pt[:], bc[:], iotap_f[:].to_broadcast([P, GC * P]),
                        op=ALU.is_equal)
                for cc in range(GC):
                    ch = g * GC + cc
                    p0col = idx_f[:, ch, 0:1]
                    p1col = idx_f[:, ch, 1:2]
                    # one-hots for all GC chunks of a group stay live until the
                    # group's scatters: bufs must cover 2 groups' worth
                    p0m = p2.tile([P, P], BF16, tag="p0m", bufs=2 * GC + 2)
                    nc.vector.tensor_single_scalar(p0m[:], iota16[:], p0col,
                                                   op=ALU.is_equal)
                    p1m = p2.tile([P, P], BF16, tag="p1m", bufs=2 * GC + 2)
                    nc.vector.tensor_single_scalar(p1m[:], iota16[:], p1col,
                                                   op=ALU.is_equal)
                    pmats.append((p0m, p1m))

                # gathers: sT/oT [d-chunk, 512] then QT = sT*oT (bf16).
                # DVE can read only one PSUM operand; bounce sT via ACT copy.
                qt = p2.tile([P, DC, GC * P], BF16, tag="qt")
                for dcc in range(DC):
                    sps = p2ps.tile([P, 512], F32, space="PSUM", tag="big")
                    nc.tensor.matmul(sps[:], S16[:, dcc * P:(dcc + 1) * P], p0t[:],
                                     start=True, stop=True)
                    ops_ = p2ps.tile([P, 512], F32, space="PSUM", tag="big")
                    nc.tensor.matmul(ops_[:], O16[:, dcc * P:(dcc + 1) * P], p1t[:],
                                     start=True, stop=True)
                    s_sb = p2.tile([P, 512], BF16, tag="s_sb")
                    nc.scalar.copy(s_sb[:], sps[:])
                    nc.vector.tensor_tensor(qt[:, dcc, :], s_sb[:], ops_[:],
                                            op=ALU.mult)

                # g = Q @ wu^T per 128-pair chunk; coeff = rowsum(union * g)
                for cc in range(GC):
                    ch = g * GC + cc
                    gps = p2ps.tile([P, D], F32, space="PSUM", tag="g", bufs=2)
                    for dcc in range(DC):
                        lhs = qt[:, dcc, cc * P:(cc + 1) * P]
                        nc.tensor.matmul(gps[:, 0:512], lhs, wu_sb[:, dcc, 0:512],
                                         start=(dcc == 0), stop=(dcc == DC - 1))
                        nc.tensor.matmul(gps[:, 512:1024], lhs,
                                         wu_sb[:, dcc, 512:1024],
                                         start=(dcc == 0), stop=(dcc == DC - 1))
                    # fused mul + row-sum via TensorScalarPtr accum_out
                    # (tensor_tensor_reduce is an ISA op this walrus rejects)
                    coeff = p2.tile([P, 1], F32, tag="coeff")
                    scr = p2.tile([P, D], F32, tag="scr")
                    nc.vector.scalar_tensor_tensor(
                        out=scr[:], in0=union_sb[:, cc, :], scalar=1.0,
                        in1=gps[:], op0=ALU.bypass, op1=ALU.mult,
                        accum_out=coeff[:])
                    if debug:
                        nc.vector.tensor_copy(coeff_dbg[:, ch:ch + 1], coeff[:])
                        if ch == RCHUNKS - 1:
                            nc.sync.dma_start(out=o_dbg_coeff[:], in_=coeff_dbg[:])

                    # scatter: A += (P0*coeff)^T @ P1  (and Count += P0^T @ P1)
                    p0m, p1m = pmats[cc]
                    p0c = p2.tile([P, P], BF16, tag="p0c")
                    nc.vector.tensor_tensor(p0c[:], p0m[:],
                                            coeff[:].to_broadcast([P, P]),
                                            op=ALU.mult)
                    first = (ch == 0)
                    last = (ch == RCHUNKS - 1)
                    nc.tensor.matmul(a_acc[:], p0c[:], p1m[:],
                                     start=first, stop=last)
                    if with_bias_path:
                        nc.tensor.matmul(cnt_acc[:], p0m[:], p1m[:],
                                         start=first, stop=last)

        # =================================================================
        # PHASE 3: A -> ctx -> nb -> ctx_rep -> scores/preds
        # =================================================================
        with tc.tile_pool(name="p3", bufs=1) as p3, \
             tc.tile_pool(name="p3ps", bufs=2, space="PSUM") as p3ps:

            if with_bias_path:
                # T^T[j,i] = sum_d O^T[d,j] (S'wu_b)^T[d,i]; A += Count*(T + w_b)
                tt_ps = p3ps.tile([P, P], F32, space="PSUM", tag="mm")
                for dcc in range(DC):
                    nc.tensor.matmul(tt_ps[:], OT2[:, dcc, :], ST2[:, dcc, :],
                                     start=(dcc == 0), stop=(dcc == DC - 1))
                ttsb = p3.tile([P, P], F32)
                nc.vector.tensor_copy(ttsb[:], tt_ps[:])
                t_ps = p3ps.tile([P, P], F32, space="PSUM", tag="mm")
                nc.tensor.transpose(t_ps[:], ttsb[:], ident[:])
                wbsc = p3.tile([1, 1], F32)
                nc.sync.dma_start(out=wbsc[:], in_=d_wb[:])
                wbcol = p3.tile([P, 1], F32)
                bb = p3ps.tile([P, P], F32, space="PSUM", tag="mm2")
                nc.tensor.matmul(bb[:, 0:1], ones_f[:], wbsc[:], start=True, stop=True)
                nc.vector.tensor_copy(wbcol[:], bb[:, 0:1])
                tpw = p3.tile([P, P], F32)
                nc.vector.tensor_scalar_add(tpw[:], t_ps[:], wbcol[:])
                cnt_term = p3.tile([P, P], F32)
                nc.vector.tensor_tensor(cnt_term[:], cnt_acc[:], tpw[:],
                                        op=ALU.mult)
                apre = p3.tile([P, P], F32)
                nc.vector.tensor_tensor(apre[:], a_acc[:], cnt_term[:],
                                        op=ALU.add)
                asig = p3.tile([P, P], F32)
                nc.scalar.activation(asig[:], apre[:], ACTF.Sigmoid)
            else:
                asig = p3.tile([P, P], F32)
                nc.scalar.activation(asig[:], a_acc[:], ACTF.Sigmoid)

            # zero diagonal, row-normalize
            nc.gpsimd.affine_select(out=asig[:], in_=asig[:],
                                    compare_op=ALU.not_equal, fill=0.0,
                                    base=0, pattern=[[-1, P]], channel_multiplier=1)
            rs = p3.tile([P, 1], F32)
            nc.vector.reduce_sum(rs[:], asig[:], axis=mybir.AxisListType.X)
            rsr = p3.tile([P, 1], F32)
            nc.vector.reciprocal(rsr[:], rs[:])
            an = p3.tile([P, P], F32)
            nc.vector.tensor_scalar_mul(an[:], asig[:], rsr[:])
            if debug:
                nc.sync.dma_start(out=o_dbg_an[:], in_=an[:])
            antp = p3ps.tile([P, P], F32, space="PSUM", tag="mm")
            nc.tensor.transpose(antp[:], an[:], ident[:])
            anT = p3.tile([P, P], F32)
            nc.vector.tensor_copy(anT[:], antp[:])

            # ctx^T [d2, i]: first 512 rows from (An@W3)^T, last 512 from (An^T@W3)^T
            ctxT = p3.tile([P, DC, P], F32R)
            for m4 in range(4):
                cps = p3ps.tile([P, P], F32, space="PSUM", tag="mm")
                nc.tensor.matmul(cps[:], W3[:, m4 * P:(m4 + 1) * P], anT[:],
                                 start=True, stop=True)
                nc.vector.tensor_copy(ctxT[:, m4, :], cps[:])
            for m4 in range(4):
                cps = p3ps.tile([P, P], F32, space="PSUM", tag="mm")
                nc.tensor.matmul(cps[:], W3[:, m4 * P:(m4 + 1) * P], an[:],
                                 start=True, stop=True)
                nc.vector.tensor_copy(ctxT[:, 4 + m4, :], cps[:])

            # t = ctx @ tr1 + b; LayerNorm; relu; transpose
            tps_ = p3ps.tile([P, 256], F32, space="PSUM", tag="mm")
            for dcc in range(DC):
                nc.tensor.matmul(tps_[:], ctxT[:, dcc, :], tr1_sb[:, dcc, :],
                                 start=(dcc == 0), stop=False)
            nc.tensor.matmul(tps_[:], ones_f[:], tr1b_sb[:], start=False, stop=True)
            stats = p3.tile([P, 6], F32)
            nc.vector.bn_stats(stats[:], tps_[:])
            aggr = p3.tile([P, 2], F32)
            nc.vector.bn_aggr(aggr[:], stats[:])
            veps = p3.tile([P, 1], F32)
            nc.vector.tensor_scalar_add(veps[:], aggr[:, 1:2], 1e-5)
            stdv = p3.tile([P, 1], F32)
            nc.scalar.sqrt(stdv[:], veps[:])
            rstd = p3.tile([P, 1], F32)
            nc.vector.reciprocal(rstd[:], stdv[:])
            tn = p3.tile([P, 256], F32)
            nc.vector.tensor_scalar(tn[:], tps_[:], aggr[:, 0:1], rstd[:],
                                    op0=ALU.subtract, op1=ALU.mult)
            if with_ln_affine:
                nc.vector.tensor_tensor(tn[:], tn[:], trg_b[:], op=ALU.mult)
                nc.vector.tensor_tensor(tn[:], tn[:], trbe_b[:], op=ALU.add)
            trl = p3.tile([P, 256], F32)
            nc.scalar.activation(trl[:], tn[:], ACTF.Relu)
            rT = p3.tile([P, 2, P], F32)
            for kc in range(2):
                tp2 = p3ps.tile([P, P], F32, space="PSUM", tag="mm")
                nc.tensor.transpose(tp2[:], trl[:, kc * P:(kc + 1) * P], ident[:])
                nc.vector.tensor_copy(rT[:, kc, :], tp2[:])

            # nb^T per d-chunk; ctx_rep^T = relu(rep^T + nb^T + b2)
            ctxrT = p3.tile([P, DC, P], F32)
            for dcc in range(DC):
                nps = p3ps.tile([P, P], F32, space="PSUM", tag="mm")
                for kc in range(2):
                    nc.tensor.matmul(nps[:], tr2_sb[:, kc, dcc * P:(dcc + 1) * P],
                                     rT[:, kc, :], start=(kc == 0), stop=(kc == 1))
                tmp = p3.tile([P, P], F32, tag="nbtmp")
                nc.vector.scalar_tensor_tensor(
                    out=tmp[:], in0=nps[:], scalar=tr2b_sb[:, dcc:dcc + 1],
                    in1=repT[:, dcc, :], op0=ALU.add, op1=ALU.add)
                nc.scalar.activation(ctxrT[:, dcc, :], tmp[:], ACTF.Relu)

            # ctx_rep output (transpose back)
            ctx_sb = p3.tile([P, D], F32)
            for dcc in range(DC):
                cps2 = p3ps.tile([P, P], F32, space="PSUM", tag="mm")
                nc.tensor.transpose(cps2[:], ctxrT[:, dcc, :], ident[:])
                nc.vector.tensor_copy(ctx_sb[:, dcc * P:(dcc + 1) * P], cps2[:])
            nc.sync.dma_start(out=o_ctx[:], in_=ctx_sb[:])

            # scores^T then scores
            scT = p3.tile([P, 2, P], F32)
            for mc in range(2):
                mwid = P if mc == 0 else C - P
                sps2 = p3ps.tile([P, P], F32, space="PSUM", tag="mm")
                for dcc in range(DC):
                    nc.tensor.matmul(sps2[:mwid, :],
                                     outw_sb[:, dcc, mc * P:mc * P + mwid],
                                     ctxrT[:, dcc, :],
                                     start=(dcc == 0), stop=(dcc == DC - 1))
                nc.scalar.activation(scT[:mwid, mc, :], sps2[:mwid, :],
                                     ACTF.Identity, bias=outb_sb[:mwid, mc:mc + 1])
            scores_sb = p3.tile([P, C], F32)
            sps3 = p3ps.tile([P, P], F32, space="PSUM", tag="mm")
            nc.tensor.transpose(sps3[:], scT[:, 0, :], ident[:])
            nc.vector.tensor_copy(scores_sb[:, 0:P], sps3[:])
            sps3 = p3ps.tile([P, P], F32, space="PSUM", tag="mm")
            nc.tensor.transpose(sps3[:, 0:C - P], scT[:C - P, 1, :],
                                ident[:C - P, 0:C - P])
            nc.vector.tensor_copy(scores_sb[:, P:C], sps3[:, 0:C - P])
            nc.sync.dma_start(out=o_scores[:], in_=scores_sb[:])

            # preds = argmax(scores[:,1:]) + 1
            mx8 = p3.tile([P, 8], F32)
            nc.vector.max(mx8[:], scores_sb[:, 1:C])
            mi8 = p3.tile([P, 8], U32)
            nc.vector.max_index(mi8[:], mx8[:], scores_sb[:, 1:C])
            predf = p3.tile([P, 1], I32)
            nc.vector.tensor_single_scalar(predf[:], mi8[:, 0:1], 1, op=ALU.add)
            nc.sync.dma_start(out=o_preds[:], in_=predf[:])

    nsplit = _split_multi_waits(nc)
    if nsplit:
        print(f"[kernel] split {nsplit} extra sync-waits into EventSemaphores")
    return nc


# ---------------------------------------------------------------------------
# Host-side input prep + execution
# ---------------------------------------------------------------------------

_cache = {}


def _prep_weights(inputs):
    f32 = np.float32
    bf16 = ml_dtypes.bfloat16
    w = np.asarray(inputs["w_w"], f32)[:, 0]

    alpha = (np.asarray(inputs["pe_g"], f32) / np.sqrt(np.float32(1.0 + 1e-5)))
    pe_w1f = np.asarray(inputs["pe_w1"], f32) * alpha[None, :]
    pe_b1col = (np.asarray(inputs["pe_b1"], f32) * alpha
                + np.asarray(inputs["pe_be"], f32))[:, None].copy()

    proj_w = np.asarray(inputs["proj_w"], f32)
    pw = np.zeros((KPROJ, P, D), f32)
    pw.reshape(KPROJ * P, D)[0:D] = proj_w[0:D]
    pw.reshape(KPROJ * P, D)[D:D + P] = proj_w[D:D + P]
    pw.reshape(KPROJ * P, D)[9 * P + 0: 9 * P + (E_EMB - P)] = proj_w[D + P:D + E_EMB]
    pw.reshape(KPROJ * P, D)[10 * P:11 * P] = proj_w[D + E_EMB:]

    emb = np.asarray(inputs["embed_w"], f32)
    emb_p = np.zeros((2, P, E_EMB), f32)
    emb_p[0] = emb[0:P]
    emb_p[1, :C - P] = emb[P:C]

    ws16 = (np.asarray(inputs["ws_w"], f32) * w[None, :]).astype(bf16).reshape(DC, P, D)
    wsb16 = (np.asarray(inputs["ws_b"], f32) * w).astype(bf16)[None, :]
    wo16 = np.asarray(inputs["wo_w"], f32).astype(bf16).reshape(DC, P, D)
    wob16 = np.asarray(inputs["wo_b"], f32).astype(bf16)[None, :]
    wu16 = np.ascontiguousarray(np.asarray(inputs["wu_w"], f32).T).astype(bf16)
    wu16 = wu16.reshape(DC, P, D)

    out_b = np.asarray(inputs["out_b"], f32)
    outbcol = np.zeros((P, 2), f32)
    outbcol[:, 0] = out_b[0:P]
    outbcol[:C - P, 1] = out_b[P:C]

    return {
        "pw": pw,
        "proj_bcol": np.ascontiguousarray(
            np.asarray(inputs["proj_b"], f32).reshape(DC, P).T),
        "proj_brow": np.asarray(inputs["proj_b"], f32)[None, :],
        "embed_w_p": emb_p,
        "pe_w1f": pe_w1f,
        "pe_b1col": pe_b1col,
        "pe_w2": np.asarray(inputs["pe_w2"], f32),
        "pe_b2col": np.asarray(inputs["pe_b2"], f32)[:, None].copy(),
        "ws16": ws16, "wsb16": wsb16,
        "wo16": wo16, "wob16": wob16,
        "wu16": wu16,
        "wt3": np.asarray(inputs["wt3_w"], f32).reshape(DC, P, D // 2),
        "wt3b": np.asarray(inputs["wt3_b"], f32)[None, :],
        "tr1": np.asarray(inputs["tr_w1"], f32).reshape(DC, P, D // 4),
        "tr1b": np.asarray(inputs["tr_b1"], f32)[None, :],
        "trg": np.asarray(inputs["tr_g"], f32)[None, :],
        "trbe": np.asarray(inputs["tr_be"], f32)[None, :],
        "tr2": np.asarray(inputs["tr_w2"], f32).reshape(2, P, D),
        "tr2bcol": np.ascontiguousarray(
            np.asarray(inputs["tr_b2"], f32).reshape(DC, P).T),
        "outw": np.asarray(inputs["out_w"], f32).reshape(DC, P, C),
        "outbcol": outbcol,
    }


def kernel(**inputs):
    global last_exec_time_ns, last_trace_path

    f32 = np.float32
    bf16 = ml_dtypes.bfloat16

    wu_b = np.asarray(inputs["wu_b"], f32)
    w_b = np.asarray(inputs["w_b"], f32)
    with_bias_path = bool(np.any(wu_b != 0) or np.any(w_b != 0))
    with_ln_affine = bool(
        np.any(np.asarray(inputs["tr_g"], f32) != 1.0)
        or np.any(np.asarray(inputs["tr_be"], f32) != 0.0))
    debug = bool(int(os.environ.get("DAMP_DEBUG", "0")))
    trace = bool(int(os.environ.get("DAMP_TRACE", "0")))

    key = (with_bias_path, with_ln_affine, debug)
    if key not in _cache:
        _cache[key] = _build_program(with_bias_path, with_ln_affine, debug)
    nc = _cache[key]

    weights = _prep_weights(inputs)
    if with_bias_path:
        weights["wub16"] = wu_b.astype(bf16)[None, :]
        weights["w_b_scalar"] = w_b.reshape(1, 1)

    obj_feats = np.asarray(inputs["obj_feats"], f32)
    obj_dists = np.asarray(inputs["obj_dists"], f32)
    box_info = np.asarray(inputs["box_info"], f32)
    rel_idx = np.asarray(inputs["rel_pair_idx"], np.int32)
    union = np.asarray(inputs["union_feats"], f32)

    in_maps = []
    for b in range(B):
        m = dict(weights)
        m["obj_feats"] = obj_feats[b]
        m["obj_dists"] = obj_dists[b]
        m["box_info"] = box_info[b]
        m["rel_idx"] = np.ascontiguousarray(rel_idx[b].reshape(RCHUNKS, P, 2))
        m["union16"] = np.ascontiguousarray(
            union[b].astype(bf16).reshape(RCHUNKS, P, D))
        in_maps.append(m)

    res = run_bass_kernel_spmd(nc, in_maps, core_ids=list(range(B)), trace=trace)
    global _last_res
    _last_res = res
    last_exec_time_ns = res.exec_time_ns
    if res.instructions_and_trace is not None:
        last_trace_path = res.instructions_and_trace[1]

    scores = np.concatenate([res.results[b]["scores"] for b in range(B)], 0)
    preds = np.concatenate(
        [res.results[b]["preds"][:, 0] for b in range(B)], 0).astype(np.int32)
    ctx_rep = np.concatenate([res.results[b]["ctx_rep"] for b in range(B)], 0)
    return scores, preds, ctx_rep


# revision 29
# speedup vs baseline: 1.0996x; 1.0351x over previous
"""Trainium2 Bass kernel for DirectionAwareMessagePassing (B=8,N=128,R=4096,D=1024).

Sharding: data-parallel over images (1 image per NeuronCore, 8 cores). Weights
replicated. Per image, the math is restructured for the PE array:

  rep^T is kept feature-major on chip (contraction dims must live on SBUF
  partitions). The per-pair coefficient
      coeff_r = (S'[p0_r] * O[p1_r] * u_r) . w  (with u = union @ wu_w)
  is computed without ever transposing union_feats:
      fold w into S:  S' = rep @ (ws_w * w) + ws_b * w
      q_r  = S'[p0_r] * O[p1_r]                  (one-hot gather matmuls)
      g    = Q @ wu_w^T                          (the big [R,D]x[D,D] matmul)
      coeff= rowsum(union * g)                   (DVE fused mul+reduce)
  Gathers are one-hot matmuls (exact selection); the scatter-add into the
  dense [N,N] attention matrix is (P0*coeff)^T @ P1 accumulated in PSUM.

  The coeff/attention path tolerates bf16 (validated: <5e-6 final rel err,
  0 argmax flips), so union_feats / wu / ws / wo / gathers run bf16 at full
  PE rate and union DMA is halved. The rep -> ctx_rep -> scores path stays
  fp32.
"""

import os
import sys
import types
import contextlib

import numpy as np
import ml_dtypes

for _p in ("/opt/trn_rl_repo",):
    if _p not in sys.path:
        sys.path.insert(0, _p)

# ---------------------------------------------------------------------------
# Environment shims for the trimmed trn_rl_repo under axon.
# ---------------------------------------------------------------------------


def _install_shims():
    # 1) antenv.axon_hooks is missing in this image; provide it so
    #    run_bass_kernel_spmd(trace=True) can register the NTFF hook.
    if "antenv.axon_hooks" not in sys.modules:
        _hook = [None]
        mod = types.ModuleType("antenv.axon_hooks")
        mod.set_axon_ntff_profile_hook = lambda h: _hook.__setitem__(0, h)
        mod.get_axon_ntff_profile_hook = lambda: _hook[0]
        sys.modules["antenv.axon_hooks"] = mod
        try:
            from trn_agent_boot.trn_boot import _ntff_profile_via_ctypes

            h = _ntff_profile_via_ctypes("/opt/axon/libaxon_pjrt.so")
            if h is not None:
                mod.set_axon_ntff_profile_hook(h)
        except Exception:
            pass

    # 2) This walrus rejects >1 sync-wait on a CTRL(Drain) instruction, but
    #    Tile's tail drain carries one wait per live semaphore. Split them.
    import concourse.tile as tile
    import concourse.mybir as mybir
    from concourse.vector_clock import ScopedClock

    if getattr(tile.TileContext, "_damp_drain_patched", False):
        return

    def _drain_and_barrier(self, tick_clock, wait_clock):
        nc = self.nc
        drain_inst = nc.sync.drain()
        wait_clock.add_sem_waits(
            drain_inst.ins, ScopedClock({None: tick_clock.global_clock})
        )
        si = drain_inst.ins.sync_info
        waits = list(si.on_wait or []) if si is not None else []
        if len(waits) > 1:
            si.on_wait = waits[:1]
            for w in waits[1:]:
                extra = nc.sync.drain()
                esi = extra.ins.sync_info
                if esi is None:
                    extra.ins.sync_info = mybir.SyncInfo(on_wait=[w], on_update=[])
                else:
                    esi.on_wait = [w]
        nc.all_engine_barrier()
        assert self.sems is not None
        popped = nc._tile_sem_poison_stack.pop()
        assert popped is self._sem_poison
        # clear_and_free_semaphores with a wide range emits a sem_clear ISA
        # whose length this walrus rejects ("ISA wrong length") — clear in
        # chunks of <=4 sems instead.
        sems = list(self.sems.allocated().values())
        if sems:
            sem_nums = [s.num if hasattr(s, "num") else s for s in sems]
            for rg in bass.compact_to_ranges(sem_nums):
                vals = list(rg)
                for i in range(0, len(vals), 4):
                    sub = vals[i:i + 4]
                    subrange = range(sub[0], sub[-1] + 1)
                    nc.gpsimd.dma_reset(subrange)
                    nc.gpsimd.sem_clear(subrange)
            nc._state.prepend_free_semaphores(sem_nums)
            for poison_set in nc._tile_sem_poison_stack:
                poison_set.update(sem_nums)
        nc.all_engine_barrier()

    tile.TileContext._drain_and_barrier = _drain_and_barrier
    tile.TileContext._damp_drain_patched = True


_install_shims()


def _split_multi_waits(nc):
    """This walrus accepts at most one sync-wait per instruction. Move extra
    waits onto freshly inserted same-engine EventSemaphore instructions placed
    immediately before the original (per-engine program order is preserved, so
    semantics are identical)."""
    import concourse.mybir as mybir

    n = 0
    for f in nc.m.functions:
        for bb in f.blocks:
            new = []
            for inst in bb.instructions:
                si = inst.sync_info
                waits = list(si.on_wait) if (si is not None and si.on_wait) else []
                if len(waits) > 1:
                    for w in waits[:-1]:
                        n += 1
                        ev = mybir.InstEventSemaphore(
                            name=f"{inst.name}_sw{n}",
                            opcode="EventSemaphore",
                            engine=inst.engine,
                            sync_info=mybir.SyncInfo(on_wait=[w], on_update=[]),
                        )
                        new.append(ev)
                    si.on_wait = waits[-1:]
                new.append(inst)
            bb.instructions[:] = new
    return n


import concourse.bass as bass  # noqa: E402
import concourse.mybir as mybir  # noqa: E402
import concourse.tile as tile  # noqa: E402
from concourse.bass_utils import run_bass_kernel_spmd  # noqa: E402
from concourse.masks import make_identity  # noqa: E402
from concourse.tile import TileContext  # noqa: E402

F32 = mybir.dt.float32
F32R = mybir.dt.float32r
BF16 = mybir.dt.bfloat16
I32 = mybir.dt.int32
U32 = mybir.dt.uint32
FP8 = mybir.dt.float8e4
ALU = mybir.AluOpType
ACTF = mybir.ActivationFunctionType

B, N, R, D = 8, 128, 4096, 1024
E_EMB, C, PIN = 200, 151, 9
P = 128
RCHUNKS = R // P            # 32 pair chunks of 128
NGROUPS = 8                 # pair groups of 512 (4 chunks)
GC = RCHUNKS // NGROUPS     # 4 chunks per group
DC = D // P                 # 8 feature chunks
KPROJ = 11                  # padded concat input chunks: 8 obj + 2 emb + 1 pos

last_exec_time_ns = None
last_trace_path = None


# ---------------------------------------------------------------------------
# Program builder
# ---------------------------------------------------------------------------

def _build_program(with_bias_path: bool, with_ln_affine: bool, debug: bool,
                   use_fp8: bool = False):
    nc = bass.Bass("TRN2", target_bir_lowering=False, debug=False, num_devices=8)

    def inp(name, shape, dt):
        return nc.declare_dram_parameter(name, list(shape), dt, isOutput=False)

    def outp(name, shape, dt):
        return nc.declare_dram_parameter(name, list(shape), dt, isOutput=True)

    d_obj = inp("obj_feats", [P, D], F32)
    d_dists = inp("obj_dists", [P, C], F32)
    d_box = inp("box_info", [P, PIN], F32)
    d_idx = inp("rel_idx", [RCHUNKS, P, 2], I32)
    d_union = inp("union16", [RCHUNKS, P, D], BF16)

    d_pw = inp("pw", [KPROJ, P, D], F32R)
    d_pbcol = inp("proj_bcol", [P, DC], F32)
    d_pbrow = inp("proj_brow", [1, D], F32)
    d_emb = inp("embed_w_p", [2, P, E_EMB], F32)
    d_pew1 = inp("pe_w1f", [PIN, 32], F32)
    d_peb1 = inp("pe_b1col", [32, 1], F32)
    d_pew2 = inp("pe_w2", [32, P], F32)
    d_peb2 = inp("pe_b2col", [P, 1], F32)
    d_ws = inp("ws16", [DC, P, D], BF16)
    d_wsb = inp("wsb16", [1, D], BF16)
    d_wo = inp("wo16", [DC, P, D], BF16)
    d_wob = inp("wob16", [1, D], BF16)
    d_wu = inp("wu16", [DC, P, D], FP8 if use_fp8 else BF16)
    d_wt3 = inp("wt3", [DC, P, D // 2], F32R)
    d_wt3b = inp("wt3b", [1, D // 2], F32)
    d_tr1 = inp("tr1", [DC, P, D // 4], F32R)
    d_tr1b = inp("tr1b", [1, D // 4], F32)
    d_trg = inp("trg", [1, D // 4], F32)
    d_trbe = inp("trbe", [1, D // 4], F32)
    d_tr2 = inp("tr2", [2, P, D], F32)
    d_tr2b = inp("tr2bcol", [P, DC], F32)
    d_outw = inp("outw", [DC, P, C], F32)
    d_outb = inp("outbcol", [P, 2], F32)
    d_wub = inp("wub16", [1, D], BF16) if with_bias_path else None
    d_wb = inp("w_b_scalar", [1, 1], F32) if with_bias_path else None

    o_scores = outp("scores", [P, C], F32)
    o_preds = outp("preds", [P, 1], I32)
    o_ctx = outp("ctx_rep", [P, D], F32)
    if debug:
        o_dbg_rep = outp("dbg_repT", [P, DC, P], F32)
        o_dbg_an = outp("dbg_an", [P, P], F32)
        o_dbg_coeff = outp("dbg_coeff", [P, RCHUNKS], F32)
        o_dbg_sp = outp("dbg_sp", [P, D], F32)
    coeff_dbg = None

    with TileContext(nc) as tc, contextlib.ExitStack() as ctx:
        wpool = ctx.enter_context(tc.tile_pool(name="wpool", bufs=1))
        respool = ctx.enter_context(tc.tile_pool(name="respool", bufs=1))

        # ---- static tiles -------------------------------------------------
        ident = wpool.tile([P, P], F32)
        make_identity(nc, ident[:])
        ident16 = wpool.tile([P, P], BF16)
        make_identity(nc, ident16[:])

        iota_i32 = wpool.tile([P, P], I32)
        nc.gpsimd.iota(iota_i32[:], pattern=[[1, P]], base=0, channel_multiplier=0)
        iota16 = wpool.tile([P, P], BF16)
        nc.vector.tensor_copy(iota16[:], iota_i32[:])
        iotap_i = wpool.tile([P, 1], I32)
        nc.gpsimd.iota(iotap_i[:], pattern=[[0, 1]], base=0, channel_multiplier=1)
        iotap_f = wpool.tile([P, 1], F32)
        nc.vector.tensor_copy(iotap_f[:], iotap_i[:])
        if debug:
            coeff_dbg = wpool.tile([P, RCHUNKS], F32)

        # persistent weights (tiles allocated here; DMAs issued later, on the
        # scalar HWDGE queue, so the SP queue serves phase-1-critical data)
        wu_sb = wpool.tile([P, DC, D], FP8 if use_fp8 else BF16)
        tr1_sb = respool.tile([P, DC, D // 4], F32R)
        tr2_sb = respool.tile([P, 2, D], F32)
        outw_sb = respool.tile([P, DC, C], F32)
        tr1b_sb = respool.tile([1, D // 4], F32)
        tr2b_sb = respool.tile([P, DC], F32)
        outb_sb = respool.tile([P, 2], F32)

        ones_f = wpool.tile([1, P], F32)
        nc.gpsimd.memset(ones_f[:], 1.0)
        ones16 = wpool.tile([1, P], BF16)
        nc.gpsimd.memset(ones16[:], 1.0)

        # index prep: [P, RCHUNKS, 2] f32 + per-chunk rows
        idx_i = wpool.tile([P, RCHUNKS, 2], I32)
        nc.sync.dma_start(out=idx_i[:], in_=d_idx.rearrange("c p two -> p c two"))
        idx_f = wpool.tile([P, RCHUNKS, 2], F32)
        nc.vector.tensor_copy(idx_f[:], idx_i[:])
        idxrows = wpool.tile([1, 2 * RCHUNKS, P], F32)

        # outputs of phase 1 kept on chip
        repT = wpool.tile([P, DC, P], F32)
        repT16 = wpool.tile([P, DC, P], BF16)
        repTr = wpool.tile([P, DC, P], F32R)
        S16 = wpool.tile([P, D], BF16)
        O16 = wpool.tile([P, D], BF16)
        W3 = wpool.tile([P, D // 2], F32)
        if with_ln_affine:
            trg_b = wpool.tile([P, D // 4], F32)
            trbe_b = wpool.tile([P, D // 4], F32)
        if with_bias_path:
            ST2 = wpool.tile([P, DC, P], BF16)   # (S' * wu_b)^T
            OT2 = wpool.tile([P, DC, P], BF16)   # O^T

        # =================================================================
        # PHASE 1: rep^T, S', O, W3
        # =================================================================
        with tc.tile_pool(name="ph1", bufs=1) as ph1, \
             tc.tile_pool(name="ph1ps", bufs=2, space="PSUM") as ph1ps, \
             tc.tile_pool(name="pwstream", bufs=3) as pwstream:

            # xT = [obj_feats^T ; obj_embed^T ; pos^T]  [P, KPROJ, P]
            xT = ph1.tile([P, KPROJ, P], F32R)
            nc.vector.memset(xT[:].bitcast(F32), 0.0)

            # transpose rel indices to rows, bounce to partition 0 so
            # gpsimd.partition_broadcast can read them (needs base partition 0)
            idx_j = ph1.tile([P, 2, RCHUNKS], F32)
            nc.vector.tensor_copy(idx_j[:], idx_f[:].rearrange("p c two -> p two c"))
            itp = ph1ps.tile([P, P], F32, space="PSUM", tag="tp")
            nc.tensor.transpose(itp[:2 * RCHUNKS, :],
                                idx_j[:].rearrange("p two c -> p (two c)"), ident[:])
            idxT_sb = ph1.tile([2 * RCHUNKS, P], F32)
            nc.vector.tensor_copy(idxT_sb[:], itp[:2 * RCHUNKS, :])
            idx_dram = nc.dram_tensor("idx_rows_dram", [2 * RCHUNKS, P], F32)
            nc.sync.dma_start(out=idx_dram[:], in_=idxT_sb[:])
            nc.sync.dma_start(out=idxrows[:],
                              in_=idx_dram[:].rearrange("r p -> (r p)")[None])

            objf = ph1.tile([P, D], F32)
            nc.sync.dma_start(out=objf[:], in_=d_obj[:])
            for c in range(DC):
                ps = ph1ps.tile([P, P], F32, space="PSUM", tag="tp")
                nc.tensor.transpose(ps[:], objf[:, c * P:(c + 1) * P], ident[:])
                nc.scalar.copy(xT[:, c, :], ps[:])

            # obj_dists^T (2 chunks, second is 23 rows zero-padded)
            dists = ph1.tile([P, C], F32)
            nc.sync.dma_start(out=dists[:], in_=d_dists[:])
            odT = ph1.tile([P, 2, P], F32)
            nc.vector.memset(odT[:], 0.0)
            ps = ph1ps.tile([P, P], F32, space="PSUM", tag="tp")
            nc.tensor.transpose(ps[:], dists[:, 0:P], ident[:])
            nc.scalar.copy(odT[:, 0, :], ps[:])
            ps = ph1ps.tile([P, P], F32, space="PSUM", tag="tp")
            nc.tensor.transpose(ps[:23, :], dists[:, P:C], ident[:])
            nc.scalar.copy(odT[:23, 1, :], ps[:23, :])

            # obj_embed^T via embed_w (c chunks padded to 128)
            emb_sb = ph1.tile([P, 2, E_EMB], F32)
            nc.sync.dma_start(out=emb_sb[:], in_=d_emb.rearrange("c p e -> p c e"))
            eps = ph1ps.tile([P, P], F32, space="PSUM", tag="mm")
            for kc in range(2):
                nc.tensor.matmul(eps[:, :], emb_sb[:, kc, 0:P], odT[:, kc, :],
                                 start=(kc == 0), stop=(kc == 1))
            nc.scalar.copy(xT[:, 8, :], eps[:])
            eps = ph1ps.tile([P, P], F32, space="PSUM", tag="mm")
            for kc in range(2):
                nc.tensor.matmul(eps[:72, :], emb_sb[:, kc, P:E_EMB], odT[:, kc, :],
                                 start=(kc == 0), stop=(kc == 1))
            nc.scalar.copy(xT[:72, 9, :], eps[:72, :])

            # pos^T: box^T -> h^T(32) -> pos^T(128) with BN folded on host
            box = ph1.tile([P, PIN], F32)
            nc.sync.dma_start(out=box[:], in_=d_box[:])
            bps = ph1ps.tile([P, P], F32, space="PSUM", tag="tp")
            nc.tensor.transpose(bps[:PIN, :], box[:], ident[:])
            boxT = ph1.tile([PIN, P], F32)
            nc.vector.tensor_copy(boxT[:], bps[:PIN, :])
            pew1 = ph1.tile([PIN, 32], F32)
            nc.sync.dma_start(out=pew1[:], in_=d_pew1[:])
            peb1 = ph1.tile([32, 1], F32)
            nc.sync.dma_start(out=peb1[:], in_=d_peb1[:])
            pew2 = ph1.tile([32, P], F32)
            nc.sync.dma_start(out=pew2[:], in_=d_pew2[:])
            peb2 = ph1.tile([P, 1], F32)
            nc.sync.dma_start(out=peb2[:], in_=d_peb2[:])
            hps = ph1ps.tile([P, P], F32, space="PSUM", tag="mm")
            nc.tensor.matmul(hps[:32, :], pew1[:], boxT[:], start=True, stop=True)
            hT = ph1.tile([32, P], F32)
            nc.scalar.activation(hT[:], hps[:32, :], ACTF.Identity, bias=peb1[:])
            pps = ph1ps.tile([P, P], F32, space="PSUM", tag="mm")
            nc.tensor.matmul(pps[:], pew2[:], hT[:], start=True, stop=True)
            nc.scalar.activation(xT[:, 10, :], pps[:], ACTF.Relu, bias=peb2[:])

            # rep = x @ proj_w: form (i), fp32r full-rate, streamed pw chunks
            # (lhsT = xT chunk stationary, rhs = proj chunk moving N=512)
            rep_ps = [ph1ps.tile([P, 512], F32, space="PSUM", tag=f"rep{h}",
                                 bufs=1, name=f"rep_ps{h}") for h in range(2)]
            for kc in range(KPROJ):
                pwc = pwstream.tile([P, D], F32R, tag="pw")
                nc.sync.dma_start(out=pwc[:], in_=d_pw[kc])
                for h in range(2):
                    nc.tensor.matmul(rep_ps[h][:], xT[:, kc, :],
                                     pwc[:, h * 512:(h + 1) * 512],
                                     start=(kc == 0), stop=False)
            pbrow = ph1.tile([1, D], F32)
            nc.sync.dma_start(out=pbrow[:], in_=d_pbrow[:])
            rep_sb = ph1.tile([P, D], F32)
            for h in range(2):
                nc.tensor.matmul(rep_ps[h][:], ones_f[:],
                                 pbrow[:, h * 512:(h + 1) * 512],
                                 start=False, stop=True)
                nc.scalar.activation(rep_sb[:, h * 512:(h + 1) * 512],
                                     rep_ps[h][:], ACTF.Relu)
            # transpose rep -> repT (+ bf16 / f32r copies for downstream lhsT)
            for m in range(DC):
                tps2 = ph1ps.tile([P, P], F32, space="PSUM", tag="tp")
                nc.tensor.transpose(tps2[:], rep_sb[:, m * P:(m + 1) * P], ident[:])
                nc.scalar.copy(repT[:, m, :], tps2[:])
                nc.vector.tensor_copy(repTr[:, m, :], tps2[:])
            nc.vector.tensor_copy(repT16[:], repT[:])
            if debug:
                nc.sync.dma_start(out=o_dbg_rep[:], in_=repT[:])

            # deferred persistent-weight DMAs (scalar HWDGE queue)
            nc.scalar.dma_start(out=wu_sb[:], in_=d_wu.rearrange("c p e -> p c e"))
            nc.scalar.dma_start(out=tr1_sb[:], in_=d_tr1.rearrange("c p e -> p c e"))
            nc.scalar.dma_start(out=tr2_sb[:], in_=d_tr2.rearrange("c p e -> p c e"))
            nc.scalar.dma_start(out=outw_sb[:], in_=d_outw.rearrange("c p e -> p c e"))
            nc.scalar.dma_start(out=tr1b_sb[:], in_=d_tr1b[:])
            nc.scalar.dma_start(out=tr2b_sb[:], in_=d_tr2b[:])
            nc.scalar.dma_start(out=outb_sb[:], in_=d_outb[:])

            # S' = rep @ ws' + b  (bf16, w_w folded);  O likewise;  W3 fp32
            ws_sb = ph1.tile([P, DC, D], BF16)
            nc.scalar.dma_start(out=ws_sb[:], in_=d_ws.rearrange("c p e -> p c e"))
            wo_sb = ph1.tile([P, DC, D], BF16)
            nc.scalar.dma_start(out=wo_sb[:], in_=d_wo.rearrange("c p e -> p c e"))
            wsb = ph1.tile([1, D], BF16)
            nc.sync.dma_start(out=wsb[:], in_=d_wsb[:])
            wob = ph1.tile([1, D], BF16)
            nc.sync.dma_start(out=wob[:], in_=d_wob[:])

            for (wmat, wbias, dst) in ((ws_sb, wsb, S16), (wo_sb, wob, O16)):
                for h in range(2):
                    sps = ph1ps.tile([P, 512], F32, space="PSUM", tag="so")
                    for kc in range(DC):
                        nc.tensor.matmul(sps[:], repT16[:, kc, :],
                                         wmat[:, kc, h * 512:(h + 1) * 512],
                                         start=(kc == 0), stop=False)
                    nc.tensor.matmul(sps[:], ones16[:],
                                     wbias[:, h * 512:(h + 1) * 512],
                                     start=False, stop=True)
                    nc.vector.tensor_copy(dst[:, h * 512:(h + 1) * 512], sps[:])
            if debug:
                sdbg = ph1.tile([P, D], F32)
                nc.vector.tensor_copy(sdbg[:], S16[:])
                nc.sync.dma_start(out=o_dbg_sp[:], in_=sdbg[:])

            wt3_sb = ph1.tile([P, DC, D // 2], F32R)
            nc.scalar.dma_start(out=wt3_sb[:], in_=d_wt3.rearrange("c p e -> p c e"))
            wt3b_sb = ph1.tile([1, D // 2], F32)
            nc.sync.dma_start(out=wt3b_sb[:], in_=d_wt3b[:])
            wps = ph1ps.tile([P, 512], F32, space="PSUM", tag="so")
            for kc in range(DC):
                nc.tensor.matmul(wps[:], repTr[:, kc, :], wt3_sb[:, kc, :],
                                 start=(kc == 0), stop=False)
            nc.tensor.matmul(wps[:], ones_f[:], wt3b_sb[:], start=False, stop=True)
            nc.scalar.activation(W3[:], wps[:], ACTF.Relu)

            if with_ln_affine:
                trg_row = ph1.tile([1, D // 4], F32)
                nc.sync.dma_start(out=trg_row[:], in_=d_trg[:])
                trbe_row = ph1.tile([1, D // 4], F32)
                nc.sync.dma_start(out=trbe_row[:], in_=d_trbe[:])
                gps = ph1ps.tile([P, 512], F32, space="PSUM", tag="so")
                nc.tensor.matmul(gps[:, :D // 4], ones_f[:], trg_row[:],
                                 start=True, stop=True)
                nc.vector.tensor_copy(trg_b[:], gps[:, :D // 4])
                gps = ph1ps.tile([P, 512], F32, space="PSUM", tag="so")
                nc.tensor.matmul(gps[:, :D // 4], ones_f[:], trbe_row[:],
                                 start=True, stop=True)
                nc.vector.tensor_copy(trbe_b[:], gps[:, :D // 4])

            if with_bias_path:
                # T[i,j] = sum_d (S'*wu_b)[i,d] * O[j,d] needs transposed copies
                wub = ph1.tile([1, D], BF16)
                nc.sync.dma_start(out=wub[:], in_=d_wub[:])
                wub_b = ph1.tile([P, D], BF16)
                bps2 = ph1ps.tile([P, 512], F32, space="PSUM", tag="so")
                for h in range(2):
                    nc.tensor.matmul(bps2[:], ones16[:], wub[:, h * 512:(h + 1) * 512],
                                     start=True, stop=True)
                    nc.vector.tensor_copy(wub_b[:, h * 512:(h + 1) * 512], bps2[:])
                s2 = ph1.tile([P, D], BF16)
                nc.vector.tensor_tensor(s2[:], S16[:], wub_b[:], op=ALU.mult)
                for c in range(DC):
                    tps = ph1ps.tile([P, P], BF16, space="PSUM", tag="tp")
                    nc.tensor.transpose(tps[:], s2[:, c * P:(c + 1) * P], ident16[:])
                    nc.vector.tensor_copy(ST2[:, c, :], tps[:])
                    tps = ph1ps.tile([P, P], BF16, space="PSUM", tag="tp")
                    nc.tensor.transpose(tps[:], O16[:, c * P:(c + 1) * P], ident16[:])
                    nc.vector.tensor_copy(OT2[:, c, :], tps[:])

        # =================================================================
        # PHASE 2: pair loop — gathers, g = Q @ wu^T, coeff, scatter into A
        # =================================================================
        # A and Count in separate PSUM banks (start=True clears a whole bank)
        apool = ctx.enter_context(tc.tile_pool(name="apool", bufs=1, space="PSUM"))
        a_acc = apool.tile([P, P], F32, space="PSUM")
        cnt_acc = (apool.tile([P, P], F32, space="PSUM", name="cnt_acc")
                   if with_bias_path else None)

        with tc.tile_pool(name="p2", bufs=2) as p2, \
             tc.tile_pool(name="p2ps", bufs=2, space="PSUM") as p2ps:

            for g in range(NGROUPS):
                union_sb = p2.tile([P, GC, D], BF16, tag="union")
                nc.sync.dma_start(
                    out=union_sb[:],
                    in_=d_union[g * GC:(g + 1) * GC].rearrange("c p e -> p c e"))

                # one-hot selectors for this group's 512 pairs.
                # P0T[i,r] = (p0[r] == i): gpsimd broadcasts the index row
                # across partitions, DVE compares vs the partition-iota.
                p0t = p2.tile([P, GC * P], BF16, tag="p0t")
                p1t = p2.tile([P, GC * P], BF16, tag="p1t")
                pmats = []  # per chunk: (p0m, p1m) [r,i] one-hots
                for j, pt in ((0, p0t), (1, p1t)):
                    bc = p2ps.tile([P, GC * P], F32, space="PSUM", tag="bc",
                                   bufs=1, name="bc")
                    nc.tensor.matmul(
                        bc[:], ones_f[:],
                        idxrows[0:1, j * RCHUNKS + g * GC:
                                j * RCHUNKS + (g + 1) * GC, :],
                        start=True, stop=True)
                    nc.vector.tensor_tensor(
                        pt[:], bc[:], iotap_f[:].to_broadcast([P, GC * P]),
                        op=ALU.is_equal)
                for cc in range(GC):
                    ch = g * GC + cc
                    p0col = idx_f[:, ch, 0:1]
                    p1col = idx_f[:, ch, 1:2]
                    # one-hots for all GC chunks of a group stay live until the
                    # group's scatters: bufs must cover 2 groups' worth
                    p0m = p2.tile([P, P], BF16, tag="p0m", bufs=2 * GC + 2)
                    nc.vector.tensor_single_scalar(p0m[:], iota16[:], p0col,
                                                   op=ALU.is_equal)
                    p1m = p2.tile([P, P], BF16, tag="p1m", bufs=2 * GC + 2)
                    nc.vector.tensor_single_scalar(p1m[:], iota16[:], p1col,
                                                   op=ALU.is_equal)
                    pmats.append((p0m, p1m))

                # gathers: sT/oT [d-chunk, 512] then QT = sT*oT (bf16).
                # DVE can read only one PSUM operand; bounce sT via ACT copy.
                qt = p2.tile([P, DC, GC * P], FP8 if use_fp8 else BF16, tag="qt")
                for dcc in range(DC):
                    sps = p2ps.tile([P, 512], F32, space="PSUM", tag="big")
                    nc.tensor.matmul(sps[:], S16[:, dcc * P:(dcc + 1) * P], p0t[:],
                                     start=True, stop=True)
                    ops_ = p2ps.tile([P, 512], F32, space="PSUM", tag="big")
                    nc.tensor.matmul(ops_[:], O16[:, dcc * P:(dcc + 1) * P], p1t[:],
                                     start=True, stop=True)
                    s_sb = p2.tile([P, 512], BF16, tag="s_sb")
                    nc.scalar.copy(s_sb[:], sps[:])
                    nc.vector.tensor_tensor(qt[:, dcc, :], s_sb[:], ops_[:],
                                            op=ALU.mult)

                # g = Q @ wu^T per 128-pair chunk; coeff = rowsum(union * g)
                for cc in range(GC):
                    ch = g * GC + cc
                    gps = p2ps.tile([P, D], F32, space="PSUM", tag="g", bufs=2)
                    if use_fp8:
                        DR = mybir.MatmulPerfMode.DoubleRow
                        for t in range(DC // 2):
                            lhs3 = qt[:, 2 * t:2 * t + 2, cc * P:(cc + 1) * P]
                            nc.tensor.matmul(
                                gps[:, 0:512], lhs3,
                                wu_sb[:, 2 * t:2 * t + 2, 0:512],
                                perf_mode=DR,
                                start=(t == 0), stop=(t == DC // 2 - 1))
                            nc.tensor.matmul(
                                gps[:, 512:1024], lhs3,
                                wu_sb[:, 2 * t:2 * t + 2, 512:1024],
                                perf_mode=DR,
                                start=(t == 0), stop=(t == DC // 2 - 1))
                    else:
                        for dcc in range(DC):
                            lhs = qt[:, dcc, cc * P:(cc + 1) * P]
                            nc.tensor.matmul(gps[:, 0:512], lhs,
                                             wu_sb[:, dcc, 0:512],
                                             start=(dcc == 0),
                                             stop=(dcc == DC - 1))
                            nc.tensor.matmul(gps[:, 512:1024], lhs,
                                             wu_sb[:, dcc, 512:1024],
                                             start=(dcc == 0),
                                             stop=(dcc == DC - 1))
                    # fused mul + row-sum via TensorScalarPtr accum_out
                    # (tensor_tensor_reduce is an ISA op this walrus rejects)
                    coeff = p2.tile([P, 1], F32, tag="coeff")
                    scr = p2.tile([P, D], F32, tag="scr")
                    nc.vector.scalar_tensor_tensor(
                        out=scr[:], in0=union_sb[:, cc, :],
                        scalar=(1.0 / 4096.0 if use_fp8 else 1.0),
                        in1=gps[:], op0=(ALU.mult if use_fp8 else ALU.bypass),
                        op1=ALU.mult, accum_out=coeff[:])
                    if debug:
                        nc.vector.tensor_copy(coeff_dbg[:, ch:ch + 1], coeff[:])
                        if ch == RCHUNKS - 1:
                            nc.sync.dma_start(out=o_dbg_coeff[:], in_=coeff_dbg[:])

                    # scatter: A += (P0*coeff)^T @ P1  (and Count += P0^T @ P1)
                    p0m, p1m = pmats[cc]
                    p0c = p2.tile([P, P], BF16, tag="p0c")
                    nc.vector.tensor_tensor(p0c[:], p0m[:],
                                            coeff[:].to_broadcast([P, P]),
                                            op=ALU.mult)
                    first = (ch == 0)
                    last = (ch == RCHUNKS - 1)
                    nc.tensor.matmul(a_acc[:], p0c[:], p1m[:],
                                     start=first, stop=last)
                    if with_bias_path:
                        nc.tensor.matmul(cnt_acc[:], p0m[:], p1m[:],
                                         start=first, stop=last)

        # =================================================================
        # PHASE 3: A -> ctx -> nb -> ctx_rep -> scores/preds
        # =================================================================
        with tc.tile_pool(name="p3", bufs=1) as p3, \
             tc.tile_pool(name="p3ps", bufs=2, space="PSUM") as p3ps:

            if with_bias_path:
                # T^T[j,i] = sum_d O^T[d,j] (S'wu_b)^T[d,i]; A += Count*(T + w_b)
                tt_ps = p3ps.tile([P, P], F32, space="PSUM", tag="mm")
                for dcc in range(DC):
                    nc.tensor.matmul(tt_ps[:], OT2[:, dcc, :], ST2[:, dcc, :],
                                     start=(dcc == 0), stop=(dcc == DC - 1))
                ttsb = p3.tile([P, P], F32)
                if use_fp8:
                    nc.scalar.mul(ttsb[:], tt_ps[:], 1.0 / 256.0)
                else:
                    nc.vector.tensor_copy(ttsb[:], tt_ps[:])
                t_ps = p3ps.tile([P, P], F32, space="PSUM", tag="mm")
                nc.tensor.transpose(t_ps[:], ttsb[:], ident[:])
                wbsc = p3.tile([1, 1], F32)
                nc.sync.dma_start(out=wbsc[:], in_=d_wb[:])
                wbcol = p3.tile([P, 1], F32)
                bb = p3ps.tile([P, P], F32, space="PSUM", tag="mm2")
                nc.tensor.matmul(bb[:, 0:1], ones_f[:], wbsc[:], start=True, stop=True)
                nc.vector.tensor_copy(wbcol[:], bb[:, 0:1])
                tpw = p3.tile([P, P], F32)
                nc.vector.tensor_scalar_add(tpw[:], t_ps[:], wbcol[:])
                cnt_term = p3.tile([P, P], F32)
                nc.vector.tensor_tensor(cnt_term[:], cnt_acc[:], tpw[:],
                                        op=ALU.mult)
                apre = p3.tile([P, P], F32)
                nc.vector.tensor_tensor(apre[:], a_acc[:], cnt_term[:],
                                        op=ALU.add)
                asig = p3.tile([P, P], F32)
                nc.scalar.activation(asig[:], apre[:], ACTF.Sigmoid)
            else:
                asig = p3.tile([P, P], F32)
                nc.scalar.activation(asig[:], a_acc[:], ACTF.Sigmoid)

            # zero diagonal, row-normalize
            nc.gpsimd.affine_select(out=asig[:], in_=asig[:],
                                    compare_op=ALU.not_equal, fill=0.0,
                                    base=0, pattern=[[-1, P]], channel_multiplier=1)
            rs = p3.tile([P, 1], F32)
            nc.vector.reduce_sum(rs[:], asig[:], axis=mybir.AxisListType.X)
            rsr = p3.tile([P, 1], F32)
            nc.vector.reciprocal(rsr[:], rs[:])
            an = p3.tile([P, P], F32)
            nc.vector.tensor_scalar_mul(an[:], asig[:], rsr[:])
            if debug:
                nc.sync.dma_start(out=o_dbg_an[:], in_=an[:])
            antp = p3ps.tile([P, P], F32, space="PSUM", tag="mm")
            nc.tensor.transpose(antp[:], an[:], ident[:])
            anT = p3.tile([P, P], F32)
            nc.vector.tensor_copy(anT[:], antp[:])

            # ctx^T [d2, i]: first 512 rows from (An@W3)^T, last 512 from (An^T@W3)^T
            ctxT = p3.tile([P, DC, P], F32R)
            for m4 in range(4):
                cps = p3ps.tile([P, P], F32, space="PSUM", tag="mm")
                nc.tensor.matmul(cps[:], W3[:, m4 * P:(m4 + 1) * P], anT[:],
                                 start=True, stop=True)
                nc.vector.tensor_copy(ctxT[:, m4, :], cps[:])
            for m4 in range(4):
                cps = p3ps.tile([P, P], F32, space="PSUM", tag="mm")
                nc.tensor.matmul(cps[:], W3[:, m4 * P:(m4 + 1) * P], an[:],
                                 start=True, stop=True)
                nc.vector.tensor_copy(ctxT[:, 4 + m4, :], cps[:])

            # t = ctx @ tr1 + b; LayerNorm; relu; transpose
            tps_ = p3ps.tile([P, 256], F32, space="PSUM", tag="mm")
            for dcc in range(DC):
                nc.tensor.matmul(tps_[:], ctxT[:, dcc, :], tr1_sb[:, dcc, :],
                                 start=(dcc == 0), stop=False)
            nc.tensor.matmul(tps_[:], ones_f[:], tr1b_sb[:], start=False, stop=True)
            stats = p3.tile([P, 6], F32)
            nc.vector.bn_stats(stats[:], tps_[:])
            aggr = p3.tile([P, 2], F32)
            nc.vector.bn_aggr(aggr[:], stats[:])
            veps = p3.tile([P, 1], F32)
            nc.vector.tensor_scalar_add(veps[:], aggr[:, 1:2], 1e-5)
            stdv = p3.tile([P, 1], F32)
            nc.scalar.sqrt(stdv[:], veps[:])
            rstd = p3.tile([P, 1], F32)
            nc.vector.reciprocal(rstd[:], stdv[:])
            tn = p3.tile([P, 256], F32)
            nc.vector.tensor_scalar(tn[:], tps_[:], aggr[:, 0:1], rstd[:],
                                    op0=ALU.subtract, op1=ALU.mult)
            if with_ln_affine:
                nc.vector.tensor_tensor(tn[:], tn[:], trg_b[:], op=ALU.mult)
                nc.vector.tensor_tensor(tn[:], tn[:], trbe_b[:], op=ALU.add)
            trl = p3.tile([P, 256], F32)
            nc.scalar.activation(trl[:], tn[:], ACTF.Relu)
            rT = p3.tile([P, 2, P], F32)
            for kc in range(2):
                tp2 = p3ps.tile([P, P], F32, space="PSUM", tag="mm")
                nc.tensor.transpose(tp2[:], trl[:, kc * P:(kc + 1) * P], ident[:])
                nc.vector.tensor_copy(rT[:, kc, :], tp2[:])

            # nb^T per d-chunk; ctx_rep^T = relu(rep^T + nb^T + b2)
            ctxrT = p3.tile([P, DC, P], F32)
            for dcc in range(DC):
                nps = p3ps.tile([P, P], F32, space="PSUM", tag="mm")
                for kc in range(2):
                    nc.tensor.matmul(nps[:], tr2_sb[:, kc, dcc * P:(dcc + 1) * P],
                                     rT[:, kc, :], start=(kc == 0), stop=(kc == 1))
                tmp = p3.tile([P, P], F32, tag="nbtmp")
                nc.vector.scalar_tensor_tensor(
                    out=tmp[:], in0=nps[:], scalar=tr2b_sb[:, dcc:dcc + 1],
                    in1=repT[:, dcc, :], op0=ALU.add, op1=ALU.add)
                nc.scalar.activation(ctxrT[:, dcc, :], tmp[:], ACTF.Relu)

            # ctx_rep output (transpose back)
            ctx_sb = p3.tile([P, D], F32)
            for dcc in range(DC):
                cps2 = p3ps.tile([P, P], F32, space="PSUM", tag="mm")
                nc.tensor.transpose(cps2[:], ctxrT[:, dcc, :], ident[:])
                nc.vector.tensor_copy(ctx_sb[:, dcc * P:(dcc + 1) * P], cps2[:])
            nc.sync.dma_start(out=o_ctx[:], in_=ctx_sb[:])

            # scores^T then scores
            scT = p3.tile([P, 2, P], F32)
            for mc in range(2):
                mwid = P if mc == 0 else C - P
                sps2 = p3ps.tile([P, P], F32, space="PSUM", tag="mm")
                for dcc in range(DC):
                    nc.tensor.matmul(sps2[:mwid, :],
                                     outw_sb[:, dcc, mc * P:mc * P + mwid],
                                     ctxrT[:, dcc, :],
                                     start=(dcc == 0), stop=(dcc == DC - 1))
                nc.scalar.activation(scT[:mwid, mc, :], sps2[:mwid, :],
                                     ACTF.Identity, bias=outb_sb[:mwid, mc:mc + 1])
            scores_sb = p3.tile([P, C], F32)
            sps3 = p3ps.tile([P, P], F32, space="PSUM", tag="mm")
            nc.tensor.transpose(sps3[:], scT[:, 0, :], ident[:])
            nc.vector.tensor_copy(scores_sb[:, 0:P], sps3[:])
            sps3 = p3ps.tile([P, P], F32, space="PSUM", tag="mm")
            nc.tensor.transpose(sps3[:, 0:C - P], scT[:C - P, 1, :],
                                ident[:C - P, 0:C - P])
            nc.vector.tensor_copy(scores_sb[:, P:C], sps3[:, 0:C - P])
            nc.sync.dma_start(out=o_scores[:], in_=scores_sb[:])

            # preds = argmax(scores[:,1:]) + 1
            mx8 = p3.tile([P, 8], F32)
            nc.vector.max(mx8[:], scores_sb[:, 1:C])
            mi8 = p3.tile([P, 8], U32)
            nc.vector.max_index(mi8[:], mx8[:], scores_sb[:, 1:C])
            predf = p3.tile([P, 1], I32)
            nc.vector.tensor_single_scalar(predf[:], mi8[:, 0:1], 1, op=ALU.add)
            nc.sync.dma_start(out=o_preds[:], in_=predf[:])

    nsplit = _split_multi_waits(nc)
    if nsplit:
        print(f"[kernel] split {nsplit} extra sync-waits into EventSemaphores")
    return nc


# ---------------------------------------------------------------------------
# Host-side input prep + execution
# ---------------------------------------------------------------------------

_cache = {}


def _prep_weights(inputs, use_fp8):
    f32 = np.float32
    bf16 = ml_dtypes.bfloat16
    fp8 = ml_dtypes.float8_e4m3
    sc = np.float32(16.0) if use_fp8 else np.float32(1.0)
    w = np.asarray(inputs["w_w"], f32)[:, 0]

    alpha = (np.asarray(inputs["pe_g"], f32) / np.sqrt(np.float32(1.0 + 1e-5)))
    pe_w1f = np.asarray(inputs["pe_w1"], f32) * alpha[None, :]
    pe_b1col = (np.asarray(inputs["pe_b1"], f32) * alpha
                + np.asarray(inputs["pe_be"], f32))[:, None].copy()

    proj_w = np.asarray(inputs["proj_w"], f32)
    pw = np.zeros((KPROJ, P, D), f32)
    pw.reshape(KPROJ * P, D)[0:D] = proj_w[0:D]
    pw.reshape(KPROJ * P, D)[D:D + P] = proj_w[D:D + P]
    pw.reshape(KPROJ * P, D)[9 * P + 0: 9 * P + (E_EMB - P)] = proj_w[D + P:D + E_EMB]
    pw.reshape(KPROJ * P, D)[10 * P:11 * P] = proj_w[D + E_EMB:]

    emb = np.asarray(inputs["embed_w"], f32)
    emb_p = np.zeros((2, P, E_EMB), f32)
    emb_p[0] = emb[0:P]
    emb_p[1, :C - P] = emb[P:C]

    ws16 = (np.asarray(inputs["ws_w"], f32) * w[None, :] * sc).astype(bf16)
    ws16 = ws16.reshape(DC, P, D)
    wsb16 = (np.asarray(inputs["ws_b"], f32) * w * sc).astype(bf16)[None, :]
    wo16 = (np.asarray(inputs["wo_w"], f32) * sc).astype(bf16).reshape(DC, P, D)
    wob16 = (np.asarray(inputs["wo_b"], f32) * sc).astype(bf16)[None, :]
    wuT = np.ascontiguousarray(np.asarray(inputs["wu_w"], f32).T) * sc
    wu16 = wuT.astype(fp8 if use_fp8 else bf16).reshape(DC, P, D)

    out_b = np.asarray(inputs["out_b"], f32)
    outbcol = np.zeros((P, 2), f32)
    outbcol[:, 0] = out_b[0:P]
    outbcol[:C - P, 1] = out_b[P:C]

    return {
        "pw": pw,
        "proj_bcol": np.ascontiguousarray(
            np.asarray(inputs["proj_b"], f32).reshape(DC, P).T),
        "proj_brow": np.asarray(inputs["proj_b"], f32)[None, :],
        "embed_w_p": emb_p,
        "pe_w1f": pe_w1f,
        "pe_b1col": pe_b1col,
        "pe_w2": np.asarray(inputs["pe_w2"], f32),
        "pe_b2col": np.asarray(inputs["pe_b2"], f32)[:, None].copy(),
        "ws16": ws16, "wsb16": wsb16,
        "wo16": wo16, "wob16": wob16,
        "wu16": wu16,
        "wt3": np.asarray(inputs["wt3_w"], f32).reshape(DC, P, D // 2),
        "wt3b": np.asarray(inputs["wt3_b"], f32)[None, :],
        "tr1": np.asarray(inputs["tr_w1"], f32).reshape(DC, P, D // 4),
        "tr1b": np.asarray(inputs["tr_b1"], f32)[None, :],
        "trg": np.asarray(inputs["tr_g"], f32)[None, :],
        "trbe": np.asarray(inputs["tr_be"], f32)[None, :],
        "tr2": np.asarray(inputs["tr_w2"], f32).reshape(2, P, D),
        "tr2bcol": np.ascontiguousarray(
            np.asarray(inputs["tr_b2"], f32).reshape(DC, P).T),
        "outw": np.asarray(inputs["out_w"], f32).reshape(DC, P, C),
        "outbcol": outbcol,
    }


def kernel(**inputs):
    global last_exec_time_ns, last_trace_path

    f32 = np.float32
    bf16 = ml_dtypes.bfloat16

    wu_b = np.asarray(inputs["wu_b"], f32)
    w_b = np.asarray(inputs["w_b"], f32)
    with_bias_path = bool(np.any(wu_b != 0) or np.any(w_b != 0))
    with_ln_affine = bool(
        np.any(np.asarray(inputs["tr_g"], f32) != 1.0)
        or np.any(np.asarray(inputs["tr_be"], f32) != 0.0))
    debug = bool(int(os.environ.get("DAMP_DEBUG", "0")))
    trace = bool(int(os.environ.get("DAMP_TRACE", "0")))
    use_fp8 = bool(int(os.environ.get("DAMP_FP8", "1")))

    key = (with_bias_path, with_ln_affine, debug, use_fp8)
    if key not in _cache:
        _cache[key] = _build_program(with_bias_path, with_ln_affine, debug,
                                     use_fp8=use_fp8)
    nc = _cache[key]

    weights = _prep_weights(inputs, use_fp8)
    if with_bias_path:
        weights["wub16"] = wu_b.astype(bf16)[None, :]
        weights["w_b_scalar"] = w_b.reshape(1, 1)

    obj_feats = np.asarray(inputs["obj_feats"], f32)
    obj_dists = np.asarray(inputs["obj_dists"], f32)
    box_info = np.asarray(inputs["box_info"], f32)
    rel_idx = np.asarray(inputs["rel_pair_idx"], np.int32)
    union = np.asarray(inputs["union_feats"], f32)

    in_maps = []
    for b in range(B):
        m = dict(weights)
        m["obj_feats"] = obj_feats[b]
        m["obj_dists"] = obj_dists[b]
        m["box_info"] = box_info[b]
        m["rel_idx"] = np.ascontiguousarray(rel_idx[b].reshape(RCHUNKS, P, 2))
        m["union16"] = np.ascontiguousarray(
            union[b].astype(bf16).reshape(RCHUNKS, P, D))
        in_maps.append(m)

    res = run_bass_kernel_spmd(nc, in_maps, core_ids=list(range(B)), trace=trace)
    global _last_res
    _last_res = res
    last_exec_time_ns = res.exec_time_ns
    if res.instructions_and_trace is not None:
        last_trace_path = res.instructions_and_trace[1]

    scores = np.concatenate([res.results[b]["scores"] for b in range(B)], 0)
    preds = np.concatenate(
        [res.results[b]["preds"][:, 0] for b in range(B)], 0).astype(np.int32)
    ctx_rep = np.concatenate([res.results[b]["ctx_rep"] for b in range(B)], 0)
    return scores, preds, ctx_rep


# revision 30
# speedup vs baseline: 1.1205x; 1.0190x over previous
"""Trainium2 Bass kernel for DirectionAwareMessagePassing (B=8,N=128,R=4096,D=1024).

Sharding: data-parallel over images (1 image per NeuronCore, 8 cores). Weights
replicated. Per image, the math is restructured for the PE array:

  rep^T is kept feature-major on chip (contraction dims must live on SBUF
  partitions). The per-pair coefficient
      coeff_r = (S'[p0_r] * O[p1_r] * u_r) . w  (with u = union @ wu_w)
  is computed without ever transposing union_feats:
      fold w into S:  S' = rep @ (ws_w * w) + ws_b * w
      q_r  = S'[p0_r] * O[p1_r]                  (one-hot gather matmuls)
      g    = Q @ wu_w^T                          (the big [R,D]x[D,D] matmul)
      coeff= rowsum(union * g)                   (DVE fused mul+reduce)
  Gathers are one-hot matmuls (exact selection); the scatter-add into the
  dense [N,N] attention matrix is (P0*coeff)^T @ P1 accumulated in PSUM.

  The coeff/attention path tolerates bf16 (validated: <5e-6 final rel err,
  0 argmax flips), so union_feats / wu / ws / wo / gathers run bf16 at full
  PE rate and union DMA is halved. The rep -> ctx_rep -> scores path stays
  fp32.
"""

import os
import sys
import types
import contextlib

import numpy as np
import ml_dtypes

for _p in ("/opt/trn_rl_repo",):
    if _p not in sys.path:
        sys.path.insert(0, _p)

# ---------------------------------------------------------------------------
# Environment shims for the trimmed trn_rl_repo under axon.
# ---------------------------------------------------------------------------


def _install_shims():
    # 1) antenv.axon_hooks is missing in this image; provide it so
    #    run_bass_kernel_spmd(trace=True) can register the NTFF hook.
    if "antenv.axon_hooks" not in sys.modules:
        _hook = [None]
        mod = types.ModuleType("antenv.axon_hooks")
        mod.set_axon_ntff_profile_hook = lambda h: _hook.__setitem__(0, h)
        mod.get_axon_ntff_profile_hook = lambda: _hook[0]
        sys.modules["antenv.axon_hooks"] = mod
        try:
            from trn_agent_boot.trn_boot import _ntff_profile_via_ctypes

            h = _ntff_profile_via_ctypes("/opt/axon/libaxon_pjrt.so")
            if h is not None:
                mod.set_axon_ntff_profile_hook(h)
        except Exception:
            pass

    # 2) This walrus rejects >1 sync-wait on a CTRL(Drain) instruction, but
    #    Tile's tail drain carries one wait per live semaphore. Split them.
    import concourse.tile as tile
    import concourse.mybir as mybir
    from concourse.vector_clock import ScopedClock

    if getattr(tile.TileContext, "_damp_drain_patched", False):
        return

    def _drain_and_barrier(self, tick_clock, wait_clock):
        nc = self.nc
        drain_inst = nc.sync.drain()
        wait_clock.add_sem_waits(
            drain_inst.ins, ScopedClock({None: tick_clock.global_clock})
        )
        si = drain_inst.ins.sync_info
        waits = list(si.on_wait or []) if si is not None else []
        if len(waits) > 1:
            si.on_wait = waits[:1]
            for w in waits[1:]:
                extra = nc.sync.drain()
                esi = extra.ins.sync_info
                if esi is None:
                    extra.ins.sync_info = mybir.SyncInfo(on_wait=[w], on_update=[])
                else:
                    esi.on_wait = [w]
        nc.all_engine_barrier()
        assert self.sems is not None
        popped = nc._tile_sem_poison_stack.pop()
        assert popped is self._sem_poison
        # clear_and_free_semaphores with a wide range emits a sem_clear ISA
        # whose length this walrus rejects ("ISA wrong length") — clear in
        # chunks of <=4 sems instead.
        sems = list(self.sems.allocated().values())
        if sems:
            sem_nums = [s.num if hasattr(s, "num") else s for s in sems]
            for rg in bass.compact_to_ranges(sem_nums):
                vals = list(rg)
                for i in range(0, len(vals), 4):
                    sub = vals[i:i + 4]
                    subrange = range(sub[0], sub[-1] + 1)
                    nc.gpsimd.dma_reset(subrange)
                    nc.gpsimd.sem_clear(subrange)
            nc._state.prepend_free_semaphores(sem_nums)
            for poison_set in nc._tile_sem_poison_stack:
                poison_set.update(sem_nums)
        nc.all_engine_barrier()

    tile.TileContext._drain_and_barrier = _drain_and_barrier
    tile.TileContext._damp_drain_patched = True


_install_shims()


def _split_multi_waits(nc):
    """This walrus accepts at most one sync-wait per instruction. Move extra
    waits onto freshly inserted same-engine EventSemaphore instructions placed
    immediately before the original (per-engine program order is preserved, so
    semantics are identical)."""
    import concourse.mybir as mybir

    n = 0
    for f in nc.m.functions:
        for bb in f.blocks:
            new = []
            for inst in bb.instructions:
                si = inst.sync_info
                waits = list(si.on_wait) if (si is not None and si.on_wait) else []
                if len(waits) > 1:
                    for w in waits[:-1]:
                        n += 1
                        ev = mybir.InstEventSemaphore(
                            name=f"{inst.name}_sw{n}",
                            opcode="EventSemaphore",
                            engine=inst.engine,
                            sync_info=mybir.SyncInfo(on_wait=[w], on_update=[]),
                        )
                        new.append(ev)
                    si.on_wait = waits[-1:]
                new.append(inst)
            bb.instructions[:] = new
    return n


import concourse.bass as bass  # noqa: E402
import concourse.mybir as mybir  # noqa: E402
import concourse.tile as tile  # noqa: E402
from concourse.bass_utils import run_bass_kernel_spmd  # noqa: E402
from concourse.masks import make_identity  # noqa: E402
from concourse.tile import TileContext  # noqa: E402

F32 = mybir.dt.float32
F32R = mybir.dt.float32r
BF16 = mybir.dt.bfloat16
I32 = mybir.dt.int32
U32 = mybir.dt.uint32
FP8 = mybir.dt.float8e4
ALU = mybir.AluOpType
ACTF = mybir.ActivationFunctionType

B, N, R, D = 8, 128, 4096, 1024
E_EMB, C, PIN = 200, 151, 9
P = 128
RCHUNKS = R // P            # 32 pair chunks of 128
NGROUPS = 8                 # pair groups of 512 (4 chunks)
GC = RCHUNKS // NGROUPS     # 4 chunks per group
DC = D // P                 # 8 feature chunks
KPROJ = 11                  # padded concat input chunks: 8 obj + 2 emb + 1 pos

last_exec_time_ns = None
last_trace_path = None


# ---------------------------------------------------------------------------
# Program builder
# ---------------------------------------------------------------------------

def _build_program(with_bias_path: bool, with_ln_affine: bool, debug: bool,
                   use_fp8: bool = False):
    nc = bass.Bass("TRN2", target_bir_lowering=False, debug=False, num_devices=8)

    def inp(name, shape, dt):
        return nc.declare_dram_parameter(name, list(shape), dt, isOutput=False)

    def outp(name, shape, dt):
        return nc.declare_dram_parameter(name, list(shape), dt, isOutput=True)

    d_obj = inp("obj_feats", [P, D], F32)
    d_dists = inp("obj_dists", [P, C], F32)
    d_box = inp("box_info", [P, PIN], F32)
    d_idx = inp("rel_idx", [RCHUNKS, P, 2], I32)
    d_union = inp("union16", [RCHUNKS, P, D], BF16)

    d_pw = inp("pw", [KPROJ, P, D], F32R)
    d_pbcol = inp("proj_bcol", [P, DC], F32)
    d_pbrow = inp("proj_brow", [1, D], F32)
    d_emb = inp("embed_w_p", [2, P, E_EMB], F32)
    d_pew1 = inp("pe_w1f", [PIN, 32], F32)
    d_peb1 = inp("pe_b1col", [32, 1], F32)
    d_pew2 = inp("pe_w2", [32, P], F32)
    d_peb2 = inp("pe_b2col", [P, 1], F32)
    d_ws = inp("ws16", [DC, P, D], BF16)
    d_wsb = inp("wsb16", [1, D], BF16)
    d_wo = inp("wo16", [DC, P, D], BF16)
    d_wob = inp("wob16", [1, D], BF16)
    d_wu = inp("wu16", [DC, P, D], FP8 if use_fp8 else BF16)
    d_wt3 = inp("wt3", [DC, P, D // 2], F32R)
    d_wt3b = inp("wt3b", [1, D // 2], F32)
    d_tr1 = inp("tr1", [DC, P, D // 4], F32R)
    d_tr1b = inp("tr1b", [1, D // 4], F32)
    d_trg = inp("trg", [1, D // 4], F32)
    d_trbe = inp("trbe", [1, D // 4], F32)
    d_tr2 = inp("tr2", [2, P, D], F32)
    d_tr2b = inp("tr2bcol", [P, DC], F32)
    d_outw = inp("outw", [DC, P, C], F32)
    d_outb = inp("outbcol", [P, 2], F32)
    d_wub = inp("wub16", [1, D], BF16) if with_bias_path else None
    d_wb = inp("w_b_scalar", [1, 1], F32) if with_bias_path else None

    o_scores = outp("scores", [P, C], F32)
    o_preds = outp("preds", [P, 1], I32)
    o_ctx = outp("ctx_rep", [P, D], F32)
    if debug:
        o_dbg_rep = outp("dbg_repT", [P, DC, P], F32)
        o_dbg_an = outp("dbg_an", [P, P], F32)
        o_dbg_coeff = outp("dbg_coeff", [P, RCHUNKS], F32)
        o_dbg_sp = outp("dbg_sp", [P, D], F32)
    coeff_dbg = None

    with TileContext(nc) as tc, contextlib.ExitStack() as ctx:
        wpool = ctx.enter_context(tc.tile_pool(name="wpool", bufs=1))
        respool = ctx.enter_context(tc.tile_pool(name="respool", bufs=1))

        # ---- static tiles -------------------------------------------------
        ident = wpool.tile([P, P], F32)
        make_identity(nc, ident[:])
        ident16 = wpool.tile([P, P], BF16)
        make_identity(nc, ident16[:])

        iota_i32 = wpool.tile([P, P], I32)
        nc.gpsimd.iota(iota_i32[:], pattern=[[1, P]], base=0, channel_multiplier=0)
        iota16 = wpool.tile([P, P], BF16)
        nc.vector.tensor_copy(iota16[:], iota_i32[:])
        iotap_i = wpool.tile([P, 1], I32)
        nc.gpsimd.iota(iotap_i[:], pattern=[[0, 1]], base=0, channel_multiplier=1)
        iotap_f = wpool.tile([P, 1], F32)
        nc.vector.tensor_copy(iotap_f[:], iotap_i[:])
        if debug:
            coeff_dbg = wpool.tile([P, RCHUNKS], F32)

        # persistent weights (tiles allocated here; DMAs issued later, on the
        # scalar HWDGE queue, so the SP queue serves phase-1-critical data)
        wu_sb = wpool.tile([P, DC, D], FP8 if use_fp8 else BF16)
        tr1_sb = respool.tile([P, DC, D // 4], F32R)
        tr2_sb = respool.tile([P, 2, D], F32)
        outw_sb = respool.tile([P, DC, C], F32)
        tr1b_sb = respool.tile([1, D // 4], F32)
        tr2b_sb = respool.tile([P, DC], F32)
        outb_sb = respool.tile([P, 2], F32)

        ones_f = wpool.tile([1, P], F32)
        nc.gpsimd.memset(ones_f[:], 1.0)
        ones16 = wpool.tile([1, P], BF16)
        nc.gpsimd.memset(ones16[:], 1.0)

        # index prep: [P, RCHUNKS, 2] f32 + per-chunk rows
        idx_i = wpool.tile([P, RCHUNKS, 2], I32)
        nc.sync.dma_start(out=idx_i[:], in_=d_idx.rearrange("c p two -> p c two"))
        idx_f = wpool.tile([P, RCHUNKS, 2], F32)
        nc.vector.tensor_copy(idx_f[:], idx_i[:])
        idxrows = wpool.tile([1, 2 * RCHUNKS, P], F32)

        # outputs of phase 1 kept on chip
        repT = wpool.tile([P, DC, P], F32)
        repT16 = wpool.tile([P, DC, P], BF16)
        repTr = wpool.tile([P, DC, P], F32R)
        S16 = wpool.tile([P, D], BF16)
        O16 = wpool.tile([P, D], BF16)
        W3 = wpool.tile([P, D // 2], F32)
        if with_ln_affine:
            trg_b = wpool.tile([P, D // 4], F32)
            trbe_b = wpool.tile([P, D // 4], F32)
        if with_bias_path:
            ST2 = wpool.tile([P, DC, P], BF16)   # (S' * wu_b)^T
            OT2 = wpool.tile([P, DC, P], BF16)   # O^T

        # =================================================================
        # PHASE 1: rep^T, S', O, W3
        # =================================================================
        with tc.tile_pool(name="ph1", bufs=1) as ph1, \
             tc.tile_pool(name="ph1ps", bufs=2, space="PSUM") as ph1ps, \
             tc.tile_pool(name="pwstream", bufs=3) as pwstream:

            # xT = [obj_feats^T ; obj_embed^T ; pos^T]  [P, KPROJ, P]
            xT = ph1.tile([P, KPROJ, P], F32R)
            nc.vector.memset(xT[:].bitcast(F32), 0.0)

            # transpose rel indices to rows, bounce to partition 0 so
            # gpsimd.partition_broadcast can read them (needs base partition 0)
            idx_j = ph1.tile([P, 2, RCHUNKS], F32)
            nc.vector.tensor_copy(idx_j[:], idx_f[:].rearrange("p c two -> p two c"))
            itp = ph1ps.tile([P, P], F32, space="PSUM", tag="tp")
            nc.tensor.transpose(itp[:2 * RCHUNKS, :],
                                idx_j[:].rearrange("p two c -> p (two c)"), ident[:])
            idxT_sb = ph1.tile([2 * RCHUNKS, P], F32)
            nc.vector.tensor_copy(idxT_sb[:], itp[:2 * RCHUNKS, :])
            idx_dram = nc.dram_tensor("idx_rows_dram", [2 * RCHUNKS, P], F32)
            nc.sync.dma_start(out=idx_dram[:], in_=idxT_sb[:])
            nc.sync.dma_start(out=idxrows[:],
                              in_=idx_dram[:].rearrange("r p -> (r p)")[None])

            objf = ph1.tile([P, D], F32)
            nc.sync.dma_start(out=objf[:], in_=d_obj[:])
            for c in range(DC):
                ps = ph1ps.tile([P, P], F32, space="PSUM", tag="tp")
                nc.tensor.transpose(ps[:], objf[:, c * P:(c + 1) * P], ident[:])
                nc.scalar.copy(xT[:, c, :], ps[:])

            # obj_dists^T (2 chunks, second is 23 rows zero-padded)
            dists = ph1.tile([P, C], F32)
            nc.sync.dma_start(out=dists[:], in_=d_dists[:])
            odT = ph1.tile([P, 2, P], F32)
            nc.vector.memset(odT[:], 0.0)
            ps = ph1ps.tile([P, P], F32, space="PSUM", tag="tp")
            nc.tensor.transpose(ps[:], dists[:, 0:P], ident[:])
            nc.scalar.copy(odT[:, 0, :], ps[:])
            ps = ph1ps.tile([P, P], F32, space="PSUM", tag="tp")
            nc.tensor.transpose(ps[:23, :], dists[:, P:C], ident[:])
            nc.scalar.copy(odT[:23, 1, :], ps[:23, :])

            # obj_embed^T via embed_w (c chunks padded to 128)
            emb_sb = ph1.tile([P, 2, E_EMB], F32)
            nc.sync.dma_start(out=emb_sb[:], in_=d_emb.rearrange("c p e -> p c e"))
            eps = ph1ps.tile([P, P], F32, space="PSUM", tag="mm")
            for kc in range(2):
                nc.tensor.matmul(eps[:, :], emb_sb[:, kc, 0:P], odT[:, kc, :],
                                 start=(kc == 0), stop=(kc == 1))
            nc.scalar.copy(xT[:, 8, :], eps[:])
            eps = ph1ps.tile([P, P], F32, space="PSUM", tag="mm")
            for kc in range(2):
                nc.tensor.matmul(eps[:72, :], emb_sb[:, kc, P:E_EMB], odT[:, kc, :],
                                 start=(kc == 0), stop=(kc == 1))
            nc.scalar.copy(xT[:72, 9, :], eps[:72, :])

            # pos^T: box^T -> h^T(32) -> pos^T(128) with BN folded on host
            box = ph1.tile([P, PIN], F32)
            nc.sync.dma_start(out=box[:], in_=d_box[:])
            bps = ph1ps.tile([P, P], F32, space="PSUM", tag="tp")
            nc.tensor.transpose(bps[:PIN, :], box[:], ident[:])
            boxT = ph1.tile([PIN, P], F32)
            nc.vector.tensor_copy(boxT[:], bps[:PIN, :])
            pew1 = ph1.tile([PIN, 32], F32)
            nc.sync.dma_start(out=pew1[:], in_=d_pew1[:])
            peb1 = ph1.tile([32, 1], F32)
            nc.sync.dma_start(out=peb1[:], in_=d_peb1[:])
            pew2 = ph1.tile([32, P], F32)
            nc.sync.dma_start(out=pew2[:], in_=d_pew2[:])
            peb2 = ph1.tile([P, 1], F32)
            nc.sync.dma_start(out=peb2[:], in_=d_peb2[:])
            hps = ph1ps.tile([P, P], F32, space="PSUM", tag="mm")
            nc.tensor.matmul(hps[:32, :], pew1[:], boxT[:], start=True, stop=True)
            hT = ph1.tile([32, P], F32)
            nc.scalar.activation(hT[:], hps[:32, :], ACTF.Identity, bias=peb1[:])
            pps = ph1ps.tile([P, P], F32, space="PSUM", tag="mm")
            nc.tensor.matmul(pps[:], pew2[:], hT[:], start=True, stop=True)
            nc.scalar.activation(xT[:, 10, :], pps[:], ACTF.Relu, bias=peb2[:])

            # rep = x @ proj_w: form (i), fp32r full-rate, streamed pw chunks
            # (lhsT = xT chunk stationary, rhs = proj chunk moving N=512)
            rep_ps = [ph1ps.tile([P, 512], F32, space="PSUM", tag=f"rep{h}",
                                 bufs=1, name=f"rep_ps{h}") for h in range(2)]
            for kc in range(KPROJ):
                pwc = pwstream.tile([P, D], F32R, tag="pw")
                nc.sync.dma_start(out=pwc[:], in_=d_pw[kc])
                for h in range(2):
                    nc.tensor.matmul(rep_ps[h][:], xT[:, kc, :],
                                     pwc[:, h * 512:(h + 1) * 512],
                                     start=(kc == 0), stop=False)
            pbrow = ph1.tile([1, D], F32)
            nc.sync.dma_start(out=pbrow[:], in_=d_pbrow[:])
            rep_sb = ph1.tile([P, D], F32)
            for h in range(2):
                nc.tensor.matmul(rep_ps[h][:], ones_f[:],
                                 pbrow[:, h * 512:(h + 1) * 512],
                                 start=False, stop=True)
                nc.scalar.activation(rep_sb[:, h * 512:(h + 1) * 512],
                                     rep_ps[h][:], ACTF.Relu)
            # transpose rep -> repT (+ bf16 / f32r copies for downstream lhsT)
            for m in range(DC):
                tps2 = ph1ps.tile([P, P], F32, space="PSUM", tag="tp")
                nc.tensor.transpose(tps2[:], rep_sb[:, m * P:(m + 1) * P], ident[:])
                nc.scalar.copy(repT[:, m, :], tps2[:])
                nc.vector.tensor_copy(repTr[:, m, :], tps2[:])
            nc.vector.tensor_copy(repT16[:], repT[:])
            if debug:
                nc.sync.dma_start(out=o_dbg_rep[:], in_=repT[:])

            # deferred persistent-weight DMAs (scalar HWDGE queue)
            nc.scalar.dma_start(out=wu_sb[:], in_=d_wu.rearrange("c p e -> p c e"))
            nc.scalar.dma_start(out=tr1_sb[:], in_=d_tr1.rearrange("c p e -> p c e"))
            nc.scalar.dma_start(out=tr2_sb[:], in_=d_tr2.rearrange("c p e -> p c e"))
            nc.scalar.dma_start(out=outw_sb[:], in_=d_outw.rearrange("c p e -> p c e"))
            nc.scalar.dma_start(out=tr1b_sb[:], in_=d_tr1b[:])
            nc.scalar.dma_start(out=tr2b_sb[:], in_=d_tr2b[:])
            nc.scalar.dma_start(out=outb_sb[:], in_=d_outb[:])

            # S' = rep @ ws' + b  (bf16, w_w folded);  O likewise;  W3 fp32
            ws_sb = ph1.tile([P, DC, D], BF16)
            nc.scalar.dma_start(out=ws_sb[:], in_=d_ws.rearrange("c p e -> p c e"))
            wo_sb = ph1.tile([P, DC, D], BF16)
            nc.scalar.dma_start(out=wo_sb[:], in_=d_wo.rearrange("c p e -> p c e"))
            wsb = ph1.tile([1, D], BF16)
            nc.sync.dma_start(out=wsb[:], in_=d_wsb[:])
            wob = ph1.tile([1, D], BF16)
            nc.sync.dma_start(out=wob[:], in_=d_wob[:])

            for (wmat, wbias, dst) in ((ws_sb, wsb, S16), (wo_sb, wob, O16)):
                for h in range(2):
                    sps = ph1ps.tile([P, 512], F32, space="PSUM", tag="so")
                    for kc in range(DC):
                        nc.tensor.matmul(sps[:], repT16[:, kc, :],
                                         wmat[:, kc, h * 512:(h + 1) * 512],
                                         start=(kc == 0), stop=False)
                    nc.tensor.matmul(sps[:], ones16[:],
                                     wbias[:, h * 512:(h + 1) * 512],
                                     start=False, stop=True)
                    nc.vector.tensor_copy(dst[:, h * 512:(h + 1) * 512], sps[:])
            if debug:
                sdbg = ph1.tile([P, D], F32)
                nc.vector.tensor_copy(sdbg[:], S16[:])
                nc.sync.dma_start(out=o_dbg_sp[:], in_=sdbg[:])

            wt3_sb = ph1.tile([P, DC, D // 2], F32R)
            nc.scalar.dma_start(out=wt3_sb[:], in_=d_wt3.rearrange("c p e -> p c e"))
            wt3b_sb = ph1.tile([1, D // 2], F32)
            nc.sync.dma_start(out=wt3b_sb[:], in_=d_wt3b[:])
            wps = ph1ps.tile([P, 512], F32, space="PSUM", tag="so")
            for kc in range(DC):
                nc.tensor.matmul(wps[:], repTr[:, kc, :], wt3_sb[:, kc, :],
                                 start=(kc == 0), stop=False)
            nc.tensor.matmul(wps[:], ones_f[:], wt3b_sb[:], start=False, stop=True)
            nc.scalar.activation(W3[:], wps[:], ACTF.Relu)

            if with_ln_affine:
                trg_row = ph1.tile([1, D // 4], F32)
                nc.sync.dma_start(out=trg_row[:], in_=d_trg[:])
                trbe_row = ph1.tile([1, D // 4], F32)
                nc.sync.dma_start(out=trbe_row[:], in_=d_trbe[:])
                gps = ph1ps.tile([P, 512], F32, space="PSUM", tag="so")
                nc.tensor.matmul(gps[:, :D // 4], ones_f[:], trg_row[:],
                                 start=True, stop=True)
                nc.vector.tensor_copy(trg_b[:], gps[:, :D // 4])
                gps = ph1ps.tile([P, 512], F32, space="PSUM", tag="so")
                nc.tensor.matmul(gps[:, :D // 4], ones_f[:], trbe_row[:],
                                 start=True, stop=True)
                nc.vector.tensor_copy(trbe_b[:], gps[:, :D // 4])

            if with_bias_path:
                # T[i,j] = sum_d (S'*wu_b)[i,d] * O[j,d] needs transposed copies
                wub = ph1.tile([1, D], BF16)
                nc.sync.dma_start(out=wub[:], in_=d_wub[:])
                wub_b = ph1.tile([P, D], BF16)
                bps2 = ph1ps.tile([P, 512], F32, space="PSUM", tag="so")
                for h in range(2):
                    nc.tensor.matmul(bps2[:], ones16[:], wub[:, h * 512:(h + 1) * 512],
                                     start=True, stop=True)
                    nc.vector.tensor_copy(wub_b[:, h * 512:(h + 1) * 512], bps2[:])
                s2 = ph1.tile([P, D], BF16)
                nc.vector.tensor_tensor(s2[:], S16[:], wub_b[:], op=ALU.mult)
                for c in range(DC):
                    tps = ph1ps.tile([P, P], BF16, space="PSUM", tag="tp")
                    nc.tensor.transpose(tps[:], s2[:, c * P:(c + 1) * P], ident16[:])
                    nc.vector.tensor_copy(ST2[:, c, :], tps[:])
                    tps = ph1ps.tile([P, P], BF16, space="PSUM", tag="tp")
                    nc.tensor.transpose(tps[:], O16[:, c * P:(c + 1) * P], ident16[:])
                    nc.vector.tensor_copy(OT2[:, c, :], tps[:])

        # =================================================================
        # PHASE 2: pair loop — gathers, g = Q @ wu^T, coeff, scatter into A
        # =================================================================
        # A and Count in separate PSUM banks (start=True clears a whole bank)
        apool = ctx.enter_context(tc.tile_pool(name="apool", bufs=1, space="PSUM"))
        a_acc = apool.tile([P, P], F32, space="PSUM")
        cnt_acc = (apool.tile([P, P], F32, space="PSUM", name="cnt_acc")
                   if with_bias_path else None)

        with tc.tile_pool(name="p2", bufs=2) as p2, \
             tc.tile_pool(name="p2ps", bufs=2, space="PSUM") as p2ps:

            for g in range(NGROUPS):
                union_sb = p2.tile([P, GC, D], BF16, tag="union")
                nc.sync.dma_start(
                    out=union_sb[:],
                    in_=d_union[g * GC:(g + 1) * GC].rearrange("c p e -> p c e"))

                # one-hot selectors for this group's 512 pairs.
                # P0T[i,r] = (p0[r] == i): gpsimd broadcasts the index row
                # across partitions, DVE compares vs the partition-iota.
                p0t = p2.tile([P, GC * P], BF16, tag="p0t")
                p1t = p2.tile([P, GC * P], BF16, tag="p1t")
                pmats = []  # per chunk: (p0m, p1m) [r,i] one-hots
                for j, pt in ((0, p0t), (1, p1t)):
                    bc = p2ps.tile([P, GC * P], F32, space="PSUM", tag="bc",
                                   bufs=1, name="bc")
                    nc.tensor.matmul(
                        bc[:], ones_f[:],
                        idxrows[0:1, j * RCHUNKS + g * GC:
                                j * RCHUNKS + (g + 1) * GC, :],
                        start=True, stop=True)
                    nc.vector.tensor_tensor(
                        pt[:], bc[:], iotap_f[:].to_broadcast([P, GC * P]),
                        op=ALU.is_equal)
                for cc in range(GC):
                    ch = g * GC + cc
                    p0col = idx_f[:, ch, 0:1]
                    p1col = idx_f[:, ch, 1:2]
                    # one-hots for all GC chunks of a group stay live until the
                    # group's scatters: bufs must cover 2 groups' worth
                    p0m = p2.tile([P, P], BF16, tag="p0m", bufs=2 * GC + 2)
                    nc.vector.tensor_single_scalar(p0m[:], iota16[:], p0col,
                                                   op=ALU.is_equal)
                    p1m = p2.tile([P, P], BF16, tag="p1m", bufs=2 * GC + 2)
                    nc.vector.tensor_single_scalar(p1m[:], iota16[:], p1col,
                                                   op=ALU.is_equal)
                    pmats.append((p0m, p1m))

                # gathers: sT/oT [d-chunk, 512] then QT = sT*oT (bf16).
                # DVE can read only one PSUM operand; bounce sT via ACT copy.
                qt = p2.tile([P, DC, GC * P], FP8 if use_fp8 else BF16, tag="qt")
                for dcc in range(DC):
                    sps = p2ps.tile([P, 512], F32, space="PSUM", tag="big")
                    nc.tensor.matmul(sps[:], S16[:, dcc * P:(dcc + 1) * P], p0t[:],
                                     start=True, stop=True)
                    ops_ = p2ps.tile([P, 512], F32, space="PSUM", tag="big")
                    nc.tensor.matmul(ops_[:], O16[:, dcc * P:(dcc + 1) * P], p1t[:],
                                     start=True, stop=True)
                    # ACT bounces both PSUM banks to SBUF (GpSimd can't read
                    # PSUM); the idle GpSimd engine does the q = s*o multiply
                    s_sb = p2.tile([P, 512], BF16, tag="s_sb")
                    nc.scalar.copy(s_sb[:], sps[:])
                    o_sb = p2.tile([P, 512], BF16, tag="o_sb")
                    nc.scalar.copy(o_sb[:], ops_[:])
                    nc.gpsimd.tensor_tensor(qt[:, dcc, :], s_sb[:], o_sb[:],
                                            op=ALU.mult)

                # g = Q @ wu^T per 128-pair chunk; coeff = rowsum(union * g)
                for cc in range(GC):
                    ch = g * GC + cc
                    gps = p2ps.tile([P, D], F32, space="PSUM", tag="g", bufs=2)
                    if use_fp8:
                        DR = mybir.MatmulPerfMode.DoubleRow
                        for t in range(DC // 2):
                            lhs3 = qt[:, 2 * t:2 * t + 2, cc * P:(cc + 1) * P]
                            nc.tensor.matmul(
                                gps[:, 0:512], lhs3,
                                wu_sb[:, 2 * t:2 * t + 2, 0:512],
                                perf_mode=DR,
                                start=(t == 0), stop=(t == DC // 2 - 1))
                            nc.tensor.matmul(
                                gps[:, 512:1024], lhs3,
                                wu_sb[:, 2 * t:2 * t + 2, 512:1024],
                                perf_mode=DR,
                                start=(t == 0), stop=(t == DC // 2 - 1))
                    else:
                        for dcc in range(DC):
                            lhs = qt[:, dcc, cc * P:(cc + 1) * P]
                            nc.tensor.matmul(gps[:, 0:512], lhs,
                                             wu_sb[:, dcc, 0:512],
                                             start=(dcc == 0),
                                             stop=(dcc == DC - 1))
                            nc.tensor.matmul(gps[:, 512:1024], lhs,
                                             wu_sb[:, dcc, 512:1024],
                                             start=(dcc == 0),
                                             stop=(dcc == DC - 1))
                    # fused mul + row-sum via TensorScalarPtr accum_out
                    # (tensor_tensor_reduce is an ISA op this walrus rejects)
                    coeff = p2.tile([P, 1], F32, tag="coeff")
                    scr = p2.tile([P, D], F32, tag="scr")
                    nc.vector.scalar_tensor_tensor(
                        out=scr[:], in0=union_sb[:, cc, :],
                        scalar=(1.0 / 4096.0 if use_fp8 else 1.0),
                        in1=gps[:], op0=(ALU.mult if use_fp8 else ALU.bypass),
                        op1=ALU.mult, accum_out=coeff[:])
                    if debug:
                        nc.vector.tensor_copy(coeff_dbg[:, ch:ch + 1], coeff[:])
                        if ch == RCHUNKS - 1:
                            nc.sync.dma_start(out=o_dbg_coeff[:], in_=coeff_dbg[:])

                    # scatter: A += (P0*coeff)^T @ P1  (and Count += P0^T @ P1)
                    p0m, p1m = pmats[cc]
                    p0c = p2.tile([P, P], BF16, tag="p0c")
                    nc.vector.tensor_tensor(p0c[:], p0m[:],
                                            coeff[:].to_broadcast([P, P]),
                                            op=ALU.mult)
                    first = (ch == 0)
                    last = (ch == RCHUNKS - 1)
                    nc.tensor.matmul(a_acc[:], p0c[:], p1m[:],
                                     start=first, stop=last)
                    if with_bias_path:
                        nc.tensor.matmul(cnt_acc[:], p0m[:], p1m[:],
                                         start=first, stop=last)

        # =================================================================
        # PHASE 3: A -> ctx -> nb -> ctx_rep -> scores/preds
        # =================================================================
        with tc.tile_pool(name="p3", bufs=1) as p3, \
             tc.tile_pool(name="p3ps", bufs=2, space="PSUM") as p3ps:

            if with_bias_path:
                # T^T[j,i] = sum_d O^T[d,j] (S'wu_b)^T[d,i]; A += Count*(T + w_b)
                tt_ps = p3ps.tile([P, P], F32, space="PSUM", tag="mm")
                for dcc in range(DC):
                    nc.tensor.matmul(tt_ps[:], OT2[:, dcc, :], ST2[:, dcc, :],
                                     start=(dcc == 0), stop=(dcc == DC - 1))
                ttsb = p3.tile([P, P], F32)
                if use_fp8:
                    nc.scalar.mul(ttsb[:], tt_ps[:], 1.0 / 256.0)
                else:
                    nc.vector.tensor_copy(ttsb[:], tt_ps[:])
                t_ps = p3ps.tile([P, P], F32, space="PSUM", tag="mm")
                nc.tensor.transpose(t_ps[:], ttsb[:], ident[:])
                wbsc = p3.tile([1, 1], F32)
                nc.sync.dma_start(out=wbsc[:], in_=d_wb[:])
                wbcol = p3.tile([P, 1], F32)
                bb = p3ps.tile([P, P], F32, space="PSUM", tag="mm2")
                nc.tensor.matmul(bb[:, 0:1], ones_f[:], wbsc[:], start=True, stop=True)
                nc.vector.tensor_copy(wbcol[:], bb[:, 0:1])
                tpw = p3.tile([P, P], F32)
                nc.vector.tensor_scalar_add(tpw[:], t_ps[:], wbcol[:])
                cnt_term = p3.tile([P, P], F32)
                nc.vector.tensor_tensor(cnt_term[:], cnt_acc[:], tpw[:],
                                        op=ALU.mult)
                apre = p3.tile([P, P], F32)
                nc.vector.tensor_tensor(apre[:], a_acc[:], cnt_term[:],
                                        op=ALU.add)
                asig = p3.tile([P, P], F32)
                nc.scalar.activation(asig[:], apre[:], ACTF.Sigmoid)
            else:
                asig = p3.tile([P, P], F32)
                nc.scalar.activation(asig[:], a_acc[:], ACTF.Sigmoid)

            # zero diagonal, row-normalize
            nc.gpsimd.affine_select(out=asig[:], in_=asig[:],
                                    compare_op=ALU.not_equal, fill=0.0,
                                    base=0, pattern=[[-1, P]], channel_multiplier=1)
            rs = p3.tile([P, 1], F32)
            nc.vector.reduce_sum(rs[:], asig[:], axis=mybir.AxisListType.X)
            rsr = p3.tile([P, 1], F32)
            nc.vector.reciprocal(rsr[:], rs[:])
            an = p3.tile([P, P], F32)
            nc.vector.tensor_scalar_mul(an[:], asig[:], rsr[:])
            if debug:
                nc.sync.dma_start(out=o_dbg_an[:], in_=an[:])
            antp = p3ps.tile([P, P], F32, space="PSUM", tag="mm")
            nc.tensor.transpose(antp[:], an[:], ident[:])
            anT = p3.tile([P, P], F32)
            nc.vector.tensor_copy(anT[:], antp[:])

            # ctx^T [d2, i]: first 512 rows from (An@W3)^T, last 512 from (An^T@W3)^T
            ctxT = p3.tile([P, DC, P], F32R)
            for m4 in range(4):
                cps = p3ps.tile([P, P], F32, space="PSUM", tag="mm")
                nc.tensor.matmul(cps[:], W3[:, m4 * P:(m4 + 1) * P], anT[:],
                                 start=True, stop=True)
                nc.vector.tensor_copy(ctxT[:, m4, :], cps[:])
            for m4 in range(4):
                cps = p3ps.tile([P, P], F32, space="PSUM", tag="mm")
                nc.tensor.matmul(cps[:], W3[:, m4 * P:(m4 + 1) * P], an[:],
                                 start=True, stop=True)
                nc.vector.tensor_copy(ctxT[:, 4 + m4, :], cps[:])

            # t = ctx @ tr1 + b; LayerNorm; relu; transpose
            tps_ = p3ps.tile([P, 256], F32, space="PSUM", tag="mm")
            for dcc in range(DC):
                nc.tensor.matmul(tps_[:], ctxT[:, dcc, :], tr1_sb[:, dcc, :],
                                 start=(dcc == 0), stop=False)
            nc.tensor.matmul(tps_[:], ones_f[:], tr1b_sb[:], start=False, stop=True)
            stats = p3.tile([P, 6], F32)
            nc.vector.bn_stats(stats[:], tps_[:])
            aggr = p3.tile([P, 2], F32)
            nc.vector.bn_aggr(aggr[:], stats[:])
            veps = p3.tile([P, 1], F32)
            nc.vector.tensor_scalar_add(veps[:], aggr[:, 1:2], 1e-5)
            stdv = p3.tile([P, 1], F32)
            nc.scalar.sqrt(stdv[:], veps[:])
            rstd = p3.tile([P, 1], F32)
            nc.vector.reciprocal(rstd[:], stdv[:])
            tn = p3.tile([P, 256], F32)
            nc.vector.tensor_scalar(tn[:], tps_[:], aggr[:, 0:1], rstd[:],
                                    op0=ALU.subtract, op1=ALU.mult)
            if with_ln_affine:
                nc.vector.tensor_tensor(tn[:], tn[:], trg_b[:], op=ALU.mult)
                nc.vector.tensor_tensor(tn[:], tn[:], trbe_b[:], op=ALU.add)
            trl = p3.tile([P, 256], F32)
            nc.scalar.activation(trl[:], tn[:], ACTF.Relu)
            rT = p3.tile([P, 2, P], F32)
            for kc in range(2):
                tp2 = p3ps.tile([P, P], F32, space="PSUM", tag="mm")
                nc.tensor.transpose(tp2[:], trl[:, kc * P:(kc + 1) * P], ident[:])
                nc.vector.tensor_copy(rT[:, kc, :], tp2[:])

            # nb^T per d-chunk; ctx_rep^T = relu(rep^T + nb^T + b2)
            ctxrT = p3.tile([P, DC, P], F32)
            for dcc in range(DC):
                nps = p3ps.tile([P, P], F32, space="PSUM", tag="mm")
                for kc in range(2):
                    nc.tensor.matmul(nps[:], tr2_sb[:, kc, dcc * P:(dcc + 1) * P],
                                     rT[:, kc, :], start=(kc == 0), stop=(kc == 1))
                tmp = p3.tile([P, P], F32, tag="nbtmp")
                nc.vector.scalar_tensor_tensor(
                    out=tmp[:], in0=nps[:], scalar=tr2b_sb[:, dcc:dcc + 1],
                    in1=repT[:, dcc, :], op0=ALU.add, op1=ALU.add)
                nc.scalar.activation(ctxrT[:, dcc, :], tmp[:], ACTF.Relu)

            # ctx_rep output (transpose back)
            ctx_sb = p3.tile([P, D], F32)
            for dcc in range(DC):
                cps2 = p3ps.tile([P, P], F32, space="PSUM", tag="mm")
                nc.tensor.transpose(cps2[:], ctxrT[:, dcc, :], ident[:])
                nc.vector.tensor_copy(ctx_sb[:, dcc * P:(dcc + 1) * P], cps2[:])
            nc.sync.dma_start(out=o_ctx[:], in_=ctx_sb[:])

            # scores^T then scores
            scT = p3.tile([P, 2, P], F32)
            for mc in range(2):
                mwid = P if mc == 0 else C - P
                sps2 = p3ps.tile([P, P], F32, space="PSUM", tag="mm")
                for dcc in range(DC):
                    nc.tensor.matmul(sps2[:mwid, :],
                                     outw_sb[:, dcc, mc * P:mc * P + mwid],
                                     ctxrT[:, dcc, :],
                                     start=(dcc == 0), stop=(dcc == DC - 1))
                nc.scalar.activation(scT[:mwid, mc, :], sps2[:mwid, :],
                                     ACTF.Identity, bias=outb_sb[:mwid, mc:mc + 1])
            scores_sb = p3.tile([P, C], F32)
            sps3 = p3ps.tile([P, P], F32, space="PSUM", tag="mm")
            nc.tensor.transpose(sps3[:], scT[:, 0, :], ident[:])
            nc.vector.tensor_copy(scores_sb[:, 0:P], sps3[:])
            sps3 = p3ps.tile([P, P], F32, space="PSUM", tag="mm")
            nc.tensor.transpose(sps3[:, 0:C - P], scT[:C - P, 1, :],
                                ident[:C - P, 0:C - P])
            nc.vector.tensor_copy(scores_sb[:, P:C], sps3[:, 0:C - P])
            nc.sync.dma_start(out=o_scores[:], in_=scores_sb[:])

            # preds = argmax(scores[:,1:]) + 1
            mx8 = p3.tile([P, 8], F32)
            nc.vector.max(mx8[:], scores_sb[:, 1:C])
            mi8 = p3.tile([P, 8], U32)
            nc.vector.max_index(mi8[:], mx8[:], scores_sb[:, 1:C])
            predf = p3.tile([P, 1], I32)
            nc.vector.tensor_single_scalar(predf[:], mi8[:, 0:1], 1, op=ALU.add)
            nc.sync.dma_start(out=o_preds[:], in_=predf[:])

    nsplit = _split_multi_waits(nc)
    if nsplit:
        print(f"[kernel] split {nsplit} extra sync-waits into EventSemaphores")
    return nc


# ---------------------------------------------------------------------------
# Host-side input prep + execution
# ---------------------------------------------------------------------------

_cache = {}


def _prep_weights(inputs, use_fp8):
    f32 = np.float32
    bf16 = ml_dtypes.bfloat16
    fp8 = ml_dtypes.float8_e4m3
    sc = np.float32(16.0) if use_fp8 else np.float32(1.0)
    w = np.asarray(inputs["w_w"], f32)[:, 0]

    alpha = (np.asarray(inputs["pe_g"], f32) / np.sqrt(np.float32(1.0 + 1e-5)))
    pe_w1f = np.asarray(inputs["pe_w1"], f32) * alpha[None, :]
    pe_b1col = (np.asarray(inputs["pe_b1"], f32) * alpha
                + np.asarray(inputs["pe_be"], f32))[:, None].copy()

    proj_w = np.asarray(inputs["proj_w"], f32)
    pw = np.zeros((KPROJ, P, D), f32)
    pw.reshape(KPROJ * P, D)[0:D] = proj_w[0:D]
    pw.reshape(KPROJ * P, D)[D:D + P] = proj_w[D:D + P]
    pw.reshape(KPROJ * P, D)[9 * P + 0: 9 * P + (E_EMB - P)] = proj_w[D + P:D + E_EMB]
    pw.reshape(KPROJ * P, D)[10 * P:11 * P] = proj_w[D + E_EMB:]

    emb = np.asarray(inputs["embed_w"], f32)
    emb_p = np.zeros((2, P, E_EMB), f32)
    emb_p[0] = emb[0:P]
    emb_p[1, :C - P] = emb[P:C]

    ws16 = (np.asarray(inputs["ws_w"], f32) * w[None, :] * sc).astype(bf16)
    ws16 = ws16.reshape(DC, P, D)
    wsb16 = (np.asarray(inputs["ws_b"], f32) * w * sc).astype(bf16)[None, :]
    wo16 = (np.asarray(inputs["wo_w"], f32) * sc).astype(bf16).reshape(DC, P, D)
    wob16 = (np.asarray(inputs["wo_b"], f32) * sc).astype(bf16)[None, :]
    wuT = np.ascontiguousarray(np.asarray(inputs["wu_w"], f32).T) * sc
    wu16 = wuT.astype(fp8 if use_fp8 else bf16).reshape(DC, P, D)

    out_b = np.asarray(inputs["out_b"], f32)
    outbcol = np.zeros((P, 2), f32)
    outbcol[:, 0] = out_b[0:P]
    outbcol[:C - P, 1] = out_b[P:C]

    return {
        "pw": pw,
        "proj_bcol": np.ascontiguousarray(
            np.asarray(inputs["proj_b"], f32).reshape(DC, P).T),
        "proj_brow": np.asarray(inputs["proj_b"], f32)[None, :],
        "embed_w_p": emb_p,
        "pe_w1f": pe_w1f,
        "pe_b1col": pe_b1col,
        "pe_w2": np.asarray(inputs["pe_w2"], f32),
        "pe_b2col": np.asarray(inputs["pe_b2"], f32)[:, None].copy(),
        "ws16": ws16, "wsb16": wsb16,
        "wo16": wo16, "wob16": wob16,
        "wu16": wu16,
        "wt3": np.asarray(inputs["wt3_w"], f32).reshape(DC, P, D // 2),
        "wt3b": np.asarray(inputs["wt3_b"], f32)[None, :],
        "tr1": np.asarray(inputs["tr_w1"], f32).reshape(DC, P, D // 4),
        "tr1b": np.asarray(inputs["tr_b1"], f32)[None, :],
        "trg": np.asarray(inputs["tr_g"], f32)[None, :],
        "trbe": np.asarray(inputs["tr_be"], f32)[None, :],
        "tr2": np.asarray(inputs["tr_w2"], f32).reshape(2, P, D),
        "tr2bcol": np.ascontiguousarray(
            np.asarray(inputs["tr_b2"], f32).reshape(DC, P).T),
        "outw": np.asarray(inputs["out_w"], f32).reshape(DC, P, C),
        "outbcol": outbcol,
    }


def kernel(**inputs):
    global last_exec_time_ns, last_trace_path

    f32 = np.float32
    bf16 = ml_dtypes.bfloat16

    wu_b = np.asarray(inputs["wu_b"], f32)
    w_b = np.asarray(inputs["w_b"], f32)
    with_bias_path = bool(np.any(wu_b != 0) or np.any(w_b != 0))
    with_ln_affine = bool(
        np.any(np.asarray(inputs["tr_g"], f32) != 1.0)
        or np.any(np.asarray(inputs["tr_be"], f32) != 0.0))
    debug = bool(int(os.environ.get("DAMP_DEBUG", "0")))
    trace = bool(int(os.environ.get("DAMP_TRACE", "0")))
    use_fp8 = bool(int(os.environ.get("DAMP_FP8", "1")))

    key = (with_bias_path, with_ln_affine, debug, use_fp8)
    if key not in _cache:
        _cache[key] = _build_program(with_bias_path, with_ln_affine, debug,
                                     use_fp8=use_fp8)
    nc = _cache[key]

    weights = _prep_weights(inputs, use_fp8)
    if with_bias_path:
        weights["wub16"] = wu_b.astype(bf16)[None, :]
        weights["w_b_scalar"] = w_b.reshape(1, 1)

    obj_feats = np.asarray(inputs["obj_feats"], f32)
    obj_dists = np.asarray(inputs["obj_dists"], f32)
    box_info = np.asarray(inputs["box_info"], f32)
    rel_idx = np.asarray(inputs["rel_pair_idx"], np.int32)
    union = np.asarray(inputs["union_feats"], f32)

    in_maps = []
    for b in range(B):
        m = dict(weights)
        m["obj_feats"] = obj_feats[b]
        m["obj_dists"] = obj_dists[b]
        m["box_info"] = box_info[b]
        m["rel_idx"] = np.ascontiguousarray(rel_idx[b].reshape(RCHUNKS, P, 2))
        m["union16"] = np.ascontiguousarray(
            union[b].astype(bf16).reshape(RCHUNKS, P, D))
        in_maps.append(m)

    res = run_bass_kernel_spmd(nc, in_maps, core_ids=list(range(B)), trace=trace)
    global _last_res
    _last_res = res
    last_exec_time_ns = res.exec_time_ns
    if res.instructions_and_trace is not None:
        last_trace_path = res.instructions_and_trace[1]

    scores = np.concatenate([res.results[b]["scores"] for b in range(B)], 0)
    preds = np.concatenate(
        [res.results[b]["preds"][:, 0] for b in range(B)], 0).astype(np.int32)
    ctx_rep = np.concatenate([res.results[b]["ctx_rep"] for b in range(B)], 0)
    return scores, preds, ctx_rep
